# revision 1
# baseline (speedup 1.0000x reference)
"""Causal self-attention (RoPE, 16 heads) on 8 TRN2 NeuronCores.

Problem: x[4,2048,2048] @ Wqkv -> RoPE(q,k) -> causal softmax(qk^T/sqrt(128)) @ v
         -> out proj Wout.  B=4, S=2048, D=2048, H=16, DH=128.

Sharding: tensor-parallel over heads. Each of the 8 cores computes 2 heads:
QKV projection columns for its heads, RoPE, attention, and its partial of the
output projection (row-sharded Wout). Host sums the 8 partials (+bout).

Design (vs the 903us v1 two-phase fp32r baseline; ~666us fast-mode):
  * bf16 operands everywhere (fp32 PSUM accumulation) — same PE rate as
    float32r, but cheaper weight loads, 2x DVE elementwise, half the DMA.
  * Fully fused per-batch pipeline: QKV+RoPE -> attention -> out-proj with
    Q^T/K^T/V/O^T resident in SBUF (no DRAM scratch round trip, no phase
    barrier, no per-head reload stalls).
  * Causal trimming: for the diagonal 128-k chunk dg, the moving q-range
    starts at dg*128 (N in {512,384,256,128}); only the leading 128-wide
    diagonal block needs the 0/1 mask (applied in place on DVE).
  * Attention latency chain (st -> exp on ACT -> av) hidden by interleaving
    both heads' chains per ki step and emitting the out-projection row-tiles
    of query tile qi right after qi completes (PE filler work); each batch's
    last out-proj group is deferred past the next batch's first QKV tile so
    the boundary always has ready PE work.
  * Softmax denominator via ones-matmul into PSUM; non-diagonal exp chunks
    are pre-summed in groups of four on DVE (bf16) and diagonal chunks in
    overlapping pairs (dg1 into dg0's q-range in place, dg3 into dg2's), so
    ~1/4 as many ones-matmuls stream through the PE.
  * reciprocal_approx_fast for 1/l (~5x faster than DVE reciprocal);
    PSUM->SBUF y copies on DVE as fp32->bf16 casts (keeps the in-order ACT
    queue free for exp); y partials in bf16, summed on host in fp64.
  * Startup: first x tile + first weight quarter interleaved in small DMAs
    ahead of all bulk loads (first matmul at ~14us instead of ~38us).
PSUM budget (8 banks): mm(2, shared QKV-acc/out-proj) + st(2) +
  av_h0/av_h1/lps_h0/lps_h1 (1 each).
"""

import math

import numpy as np


def _ensure_imports():
    try:
        import concourse.bass  # noqa: F401
    except ImportError:
        import sys
        for p in (
            "/root/.axon_site",
            "/root/.axon_site/_ro/trn_rl_repo",
            "/root/.axon_site/_ro/pypackages",
            "/opt/trn_rl_repo",
        ):
            if p not in sys.path:
                sys.path.append(p)


DH = 128
TOK = 512            # token tile (matmul moving free dim)
SHUF_MASK = [(i + 16) % 32 for i in range(32)]


def _perm_orig_of_p():
    """orig head-dim index stored at partition p, for the RoPE layout.

    Partition p = 32*quad + j. Rotation pair index i = 16*quad + (j % 16).
    j < 16 holds the even element (2i), j >= 16 holds the odd (2i+1).
    """
    orig = np.empty(DH, dtype=np.int64)
    for p in range(DH):
        quad, j = divmod(p, 32)
        i = 16 * quad + (j % 16)
        orig[p] = 2 * i if j < 16 else 2 * i + 1
    return orig


def _build_program(B, S, D, HPC):
    """Build the per-core SPMD program. Returns compiled Bacc."""
    import concourse.mybir as mybir
    import concourse.tile as tile
    from concourse import bacc
    from contextlib import ExitStack

    F32 = mybir.dt.float32
    BF16 = mybir.dt.bfloat16
    AF = mybir.ActivationFunctionType
    OP = mybir.AluOpType

    T = B * S
    NKO = D // 128           # contraction chunks for projections
    QCOLS = 2 * HPC          # q + k col-tiles of 128
    VCOLS = HPC * 128
    WCOLS = QCOLS * 128 + VCOLS
    NQI = S // TOK           # q tiles per (b,h)
    NDC = TOK // 128         # 128-chunks per token tile (diag masks)
    NDO = D // TOK           # output Dout tiles
    NKV = S // 128           # v chunks per batch
    scale = 1.0 / math.sqrt(DH)

    nc = bacc.Bacc()
    xT = nc.dram_tensor("xT", [D, T], BF16, kind="ExternalInput")
    w_c = nc.dram_tensor("w_c", [D, WCOLS], BF16, kind="ExternalInput")
    wout = nc.dram_tensor("wout", [VCOLS, D], BF16, kind="ExternalInput")
    cosP = nc.dram_tensor("cosP", [128, S], BF16, kind="ExternalInput")
    sinP = nc.dram_tensor("sinP", [128, S], BF16, kind="ExternalInput")
    maskT = nc.dram_tensor("maskT", [128, NDC, TOK], BF16, kind="ExternalInput")
    ones = nc.dram_tensor("ones", [128, 128], BF16, kind="ExternalInput")
    qb = nc.dram_tensor("qb", [128, QCOLS], F32, kind="ExternalInput")
    vb = nc.dram_tensor("vb", [128, VCOLS], F32, kind="ExternalInput")
    y = nc.dram_tensor("y", [T, D], BF16, kind="ExternalOutput")

    xTr = xT.rearrange("(ko p) t -> p ko t", p=128)
    w_r = w_c.rearrange("(ko p) c -> p ko c", p=128)
    wout_r = wout.rearrange("(h p) d -> p h d", p=128)

    with tile.TileContext(nc) as tc:
        with ExitStack() as ctx:
            s1 = ctx.enter_context(tc.tile_pool(name="singles", bufs=1))
            xp = ctx.enter_context(tc.tile_pool(name="xp", bufs=2))
            qkvp = ctx.enter_context(tc.tile_pool(name="qkvp", bufs=2))
            wk = ctx.enter_context(tc.tile_pool(name="wk", bufs=2))
            ptp = ctx.enter_context(tc.tile_pool(name="ptp", bufs=4))
            ptq = ctx.enter_context(tc.tile_pool(name="ptq", bufs=10))
            ysp = ctx.enter_context(tc.tile_pool(name="ysp", bufs=6))
            psA = ctx.enter_context(
                tc.tile_pool(name="psA", bufs=2, space="PSUM"))
            psB = ctx.enter_context(
                tc.tile_pool(name="psB", bufs=2, space="PSUM"))
            psC = ctx.enter_context(
                tc.tile_pool(name="psC", bufs=1, space="PSUM"))

            # ---- resident tensors -------------------------------------
            # Issue order matters for startup latency: the first x tile and
            # the first weight quarter go first so the opening matmul group
            # isn't queued behind bulk loads.
            xt00 = xp.tile([128, NKO, TOK], BF16, tag="xt")
            w_sb = s1.tile([128, NKO, WCOLS], BF16)
            ck = NKO // 8
            # interleave the first x tile and the weights at eighth
            # granularity so the opening 16-ko matmul group never outruns
            # the weight chunks still in flight
            for i in range(8):
                nc.sync.dma_start(out=xt00[:, i * ck:(i + 1) * ck, :],
                                  in_=xTr[:, i * ck:(i + 1) * ck, 0:TOK])
                nc.sync.dma_start(out=w_sb[:, i * ck:(i + 1) * ck, :],
                                  in_=w_r[:, i * ck:(i + 1) * ck, :])
            qb_sb = s1.tile([128, QCOLS], F32)
            vb_sb = s1.tile([128, VCOLS], F32)
            nc.sync.dma_start(out=qb_sb, in_=qb[:, :])
            nc.sync.dma_start(out=vb_sb, in_=vb[:, :])
            cos_sb = s1.tile([128, S], BF16)
            sin_sb = s1.tile([128, S], BF16)
            nc.sync.dma_start(out=cos_sb, in_=cosP[:, :])
            nc.sync.dma_start(out=sin_sb, in_=sinP[:, :])
            # allocated now, DMA'd after the first xt tile (see loop)
            wout_sb = s1.tile([128, HPC, D], BF16)
            mask_sb = s1.tile([128, NDC, TOK], BF16)
            ones_sb = s1.tile([128, 128], BF16)

            def emit_c(b0_, ot_, qi_, use_act=False):
                # out-projection row-tiles for query tile qi_ of batch at b0_
                for qs in range(qi_ * NDC, (qi_ + 1) * NDC):
                    ysb = ysp.tile([128, D], BF16, tag="ysb",
                                   name=f"ysb_{b0_}_{qs}")
                    for do in range(NDO):
                        yp = psA.tile([128, TOK], F32, tag="mm",
                                      name=f"yp_{b0_}_{qs}_{do}")
                        for h in range(HPC):
                            nc.tensor.matmul(
                                yp, ot_[:, h, qs * 128:(qs + 1) * 128],
                                wout_sb[:, h, do * TOK:(do + 1) * TOK],
                                start=(h == 0), stop=(h == HPC - 1))
                        if use_act:
                            # batch-boundary group: ACT is idle here and the
                            # DVE queue is backed up behind the recip chain —
                            # drain the mm slots fast so the next batch's
                            # QKV accumulations aren't blocked
                            nc.scalar.activation(
                                ysb[:, do * TOK:(do + 1) * TOK], yp, AF.Copy)
                        else:
                            nc.vector.tensor_copy(
                                ysb[:, do * TOK:(do + 1) * TOK], yp)
                    nc.sync.dma_start(
                        out=y[b0_ + qs * 128:b0_ + (qs + 1) * 128, :],
                        in_=ysb)

            pending_c = None
            for b in range(B):
                b0 = b * S
                qt = qkvp.tile([128, HPC, S], BF16, tag="qt")
                kt = qkvp.tile([128, HPC, S], BF16, tag="kt")
                vt = qkvp.tile([128, NKV, VCOLS], BF16, tag="vt")
                ot = qkvp.tile([128, HPC, S], BF16, tag="ot")
                for t in range(NQI):
                    # ---- A(t): QKV projection + RoPE for token tile t ----
                    tg = b0 + t * TOK
                    if b == 0 and t == 0:
                        xt = xt00  # prefetched before the resident loads
                        # bulk B/C-phase inputs: issued behind the first xt
                        nc.sync.dma_start(out=mask_sb,
                                          in_=maskT.rearrange("p n s -> p n s"))
                        nc.sync.dma_start(out=ones_sb, in_=ones[:, :])
                        for h in range(HPC):
                            nc.sync.dma_start(out=wout_sb[:, h, :],
                                              in_=wout_r[:, h, :])
                    else:
                        xt = xp.tile([128, NKO, TOK], BF16, tag="xt")
                        for i in range(2):
                            hk = NKO // 2
                            nc.sync.dma_start(
                                out=xt[:, i * hk:(i + 1) * hk, :],
                                in_=xTr[:, i * hk:(i + 1) * hk, tg:tg + TOK])
                    for c4 in range(QCOLS):
                        acc = psA.tile([128, TOK], F32, tag="mm")
                        for ko in range(NKO):
                            nc.tensor.matmul(
                                acc, w_sb[:, ko, c4 * 128:(c4 + 1) * 128],
                                xt[:, ko, :],
                                start=(ko == 0), stop=(ko == NKO - 1))
                        raw = wk.tile([128, TOK], BF16, tag="raw")
                        nc.scalar.activation(raw, acc, AF.Identity,
                                             bias=qb_sb[:, c4:c4 + 1])
                        sw = wk.tile([128, TOK], BF16, tag="sw")
                        # partition-only permute: bitcast to u32 halves the
                        # streamed element count (pairs along free dim)
                        nc.vector.stream_shuffle(
                            sw.bitcast(mybir.dt.uint32),
                            raw.bitcast(mybir.dt.uint32), SHUF_MASK)
                        m1 = wk.tile([128, TOK], BF16, tag="m1")
                        nc.vector.tensor_tensor(
                            m1, raw, cos_sb[:, t * TOK:(t + 1) * TOK],
                            op=OP.mult)
                        m2 = wk.tile([128, TOK], BF16, tag="m2")
                        nc.vector.tensor_tensor(
                            m2, sw, sin_sb[:, t * TOK:(t + 1) * TOK],
                            op=OP.mult)
                        dst = qt if c4 < HPC else kt
                        nc.vector.tensor_tensor(
                            dst[:, c4 % HPC, t * TOK:(t + 1) * TOK], m1, m2,
                            op=OP.add)
                    for sub in range(NDC):
                        accv = psA.tile([128, VCOLS], F32, tag="mm")
                        for ko in range(NKO):
                            nc.tensor.matmul(
                                accv, xt[:, ko, sub * 128:(sub + 1) * 128],
                                w_sb[:, ko, QCOLS * 128:WCOLS],
                                start=(ko == 0), stop=(ko == NKO - 1))
                        nc.vector.tensor_tensor(
                            vt[:, t * NDC + sub, :], accv, vb_sb, op=OP.add)

                    if t == 0 and pending_c is not None:
                        # previous batch's deferred last out-projection:
                        # emitted after this batch's first QKV tile so the
                        # batch boundary has ready PE work on both sides
                        emit_c(*pending_c)
                        pending_c = None

                # ---- B: attention per query tile (heads interleaved),
                # ---- each followed by its out-projection row-tiles (C)
                for qi in range(NQI):
                    q0 = qi * TOK
                    nki = NDC * qi + NDC
                    avs, lpss = [], []
                    for h in range(HPC):
                        av_h = psC.tile([128, TOK], F32, tag=f"av{h}",
                                        name=f"av{h}_{b}_{qi}")
                        lps_h = psC.tile([128, TOK], F32, tag=f"lps{h}",
                                         name=f"lps{h}_{b}_{qi}")
                        avs.append(av_h)
                        lpss.append(lps_h)
                    pend = [[], []]          # ungrouped non-diag pt, per head
                    pend_d = [None, None]    # unpaired diag pt, per head
                    lps_open = [False] * HPC
                    for ki in range(nki):
                        dg = ki - NDC * qi
                        qoff = max(dg, 0) * 128
                        N = TOK - qoff
                        pts = []
                        for h in range(HPC):
                            # both heads' score matmuls + exps issued first so
                            # ACT gets the pair ASAP and each head's PV work
                            # overlaps the other head's exp
                            st = psB.tile([128, TOK], F32, tag="st")
                            nc.tensor.matmul(
                                st[:, :N], kt[:, h, ki * 128:(ki + 1) * 128],
                                qt[:, h, q0 + qoff:q0 + TOK],
                                start=True, stop=True)
                            pt = ptq.tile([128, TOK], BF16, tag="pt",
                                          name=f"pt_{b}_{qi}_{ki}_{h}")
                            nc.scalar.activation(pt[:, :N], st[:, :N], AF.Exp,
                                                 scale=scale)
                            if dg >= 0:
                                # after trimming, only the leading 128-wide
                                # diagonal block is partially masked; zero it
                                # in place and feed pt to the PV matmuls
                                nc.vector.tensor_tensor(
                                    pt[:, 0:128], pt[:, 0:128],
                                    mask_sb[:, dg, qoff:qoff + 128],
                                    op=OP.mult)
                            pts.append(pt)
                        for h in range(HPC):
                            pt = pts[h]
                            nc.tensor.matmul(
                                avs[h][:, qoff:TOK],
                                vt[:, ki, h * 128:(h + 1) * 128], pt[:, :N],
                                start=(ki == 0), stop=(ki == nki - 1))
                            # softmax denominator: pre-sum groups of four
                            # non-diag exp chunks on DVE (bf16) so only one
                            # ones-matmul streams per quad (non-diag count
                            # per qi is 4*qi — always a multiple of 4);
                            # diag chunks pair (dg0+dg1, dg2+dg3) by adding
                            # the later chunk into the earlier one's
                            # overlapping q-range in place
                            if dg < 0:
                                pend[h].append(pt)
                                if len(pend[h]) < 4:
                                    continue
                                p0, p1, p2, p3 = pend[h]
                                pend[h] = []
                                pa = ptp.tile([128, TOK], BF16, tag="ppa",
                                              name=f"pa_{b}_{qi}_{ki}_{h}")
                                nc.vector.tensor_tensor(pa, p0, p1, op=OP.add)
                                pb = ptp.tile([128, TOK], BF16, tag="ppb",
                                              name=f"pb_{b}_{qi}_{ki}_{h}")
                                nc.vector.tensor_tensor(pb, p2, p3, op=OP.add)
                                pp = ptp.tile([128, TOK], BF16, tag="pp",
                                              name=f"pp_{b}_{qi}_{ki}_{h}")
                                nc.vector.tensor_tensor(pp, pa, pb, op=OP.add)
                                nc.tensor.matmul(
                                    lpss[h][:, qoff:TOK], ones_sb, pp[:, :N],
                                    start=(not lps_open[h]), stop=False)
                                lps_open[h] = True
                            elif dg in (0, 2):
                                pend_d[h] = pt
                            else:
                                base = pend_d[h]
                                pend_d[h] = None
                                # base covers q-local [qoff-128, TOK); this
                                # chunk covers [qoff, TOK) = base cols 128:
                                nc.vector.tensor_tensor(
                                    base[:, 128:128 + N], base[:, 128:128 + N],
                                    pt[:, :N], op=OP.add)
                                nc.tensor.matmul(
                                    lpss[h][:, qoff - 128:TOK], ones_sb,
                                    base[:, :N + 128],
                                    start=(not lps_open[h]),
                                    stop=(ki == nki - 1))
                                lps_open[h] = True
                    for h in range(HPC):
                        recl = wk.tile([128, TOK], F32, tag="recl")
                        nc.vector.reciprocal_approx_fast(recl, lpss[h])
                        nc.vector.tensor_tensor(
                            ot[:, h, q0:q0 + TOK], avs[h], recl, op=OP.mult)
                    if qi < NQI - 1 or b == B - 1:
                        emit_c(b0, ot, qi)
                    else:
                        pending_c = (b0, ot, qi)

            if pending_c is not None:
                emit_c(*pending_c)

    nc.compile()
    return nc


def _host_prep(x, rope_cos, rope_sin, Wqkv, bqkv, Wout, B, S, D, H, n_cores):
    """Build per-core input maps (bf16 data, fp32 biases)."""
    import ml_dtypes
    BF = ml_dtypes.bfloat16

    T = B * S
    HPC = H // n_cores
    orig = _perm_orig_of_p()
    quad_j = np.arange(DH)
    jmod = quad_j % 32
    i_of_p = (quad_j // 32) * 16 + (jmod % 16)
    sign = np.where(jmod < 16, -1.0, 1.0).astype(np.float32)

    xT = np.ascontiguousarray(x.reshape(T, D).T.astype(BF))  # [D, T]
    cosP = np.ascontiguousarray(rope_cos[:, i_of_p].T.astype(BF))
    sinP = np.ascontiguousarray((rope_sin[:, i_of_p] * sign).T.astype(BF))

    NDC = TOK // 128
    pl = np.arange(128)[:, None]
    ql = np.arange(TOK)[None, :]
    maskT = np.stack([(d * 128 + pl <= ql) for d in range(NDC)], axis=1)
    maskT = np.ascontiguousarray(maskT.astype(BF))  # [128, NDC, TOK]

    ones = np.ones((128, 128), dtype=BF)

    in_maps = []
    for c in range(n_cores):
        heads = [c * HPC + i for i in range(HPC)]
        wq = [Wqkv[:, h * DH + orig] for h in heads]
        wk = [Wqkv[:, H * DH + h * DH + orig] for h in heads]
        wv = [Wqkv[:, 2 * H * DH + h * DH:2 * H * DH + (h + 1) * DH]
              for h in heads]
        w_c = np.ascontiguousarray(
            np.concatenate(wq + wk + wv, axis=1).astype(BF))
        wout_c = np.ascontiguousarray(
            Wout[c * HPC * DH:(c + 1) * HPC * DH, :].astype(BF))
        qb_cols = ([bqkv[h * DH + orig] for h in heads] +
                   [bqkv[H * DH + h * DH + orig] for h in heads])
        qb = np.ascontiguousarray(np.stack(qb_cols, axis=1).astype(np.float32))
        vb_flat = np.concatenate(
            [bqkv[2 * H * DH + h * DH:2 * H * DH + (h + 1) * DH]
             for h in heads])
        vb = np.ascontiguousarray(
            np.broadcast_to(vb_flat[None, :], (128, HPC * DH)).astype(
                np.float32))
        in_maps.append({
            "xT": xT, "w_c": w_c, "wout": wout_c, "cosP": cosP, "sinP": sinP,
            "maskT": maskT, "ones": ones, "qb": qb, "vb": vb,
        })
    return in_maps


def _run(x, rope_cos, rope_sin, Wqkv, bqkv, Wout, bout,
         B, S, D, H, n_cores, trace=False):
    _ensure_imports()
    from concourse.bass_utils import run_bass_kernel_spmd

    HPC = H // n_cores
    import time as _time
    _t0 = _time.time()
    nc = _build_program(B, S, D, HPC)
    print(f"[kernel] build+compile wall: {_time.time() - _t0:.1f}s", flush=True)
    in_maps = _host_prep(np.asarray(x, dtype=np.float32),
                         np.asarray(rope_cos, dtype=np.float32),
                         np.asarray(rope_sin, dtype=np.float32),
                         np.asarray(Wqkv, dtype=np.float32),
                         np.asarray(bqkv, dtype=np.float32),
                         np.asarray(Wout, dtype=np.float32),
                         B, S, D, H, n_cores)
    _t0 = _time.time()
    res = run_bass_kernel_spmd(nc, in_maps, list(range(n_cores)), trace=trace)
    print(f"[kernel] spmd run wall: {_time.time() - _t0:.1f}s", flush=True)
    y = res.results[0]["y"].astype(np.float64)
    for i in range(1, n_cores):
        y += res.results[i]["y"]
    y += np.asarray(bout, dtype=np.float64)[None, :]
    out = y.astype(np.float32).reshape(B, S, D)
    return out, res


def kernel(x, rope_cos, rope_sin, Wqkv, bqkv, Wout, bout):
    out, _ = _run(x, rope_cos, rope_sin, Wqkv, bqkv, Wout, bout,
                  B=4, S=2048, D=2048, H=16, n_cores=8)
    return out



# revision 9
# speedup vs baseline: 1.0063x; 1.0063x over previous
"""Causal self-attention (RoPE, 16 heads) on 8 TRN2 NeuronCores.

Problem: x[4,2048,2048] @ Wqkv -> RoPE(q,k) -> causal softmax(qk^T/sqrt(128)) @ v
         -> out proj Wout.  B=4, S=2048, D=2048, H=16, DH=128.

Sharding: tensor-parallel over heads. Each of the 8 cores computes 2 heads:
QKV projection columns for its heads, RoPE, attention, and its partial of the
output projection (row-sharded Wout). Host sums the 8 partials (+bout).

Design (vs the 903us v1 two-phase fp32r baseline; ~666us fast-mode):
  * bf16 operands everywhere (fp32 PSUM accumulation) — same PE rate as
    float32r, but cheaper weight loads, 2x DVE elementwise, half the DMA.
  * Fully fused per-batch pipeline: QKV+RoPE -> attention -> out-proj with
    Q^T/K^T/V/O^T resident in SBUF (no DRAM scratch round trip, no phase
    barrier, no per-head reload stalls).
  * Causal trimming: for the diagonal 128-k chunk dg, the moving q-range
    starts at dg*128 (N in {512,384,256,128}); only the leading 128-wide
    diagonal block needs the 0/1 mask (applied in place on DVE).
  * Attention latency chain (st -> exp on ACT -> av) hidden by interleaving
    both heads' chains per ki step and emitting the out-projection row-tiles
    of query tile qi right after qi completes (PE filler work); each batch's
    last out-proj group is deferred past the next batch's first QKV tile so
    the boundary always has ready PE work.
  * Softmax denominator via ones-matmul into PSUM; non-diagonal exp chunks
    are pre-summed in groups of four on DVE (bf16) and diagonal chunks in
    overlapping pairs (dg1 into dg0's q-range in place, dg3 into dg2's), so
    ~1/4 as many ones-matmuls stream through the PE.
  * reciprocal_approx_fast for 1/l (~5x faster than DVE reciprocal);
    PSUM->SBUF y copies on DVE as fp32->bf16 casts (keeps the in-order ACT
    queue free for exp); y partials in bf16, summed on host in fp64.
  * Startup: first x tile + first weight quarter interleaved in small DMAs
    ahead of all bulk loads (first matmul at ~14us instead of ~38us).
PSUM budget (8 banks): mm(2, shared QKV-acc/out-proj) + st(2) +
  av_h0/av_h1/lps_h0/lps_h1 (1 each).
"""

import math

import numpy as np


def _ensure_imports():
    try:
        import concourse.bass  # noqa: F401
    except ImportError:
        import sys
        for p in (
            "/root/.axon_site",
            "/root/.axon_site/_ro/trn_rl_repo",
            "/root/.axon_site/_ro/pypackages",
            "/opt/trn_rl_repo",
        ):
            if p not in sys.path:
                sys.path.append(p)


DH = 128
TOK = 512            # token tile (matmul moving free dim)
SHUF_MASK = [(i + 16) % 32 for i in range(32)]


def _perm_orig_of_p():
    """orig head-dim index stored at partition p, for the RoPE layout.

    Partition p = 32*quad + j. Rotation pair index i = 16*quad + (j % 16).
    j < 16 holds the even element (2i), j >= 16 holds the odd (2i+1).
    """
    orig = np.empty(DH, dtype=np.int64)
    for p in range(DH):
        quad, j = divmod(p, 32)
        i = 16 * quad + (j % 16)
        orig[p] = 2 * i if j < 16 else 2 * i + 1
    return orig


def _build_program(B, S, D, HPC):
    """Build the per-core SPMD program. Returns compiled Bacc."""
    import concourse.mybir as mybir
    import concourse.tile as tile
    from concourse import bacc
    from contextlib import ExitStack

    F32 = mybir.dt.float32
    BF16 = mybir.dt.bfloat16
    AF = mybir.ActivationFunctionType
    OP = mybir.AluOpType

    T = B * S
    NKO = D // 128           # contraction chunks for projections
    QCOLS = 2 * HPC          # q + k col-tiles of 128
    VCOLS = HPC * 128
    WCOLS = QCOLS * 128 + VCOLS
    NQI = S // TOK           # q tiles per (b,h)
    NDC = TOK // 128         # 128-chunks per token tile (diag masks)
    NDO = D // TOK           # output Dout tiles
    NKV = S // 128           # v chunks per batch
    scale = 1.0 / math.sqrt(DH)

    nc = bacc.Bacc()
    xT = nc.dram_tensor("xT", [D, T], BF16, kind="ExternalInput")
    w_c = nc.dram_tensor("w_c", [D, WCOLS], BF16, kind="ExternalInput")
    wout = nc.dram_tensor("wout", [VCOLS, D], BF16, kind="ExternalInput")
    cosP = nc.dram_tensor("cosP", [128, S], BF16, kind="ExternalInput")
    sinP = nc.dram_tensor("sinP", [128, S], BF16, kind="ExternalInput")
    madd = nc.dram_tensor("madd", [128, 128], BF16, kind="ExternalInput")
    ident = nc.dram_tensor("ident", [128, 128], BF16, kind="ExternalInput")
    ones = nc.dram_tensor("ones", [128, 128], BF16, kind="ExternalInput")
    qb = nc.dram_tensor("qb", [128, QCOLS], F32, kind="ExternalInput")
    vb = nc.dram_tensor("vb", [128, VCOLS], F32, kind="ExternalInput")
    y = nc.dram_tensor("y", [T, D], BF16, kind="ExternalOutput")

    xTr = xT.rearrange("(ko p) t -> p ko t", p=128)
    w_r = w_c.rearrange("(ko p) c -> p ko c", p=128)
    wout_r = wout.rearrange("(h p) d -> p h d", p=128)

    with tile.TileContext(nc) as tc:
        with ExitStack() as ctx:
            s1 = ctx.enter_context(tc.tile_pool(name="singles", bufs=1))
            xp = ctx.enter_context(tc.tile_pool(name="xp", bufs=2))
            qkvp = ctx.enter_context(tc.tile_pool(name="qkvp", bufs=2))
            wk = ctx.enter_context(tc.tile_pool(name="wk", bufs=2))
            ptp = ctx.enter_context(tc.tile_pool(name="ptp", bufs=4))
            ptq = ctx.enter_context(tc.tile_pool(name="ptq", bufs=10))
            ysp = ctx.enter_context(tc.tile_pool(name="ysp", bufs=6))
            psA = ctx.enter_context(
                tc.tile_pool(name="psA", bufs=2, space="PSUM"))
            psB = ctx.enter_context(
                tc.tile_pool(name="psB", bufs=2, space="PSUM"))
            psC = ctx.enter_context(
                tc.tile_pool(name="psC", bufs=1, space="PSUM"))

            # ---- resident tensors -------------------------------------
            # Issue order matters for startup latency: the opening QKV
            # chain (c4=0) needs only the first 128 weight columns + the
            # first x tile, so those stream first at ko granularity and
            # the first matmul starts ~1.5us in; later weight column
            # slices land just ahead of the chains that consume them.
            xt00 = xp.tile([128, NKO, TOK], BF16, tag="xt")
            w_sb = s1.tile([128, NKO, WCOLS], BF16)
            qb_sb = s1.tile([128, QCOLS], F32)
            vb_sb = s1.tile([128, VCOLS], F32)
            nc.sync.dma_start(out=qb_sb, in_=qb[:, :])
            nc.sync.dma_start(out=vb_sb, in_=vb[:, :])
            qk = NKO // 4
            for i in range(4):
                nc.sync.dma_start(out=w_sb[:, i * qk:(i + 1) * qk, 0:128],
                                  in_=w_r[:, i * qk:(i + 1) * qk, 0:128])
            ck = NKO // 8
            for i in range(8):
                nc.sync.dma_start(out=xt00[:, i * ck:(i + 1) * ck, :],
                                  in_=xTr[:, i * ck:(i + 1) * ck, 0:TOK])
            for i in range(4):
                nc.sync.dma_start(out=w_sb[:, i * qk:(i + 1) * qk, 128:256],
                                  in_=w_r[:, i * qk:(i + 1) * qk, 128:256])
            cos_sb = s1.tile([128, S], BF16)
            sin_sb = s1.tile([128, S], BF16)
            nc.sync.dma_start(out=cos_sb, in_=cosP[:, :])
            nc.sync.dma_start(out=sin_sb, in_=sinP[:, :])
            for i in range(2):
                hk = NKO // 2
                nc.sync.dma_start(out=w_sb[:, i * hk:(i + 1) * hk, 256:512],
                                  in_=w_r[:, i * hk:(i + 1) * hk, 256:512])
                nc.sync.dma_start(out=w_sb[:, i * hk:(i + 1) * hk, 512:WCOLS],
                                  in_=w_r[:, i * hk:(i + 1) * hk, 512:WCOLS])
            # allocated now, DMA'd after the first xt tile (see loop)
            wout_sb = s1.tile([128, HPC, D], BF16)
            madd_sb = s1.tile([128, 128], BF16)
            ident_sb = s1.tile([128, 128], BF16)
            ones_sb = s1.tile([128, 128], BF16)

            def emit_qs(b0_, ot_, qs, use_act=False, tail=False):
                # out-projection row-tile for 128-query chunk qs
                ysb = ysp.tile([128, D], BF16, tag="ysb",
                               name=f"ysb_{b0_}_{qs}")
                for do in range(NDO):
                    yp = psA.tile([128, TOK], F32, tag="mm",
                                  name=f"yp_{b0_}_{qs}_{do}")
                    for h in range(HPC):
                        nc.tensor.matmul(
                            yp, ot_[:, h, qs * 128:(qs + 1) * 128],
                            wout_sb[:, h, do * TOK:(do + 1) * TOK],
                            start=(h == 0), stop=(h == HPC - 1))
                    if use_act:
                        # batch-boundary / tail group: ACT is idle here and
                        # the DVE queue is backed up behind the recip chain —
                        # drain the mm slots fast
                        nc.scalar.activation(
                            ysb[:, do * TOK:(do + 1) * TOK], yp, AF.Copy)
                    else:
                        nc.vector.tensor_copy(
                            ysb[:, do * TOK:(do + 1) * TOK], yp)
                    if tail and do % 2 == 1:
                        # drain the final row-tiles in halves so the last
                        # DMA is small and overlaps the remaining copies
                        nc.sync.dma_start(
                            out=y[b0_ + qs * 128:b0_ + (qs + 1) * 128,
                                  (do - 1) * TOK:(do + 1) * TOK],
                            in_=ysb[:, (do - 1) * TOK:(do + 1) * TOK])
                if not tail:
                    nc.sync.dma_start(
                        out=y[b0_ + qs * 128:b0_ + (qs + 1) * 128, :],
                        in_=ysb)

            def emit_c(b0_, ot_, qi_, use_act=False):
                for qs in range(qi_ * NDC, (qi_ + 1) * NDC):
                    emit_qs(b0_, ot_, qs, use_act=use_act)

            pending_c = None
            xt_pre = None
            for b in range(B):
                b0 = b * S
                qt = qkvp.tile([128, HPC, S], BF16, tag="qt")
                kt = qkvp.tile([128, HPC, S], BF16, tag="kt")
                vt = qkvp.tile([128, NKV, VCOLS], BF16, tag="vt")
                ot = qkvp.tile([128, HPC, S], BF16, tag="ot")
                for t in range(NQI):
                    # ---- A(t): QKV projection + RoPE for token tile t ----
                    tg = b0 + t * TOK
                    if b == 0 and t == 0:
                        xt = xt00  # prefetched before the resident loads
                        # bulk B/C-phase inputs: issued behind the first xt
                        nc.sync.dma_start(out=madd_sb, in_=madd[:, :])
                        nc.sync.dma_start(out=ident_sb, in_=ident[:, :])
                        nc.sync.dma_start(out=ones_sb, in_=ones[:, :])
                        nc.sync.dma_start(out=wout_sb, in_=wout_r)
                    elif xt_pre is not None:
                        xt = xt_pre  # prefetched during previous B-phase
                        xt_pre = None
                    else:
                        xt = xp.tile([128, NKO, TOK], BF16, tag="xt")
                        for i in range(2):
                            hk = NKO // 2
                            nc.sync.dma_start(
                                out=xt[:, i * hk:(i + 1) * hk, :],
                                in_=xTr[:, i * hk:(i + 1) * hk, tg:tg + TOK])
                    for c4 in range(QCOLS):
                        acc = psA.tile([128, TOK], F32, tag="mm")
                        for ko in range(NKO):
                            nc.tensor.matmul(
                                acc, w_sb[:, ko, c4 * 128:(c4 + 1) * 128],
                                xt[:, ko, :],
                                start=(ko == 0), stop=(ko == NKO - 1))
                        raw = wk.tile([128, TOK], BF16, tag="raw")
                        nc.scalar.activation(raw, acc, AF.Identity,
                                             bias=qb_sb[:, c4:c4 + 1])
                        sw = wk.tile([128, TOK], BF16, tag="sw")
                        # partition-only permute: bitcast to u32 halves the
                        # streamed element count (pairs along free dim)
                        nc.vector.stream_shuffle(
                            sw.bitcast(mybir.dt.uint32),
                            raw.bitcast(mybir.dt.uint32), SHUF_MASK)
                        m1 = wk.tile([128, TOK], BF16, tag="m1")
                        nc.vector.tensor_tensor(
                            m1, raw, cos_sb[:, t * TOK:(t + 1) * TOK],
                            op=OP.mult)
                        m2 = wk.tile([128, TOK], BF16, tag="m2")
                        nc.vector.tensor_tensor(
                            m2, sw, sin_sb[:, t * TOK:(t + 1) * TOK],
                            op=OP.mult)
                        dst = qt if c4 < HPC else kt
                        nc.vector.tensor_tensor(
                            dst[:, c4 % HPC, t * TOK:(t + 1) * TOK], m1, m2,
                            op=OP.add)
                    for sub in range(NDC):
                        accv = psA.tile([128, VCOLS], F32, tag="mm")
                        for ko in range(NKO):
                            nc.tensor.matmul(
                                accv, xt[:, ko, sub * 128:(sub + 1) * 128],
                                w_sb[:, ko, QCOLS * 128:WCOLS],
                                start=(ko == 0), stop=(ko == NKO - 1))
                        nc.vector.tensor_tensor(
                            vt[:, t * NDC + sub, :], accv, vb_sb, op=OP.add)

                    if t == 0 and pending_c is not None:
                        # previous batch's deferred last out-projection:
                        # emitted after this batch's first QKV tile so the
                        # batch boundary has ready PE work on both sides
                        emit_c(*pending_c)
                        pending_c = None

                # ---- B: attention per query tile (heads interleaved),
                # ---- each followed by its out-projection row-tiles (C)
                for qi in range(NQI):
                    q0 = qi * TOK
                    nki = NDC * qi + NDC
                    avs, lpss = [], []
                    for h in range(HPC):
                        av_h = psC.tile([128, TOK], F32, tag=f"av{h}",
                                        name=f"av{h}_{b}_{qi}")
                        lps_h = psC.tile([128, TOK], F32, tag=f"lps{h}",
                                         name=f"lps{h}_{b}_{qi}")
                        avs.append(av_h)
                        lpss.append(lps_h)
                    pend = [[], []]          # ungrouped non-diag pt, per head
                    pend_d = [None, None]    # unpaired diag pt, per head
                    lps_open = [False] * HPC
                    for ki in range(nki):
                        dg = ki - NDC * qi
                        qoff = max(dg, 0) * 128
                        N = TOK - qoff
                        pts = []
                        for h in range(HPC):
                            # both heads' score matmuls + exps issued first so
                            # ACT gets the pair ASAP and each head's PV work
                            # overlaps the other head's exp
                            st = psB.tile([128, TOK], F32, tag="st")
                            if dg >= 0:
                                # causal mask folded into the score psum: an
                                # identity-stationary matmul adds -1e9 above
                                # the diagonal of the leading 128-block, so
                                # exp emits exact zeros there and the DVE
                                # mask multiply disappears from the st->av
                                # chain
                                nc.tensor.matmul(
                                    st[:, :N],
                                    kt[:, h, ki * 128:(ki + 1) * 128],
                                    qt[:, h, q0 + qoff:q0 + TOK],
                                    start=True, stop=False)
                                nc.tensor.matmul(
                                    st[:, 0:128], ident_sb, madd_sb,
                                    start=False, stop=True)
                            else:
                                nc.tensor.matmul(
                                    st[:, :N],
                                    kt[:, h, ki * 128:(ki + 1) * 128],
                                    qt[:, h, q0 + qoff:q0 + TOK],
                                    start=True, stop=True)
                            pt = ptq.tile([128, TOK], BF16, tag="pt",
                                          name=f"pt_{b}_{qi}_{ki}_{h}")
                            nc.scalar.activation(pt[:, :N], st[:, :N], AF.Exp,
                                                 scale=scale)
                            pts.append(pt)
                        for h in range(HPC):
                            pt = pts[h]
                            nc.tensor.matmul(
                                avs[h][:, qoff:TOK],
                                vt[:, ki, h * 128:(h + 1) * 128], pt[:, :N],
                                start=(ki == 0), stop=(ki == nki - 1))
                            # softmax denominator: pre-sum groups of four
                            # non-diag exp chunks on DVE (bf16) so only one
                            # ones-matmul streams per quad (non-diag count
                            # per qi is 4*qi — always a multiple of 4);
                            # diag chunks pair (dg0+dg1, dg2+dg3) by adding
                            # the later chunk into the earlier one's
                            # overlapping q-range in place
                            if dg < 0:
                                pend[h].append(pt)
                                if len(pend[h]) < 4:
                                    continue
                                p0, p1, p2, p3 = pend[h]
                                pend[h] = []
                                pa = ptp.tile([128, TOK], BF16, tag="ppa",
                                              name=f"pa_{b}_{qi}_{ki}_{h}")
                                nc.vector.tensor_tensor(pa, p0, p1, op=OP.add)
                                pb = ptp.tile([128, TOK], BF16, tag="ppb",
                                              name=f"pb_{b}_{qi}_{ki}_{h}")
                                nc.vector.tensor_tensor(pb, p2, p3, op=OP.add)
                                pp = ptp.tile([128, TOK], BF16, tag="pp",
                                              name=f"pp_{b}_{qi}_{ki}_{h}")
                                nc.vector.tensor_tensor(pp, pa, pb, op=OP.add)
                                nc.tensor.matmul(
                                    lpss[h][:, qoff:TOK], ones_sb, pp[:, :N],
                                    start=(not lps_open[h]), stop=False)
                                lps_open[h] = True
                            elif dg in (0, 2):
                                pend_d[h] = pt
                            else:
                                base = pend_d[h]
                                pend_d[h] = None
                                # base covers q-local [qoff-128, TOK); this
                                # chunk covers [qoff, TOK) = base cols 128:
                                nc.vector.tensor_tensor(
                                    base[:, 128:128 + N], base[:, 128:128 + N],
                                    pt[:, :N], op=OP.add)
                                nc.tensor.matmul(
                                    lpss[h][:, qoff - 128:TOK], ones_sb,
                                    base[:, :N + 128],
                                    start=(not lps_open[h]),
                                    stop=(ki == nki - 1))
                                lps_open[h] = True
                    if b == B - 1 and qi == NQI - 1:
                        # kernel tail: no later compute hides this chain, so
                        # chunk the reciprocal/divide per 128-query block and
                        # emit each row-tile as soon as its block is ready
                        # (ACT copies + split DMA drain)
                        for sub in range(NDC):
                            c0 = sub * 128
                            for h in range(HPC):
                                recl = wk.tile([128, 128], F32, tag="reclc")
                                nc.vector.reciprocal_approx_fast(
                                    recl, lpss[h][:, c0:c0 + 128])
                                nc.vector.tensor_tensor(
                                    ot[:, h, q0 + c0:q0 + c0 + 128],
                                    avs[h][:, c0:c0 + 128], recl, op=OP.mult)
                            emit_qs(b0, ot, qi * NDC + sub,
                                    use_act=True, tail=True)
                        continue
                    for h in range(HPC):
                        recl = wk.tile([128, TOK], F32, tag="recl")
                        nc.vector.reciprocal_approx_fast(recl, lpss[h])
                        nc.vector.tensor_tensor(
                            ot[:, h, q0:q0 + TOK], avs[h], recl, op=OP.mult)
                    if qi < NQI - 1 or b == B - 1:
                        emit_c(b0, ot, qi)
                    else:
                        pending_c = (b0, ot, qi)
                    if qi == NQI - 2 and b < B - 1:
                        # prefetch next batch's first x tile during this
                        # B-phase so the batch boundary never waits on DMA
                        xt_pre = xp.tile([128, NKO, TOK], BF16, tag="xt")
                        for i in range(2):
                            hk = NKO // 2
                            nc.sync.dma_start(
                                out=xt_pre[:, i * hk:(i + 1) * hk, :],
                                in_=xTr[:, i * hk:(i + 1) * hk,
                                        (b + 1) * S:(b + 1) * S + TOK])

            if pending_c is not None:
                emit_c(*pending_c)

    nc.compile()
    return nc


def _host_prep(x, rope_cos, rope_sin, Wqkv, bqkv, Wout, B, S, D, H, n_cores):
    """Build per-core input maps (bf16 data, fp32 biases)."""
    import ml_dtypes
    BF = ml_dtypes.bfloat16

    T = B * S
    HPC = H // n_cores
    orig = _perm_orig_of_p()
    quad_j = np.arange(DH)
    jmod = quad_j % 32
    i_of_p = (quad_j // 32) * 16 + (jmod % 16)
    sign = np.where(jmod < 16, -1.0, 1.0).astype(np.float32)

    xT = np.ascontiguousarray(x.reshape(T, D).T.astype(BF))  # [D, T]
    cosP = np.ascontiguousarray(rope_cos[:, i_of_p].T.astype(BF))
    sinP = np.ascontiguousarray((rope_sin[:, i_of_p] * sign).T.astype(BF))

    pl = np.arange(128)[:, None]
    ql = np.arange(128)[None, :]
    # additive causal mask for the 128-wide diagonal block: 0 at/below the
    # diagonal (key p <= query q), -1e9 above (exp -> exact 0)
    madd = np.ascontiguousarray(
        np.where(pl <= ql, 0.0, -1e9).astype(BF))  # [128, 128]
    ident = np.ascontiguousarray(np.eye(128, dtype=np.float32).astype(BF))

    ones = np.ones((128, 128), dtype=BF)

    in_maps = []
    for c in range(n_cores):
        heads = [c * HPC + i for i in range(HPC)]
        wq = [Wqkv[:, h * DH + orig] for h in heads]
        wk = [Wqkv[:, H * DH + h * DH + orig] for h in heads]
        wv = [Wqkv[:, 2 * H * DH + h * DH:2 * H * DH + (h + 1) * DH]
              for h in heads]
        w_c = np.ascontiguousarray(
            np.concatenate(wq + wk + wv, axis=1).astype(BF))
        wout_c = np.ascontiguousarray(
            Wout[c * HPC * DH:(c + 1) * HPC * DH, :].astype(BF))
        qb_cols = ([bqkv[h * DH + orig] for h in heads] +
                   [bqkv[H * DH + h * DH + orig] for h in heads])
        qb = np.ascontiguousarray(np.stack(qb_cols, axis=1).astype(np.float32))
        vb_flat = np.concatenate(
            [bqkv[2 * H * DH + h * DH:2 * H * DH + (h + 1) * DH]
             for h in heads])
        vb = np.ascontiguousarray(
            np.broadcast_to(vb_flat[None, :], (128, HPC * DH)).astype(
                np.float32))
        in_maps.append({
            "xT": xT, "w_c": w_c, "wout": wout_c, "cosP": cosP, "sinP": sinP,
            "madd": madd, "ident": ident, "ones": ones, "qb": qb, "vb": vb,
        })
    return in_maps


def _run(x, rope_cos, rope_sin, Wqkv, bqkv, Wout, bout,
         B, S, D, H, n_cores, trace=False):
    _ensure_imports()
    from concourse.bass_utils import run_bass_kernel_spmd

    HPC = H // n_cores
    import time as _time
    _t0 = _time.time()
    nc = _build_program(B, S, D, HPC)
    print(f"[kernel] build+compile wall: {_time.time() - _t0:.1f}s", flush=True)
    in_maps = _host_prep(np.asarray(x, dtype=np.float32),
                         np.asarray(rope_cos, dtype=np.float32),
                         np.asarray(rope_sin, dtype=np.float32),
                         np.asarray(Wqkv, dtype=np.float32),
                         np.asarray(bqkv, dtype=np.float32),
                         np.asarray(Wout, dtype=np.float32),
                         B, S, D, H, n_cores)
    _t0 = _time.time()
    res = run_bass_kernel_spmd(nc, in_maps, list(range(n_cores)), trace=trace)
    print(f"[kernel] spmd run wall: {_time.time() - _t0:.1f}s", flush=True)
    y = res.results[0]["y"].astype(np.float64)
    for i in range(1, n_cores):
        y += res.results[i]["y"]
    y += np.asarray(bout, dtype=np.float64)[None, :]
    out = y.astype(np.float32).reshape(B, S, D)
    return out, res


def kernel(x, rope_cos, rope_sin, Wqkv, bqkv, Wout, bout):
    out, _ = _run(x, rope_cos, rope_sin, Wqkv, bqkv, Wout, bout,
                  B=4, S=2048, D=2048, H=16, n_cores=8)
    return out



# revision 21
# speedup vs baseline: 1.0181x; 1.0117x over previous
"""Causal self-attention (RoPE, 16 heads) on 8 TRN2 NeuronCores.

Problem: x[4,2048,2048] @ Wqkv -> RoPE(q,k) -> causal softmax(qk^T/sqrt(128)) @ v
         -> out proj Wout.  B=4, S=2048, D=2048, H=16, DH=128.

Sharding: tensor-parallel over heads. Each of the 8 cores computes 2 heads:
QKV projection columns for its heads, RoPE, attention, and its partial of the
output projection (row-sharded Wout). Host sums the 8 partials (+bout).

Design (vs the 903us v1 two-phase fp32r baseline; ~666us fast-mode):
  * bf16 operands everywhere (fp32 PSUM accumulation) — same PE rate as
    float32r, but cheaper weight loads, 2x DVE elementwise, half the DMA.
  * Fully fused per-batch pipeline: QKV+RoPE -> attention -> out-proj with
    Q^T/K^T/V/O^T resident in SBUF (no DRAM scratch round trip, no phase
    barrier, no per-head reload stalls).
  * Causal trimming: for the diagonal 128-k chunk dg, the moving q-range
    starts at dg*128 (N in {512,384,256,128}); only the leading 128-wide
    diagonal block needs the 0/1 mask (applied in place on DVE).
  * Attention latency chain (st -> exp on ACT -> av) hidden by interleaving
    both heads' chains per ki step and emitting the out-projection row-tiles
    of query tile qi right after qi completes (PE filler work); each batch's
    last out-proj group is deferred past the next batch's first QKV tile so
    the boundary always has ready PE work.
  * Softmax denominator via ones-matmul into PSUM; non-diagonal exp chunks
    are pre-summed in groups of four on DVE (bf16) and diagonal chunks in
    overlapping pairs (dg1 into dg0's q-range in place, dg3 into dg2's), so
    ~1/4 as many ones-matmuls stream through the PE.
  * reciprocal_approx_fast for 1/l (~5x faster than DVE reciprocal);
    PSUM->SBUF y copies on DVE as fp32->bf16 casts (keeps the in-order ACT
    queue free for exp); y partials in bf16, summed on host in fp64.
  * Startup: first x tile + first weight quarter interleaved in small DMAs
    ahead of all bulk loads (first matmul at ~14us instead of ~38us).
PSUM budget (8 banks): mm(2, shared QKV-acc/out-proj) + st(2) +
  av_h0/av_h1/lps_h0/lps_h1 (1 each).
"""

import math

import numpy as np


def _ensure_imports():
    try:
        import concourse.bass  # noqa: F401
    except ImportError:
        import sys
        for p in (
            "/root/.axon_site",
            "/root/.axon_site/_ro/trn_rl_repo",
            "/root/.axon_site/_ro/pypackages",
            "/opt/trn_rl_repo",
        ):
            if p not in sys.path:
                sys.path.append(p)


DH = 128
TOK = 512            # token tile (matmul moving free dim)
SHUF_MASK = [(i + 16) % 32 for i in range(32)]


def _perm_orig_of_p():
    """orig head-dim index stored at partition p, for the RoPE layout.

    Partition p = 32*quad + j. Rotation pair index i = 16*quad + (j % 16).
    j < 16 holds the even element (2i), j >= 16 holds the odd (2i+1).
    """
    orig = np.empty(DH, dtype=np.int64)
    for p in range(DH):
        quad, j = divmod(p, 32)
        i = 16 * quad + (j % 16)
        orig[p] = 2 * i if j < 16 else 2 * i + 1
    return orig


def _build_program(B, S, D, HPC):
    """Build the per-core SPMD program. Returns compiled Bacc."""
    import concourse.mybir as mybir
    import concourse.tile as tile
    from concourse import bacc
    from contextlib import ExitStack

    F32 = mybir.dt.float32
    BF16 = mybir.dt.bfloat16
    AF = mybir.ActivationFunctionType
    OP = mybir.AluOpType

    T = B * S
    NKO = D // 128           # contraction chunks for projections
    QCOLS = 2 * HPC          # q + k col-tiles of 128
    VCOLS = HPC * 128
    WCOLS = QCOLS * 128 + VCOLS
    NQI = S // TOK           # q tiles per (b,h)
    NDC = TOK // 128         # 128-chunks per token tile (diag masks)
    NDO = D // TOK           # output Dout tiles
    NKV = S // 128           # v chunks per batch
    scale = 1.0 / math.sqrt(DH)

    CB = WCOLS // 128        # weight column blocks (q,q,k,k,v,v)

    nc = bacc.Bacc()
    xT = nc.dram_tensor("xT", [D, T], BF16, kind="ExternalInput")
    w4 = nc.dram_tensor("w4", [128, CB, NKO, 128], BF16,
                        kind="ExternalInput")
    wout = nc.dram_tensor("wout", [VCOLS, D], BF16, kind="ExternalInput")
    cosP = nc.dram_tensor("cosP", [128, S], BF16, kind="ExternalInput")
    sinP = nc.dram_tensor("sinP", [128, S], BF16, kind="ExternalInput")
    madd = nc.dram_tensor("madd", [128, 128], BF16, kind="ExternalInput")
    ident = nc.dram_tensor("ident", [128, 128], BF16, kind="ExternalInput")
    ones = nc.dram_tensor("ones", [128, 128], BF16, kind="ExternalInput")
    qb = nc.dram_tensor("qb", [128, QCOLS], F32, kind="ExternalInput")
    vb = nc.dram_tensor("vb", [128, VCOLS], F32, kind="ExternalInput")
    y = nc.dram_tensor("y", [T, D], BF16, kind="ExternalOutput")

    xTr = xT.rearrange("(ko p) t -> p ko t", p=128)
    wout_r = wout.rearrange("(h p) d -> p h d", p=128)

    with tile.TileContext(nc) as tc:
        with ExitStack() as ctx:
            s1 = ctx.enter_context(tc.tile_pool(name="singles", bufs=1))
            xp = ctx.enter_context(tc.tile_pool(name="xp", bufs=2))
            qkvp = ctx.enter_context(tc.tile_pool(name="qkvp", bufs=2))
            wk = ctx.enter_context(tc.tile_pool(name="wk", bufs=2))
            ptp = ctx.enter_context(tc.tile_pool(name="ptp", bufs=4))
            ptq = ctx.enter_context(tc.tile_pool(name="ptq", bufs=10))
            ysp = ctx.enter_context(tc.tile_pool(name="ysp", bufs=8))
            psA = ctx.enter_context(
                tc.tile_pool(name="psA", bufs=2, space="PSUM"))
            psB = ctx.enter_context(
                tc.tile_pool(name="psB", bufs=2, space="PSUM"))
            psC = ctx.enter_context(
                tc.tile_pool(name="psC", bufs=1, space="PSUM"))

            # ---- resident tensors -------------------------------------
            # Each dma_start costs ~650ns of issue time on its engine
            # queue, so startup keeps the sync queue to the critical path
            # (first weight column block + the first x tile) and routes
            # every bulk load through the scalar queue, which is idle at
            # startup.  The w4 host layout makes each column-block slice a
            # fully contiguous 4KB-per-partition transfer.
            xt00 = xp.tile([128, NKO, TOK], BF16, tag="xt")
            w_sb = s1.tile([128, CB, NKO, 128], BF16)
            qb_sb = s1.tile([128, QCOLS], F32)
            vb_sb = s1.tile([128, VCOLS], F32)
            cos_sb = s1.tile([128, S], BF16)
            sin_sb = s1.tile([128, S], BF16)
            nc.sync.dma_start(out=w_sb[:, 0], in_=w4[:, 0])
            qk = NKO // 4
            for i in range(4):
                nc.sync.dma_start(out=xt00[:, i * qk:(i + 1) * qk, :],
                                  in_=xTr[:, i * qk:(i + 1) * qk, 0:TOK])
            nc.sync.dma_start(out=w_sb[:, 1], in_=w4[:, 1])
            nc.scalar.dma_start(out=qb_sb, in_=qb[:, :])
            nc.scalar.dma_start(out=vb_sb, in_=vb[:, :])
            nc.scalar.dma_start(out=cos_sb, in_=cosP[:, :])
            nc.scalar.dma_start(out=sin_sb, in_=sinP[:, :])
            nc.scalar.dma_start(out=w_sb[:, 2:4], in_=w4[:, 2:4])
            nc.scalar.dma_start(out=w_sb[:, 4:CB], in_=w4[:, 4:CB])
            # allocated now, DMA'd after the first xt tile (see loop)
            wout_sb = s1.tile([128, HPC, D], BF16)
            madd_sb = s1.tile([128, 128], BF16)
            ident_sb = s1.tile([128, 128], BF16)
            ones_sb = s1.tile([128, 128], BF16)

            def emit_group(g):
                # one out-projection (qs, do) group: 2 accumulating matmuls
                # + a PSUM->SBUF cast on the chosen engine + the row DMA
                # after its last group.  Groups are drained one-or-two per
                # ki step of the NEXT query tile's attention loop so the
                # PE's spare time there absorbs them and the casts never
                # pace the pipeline.
                b0_, ot_, qs, do, ysb, eng = g
                yp = psA.tile([128, TOK], F32, tag="mm",
                              name=f"yp_{b0_}_{qs}_{do}")
                for h in range(HPC):
                    nc.tensor.matmul(
                        yp, ot_[:, h, qs * 128:(qs + 1) * 128],
                        wout_sb[:, h, do * TOK:(do + 1) * TOK],
                        start=(h == 0), stop=(h == HPC - 1))
                if eng == "act":
                    nc.scalar.activation(
                        ysb[:, do * TOK:(do + 1) * TOK], yp, AF.Copy)
                else:
                    nc.vector.tensor_copy(
                        ysb[:, do * TOK:(do + 1) * TOK], yp)
                if do == NDO - 1:
                    nc.sync.dma_start(
                        out=y[b0_ + qs * 128:b0_ + (qs + 1) * 128, :],
                        in_=ysb)

            def emit_qs_tail(b0_, ot_, qs):
                # kernel-tail row-tile: ACT copies (DVE is busy with the
                # reciprocal chain), DMA in halves so the drain overlaps
                ysb = ysp.tile([128, D], BF16, tag="ysb",
                               name=f"ysb_{b0_}_{qs}")
                for do in range(NDO):
                    yp = psA.tile([128, TOK], F32, tag="mm",
                                  name=f"yp_{b0_}_{qs}_{do}")
                    for h in range(HPC):
                        nc.tensor.matmul(
                            yp, ot_[:, h, qs * 128:(qs + 1) * 128],
                            wout_sb[:, h, do * TOK:(do + 1) * TOK],
                            start=(h == 0), stop=(h == HPC - 1))
                    nc.scalar.activation(
                        ysb[:, do * TOK:(do + 1) * TOK], yp, AF.Copy)
                    if do % 2 == 1:
                        nc.sync.dma_start(
                            out=y[b0_ + qs * 128:b0_ + (qs + 1) * 128,
                                  (do - 1) * TOK:(do + 1) * TOK],
                            in_=ysb[:, (do - 1) * TOK:(do + 1) * TOK])

            equeue = []
            xt_pre = None
            for b in range(B):
                b0 = b * S
                qt = qkvp.tile([128, HPC, S], BF16, tag="qt")
                kt = qkvp.tile([128, HPC, S], BF16, tag="kt")
                vt = qkvp.tile([128, NKV, VCOLS], BF16, tag="vt")
                ot = qkvp.tile([128, HPC, S], BF16, tag="ot")
                for t in range(NQI):
                    # ---- A(t): QKV projection + RoPE for token tile t ----
                    tg = b0 + t * TOK
                    if b == 0 and t == 0:
                        xt = xt00  # prefetched before the resident loads
                        # bulk B/C-phase inputs: issued behind the first xt
                        nc.scalar.dma_start(out=madd_sb, in_=madd[:, :])
                        nc.scalar.dma_start(out=ident_sb, in_=ident[:, :])
                        nc.scalar.dma_start(out=ones_sb, in_=ones[:, :])
                        nc.scalar.dma_start(out=wout_sb, in_=wout_r)
                    elif xt_pre is not None:
                        xt = xt_pre  # prefetched during previous B-phase
                        xt_pre = None
                    else:
                        xt = xp.tile([128, NKO, TOK], BF16, tag="xt")
                        for i in range(2):
                            hk = NKO // 2
                            nc.scalar.dma_start(
                                out=xt[:, i * hk:(i + 1) * hk, :],
                                in_=xTr[:, i * hk:(i + 1) * hk, tg:tg + TOK])
                    for c4 in range(QCOLS):
                        acc = psA.tile([128, TOK], F32, tag="mm")
                        for ko in range(NKO):
                            nc.tensor.matmul(
                                acc, w_sb[:, c4, ko, :],
                                xt[:, ko, :],
                                start=(ko == 0), stop=(ko == NKO - 1))
                        raw = wk.tile([128, TOK], BF16, tag="raw")
                        nc.scalar.activation(raw, acc, AF.Identity,
                                             bias=qb_sb[:, c4:c4 + 1])
                        sw = wk.tile([128, TOK], BF16, tag="sw")
                        # partition-only permute: bitcast to u32 halves the
                        # streamed element count (pairs along free dim)
                        nc.vector.stream_shuffle(
                            sw.bitcast(mybir.dt.uint32),
                            raw.bitcast(mybir.dt.uint32), SHUF_MASK)
                        m1 = wk.tile([128, TOK], BF16, tag="m1")
                        nc.vector.tensor_tensor(
                            m1, raw, cos_sb[:, t * TOK:(t + 1) * TOK],
                            op=OP.mult)
                        m2 = wk.tile([128, TOK], BF16, tag="m2")
                        nc.vector.tensor_tensor(
                            m2, sw, sin_sb[:, t * TOK:(t + 1) * TOK],
                            op=OP.mult)
                        dst = qt if c4 < HPC else kt
                        nc.vector.tensor_tensor(
                            dst[:, c4 % HPC, t * TOK:(t + 1) * TOK], m1, m2,
                            op=OP.add)
                    for sub in range(NDC):
                        accv = psA.tile([128, VCOLS], F32, tag="mm")
                        for ko in range(NKO):
                            nc.tensor.matmul(
                                accv, xt[:, ko, sub * 128:(sub + 1) * 128],
                                w_sb[:, QCOLS:CB, ko, :],
                                start=(ko == 0), stop=(ko == NKO - 1))
                        nc.vector.tensor_tensor(
                            vt[:, t * NDC + sub, :], accv, vb_sb, op=OP.add)

                # ---- B: attention per query tile (heads interleaved),
                # ---- each followed by its out-projection row-tiles (C)
                for qi in range(NQI):
                    q0 = qi * TOK
                    nki = NDC * qi + NDC
                    avs, lpss = [], []
                    for h in range(HPC):
                        av_h = psC.tile([128, TOK], F32, tag=f"av{h}",
                                        name=f"av{h}_{b}_{qi}")
                        lps_h = psC.tile([128, TOK], F32, tag=f"lps{h}",
                                         name=f"lps{h}_{b}_{qi}")
                        avs.append(av_h)
                        lpss.append(lps_h)
                    pend = [[], []]          # ungrouped non-diag pt, per head
                    pend_d = [None, None]    # unpaired diag pt, per head
                    lps_open = [False] * HPC
                    groups = equeue          # previous qi's out-proj groups
                    equeue = []
                    ndrained = 0
                    for ki in range(nki):
                        dg = ki - NDC * qi
                        qoff = max(dg, 0) * 128
                        N = TOK - qoff
                        pts = []
                        for h in range(HPC):
                            # both heads' score matmuls + exps issued first so
                            # ACT gets the pair ASAP and each head's PV work
                            # overlaps the other head's exp
                            st = psB.tile([128, TOK], F32, tag="st")
                            if dg >= 0:
                                # causal mask folded into the score psum: an
                                # identity-stationary matmul adds -1e9 above
                                # the diagonal of the leading 128-block, so
                                # exp emits exact zeros there and the DVE
                                # mask multiply disappears from the st->av
                                # chain
                                nc.tensor.matmul(
                                    st[:, :N],
                                    kt[:, h, ki * 128:(ki + 1) * 128],
                                    qt[:, h, q0 + qoff:q0 + TOK],
                                    start=True, stop=False)
                                nc.tensor.matmul(
                                    st[:, 0:128], ident_sb, madd_sb,
                                    start=False, stop=True)
                            else:
                                nc.tensor.matmul(
                                    st[:, :N],
                                    kt[:, h, ki * 128:(ki + 1) * 128],
                                    qt[:, h, q0 + qoff:q0 + TOK],
                                    start=True, stop=True)
                            pt = ptq.tile([128, TOK], BF16, tag="pt",
                                          name=f"pt_{b}_{qi}_{ki}_{h}")
                            nc.scalar.activation(pt[:, :N], st[:, :N], AF.Exp,
                                                 scale=scale)
                            pts.append(pt)
                        for h in range(HPC):
                            pt = pts[h]
                            nc.tensor.matmul(
                                avs[h][:, qoff:TOK],
                                vt[:, ki, h * 128:(h + 1) * 128], pt[:, :N],
                                start=(ki == 0), stop=(ki == nki - 1))
                            # softmax denominator: pre-sum groups of four
                            # non-diag exp chunks on DVE (bf16) so only one
                            # ones-matmul streams per quad (non-diag count
                            # per qi is 4*qi — always a multiple of 4);
                            # diag chunks pair (dg0+dg1, dg2+dg3) by adding
                            # the later chunk into the earlier one's
                            # overlapping q-range in place
                            if dg < 0:
                                pend[h].append(pt)
                                if len(pend[h]) < 4:
                                    continue
                                p0, p1, p2, p3 = pend[h]
                                pend[h] = []
                                pa = ptp.tile([128, TOK], BF16, tag="ppa",
                                              name=f"pa_{b}_{qi}_{ki}_{h}")
                                nc.vector.tensor_tensor(pa, p0, p1, op=OP.add)
                                pb = ptp.tile([128, TOK], BF16, tag="ppb",
                                              name=f"pb_{b}_{qi}_{ki}_{h}")
                                nc.vector.tensor_tensor(pb, p2, p3, op=OP.add)
                                pp = ptp.tile([128, TOK], BF16, tag="pp",
                                              name=f"pp_{b}_{qi}_{ki}_{h}")
                                nc.vector.tensor_tensor(pp, pa, pb, op=OP.add)
                                nc.tensor.matmul(
                                    lpss[h][:, qoff:TOK], ones_sb, pp[:, :N],
                                    start=(not lps_open[h]), stop=False)
                                lps_open[h] = True
                            elif dg in (0, 2):
                                pend_d[h] = pt
                            else:
                                base = pend_d[h]
                                pend_d[h] = None
                                # base covers q-local [qoff-128, TOK); this
                                # chunk covers [qoff, TOK) = base cols 128:
                                nc.vector.tensor_tensor(
                                    base[:, 128:128 + N], base[:, 128:128 + N],
                                    pt[:, :N], op=OP.add)
                                nc.tensor.matmul(
                                    lpss[h][:, qoff - 128:TOK], ones_sb,
                                    base[:, :N + 128],
                                    start=(not lps_open[h]),
                                    stop=(ki == nki - 1))
                                lps_open[h] = True
                        tgt = (len(groups) * (ki + 1)) // nki
                        while ndrained < tgt:
                            emit_group(groups[ndrained])
                            ndrained += 1
                    while ndrained < len(groups):
                        emit_group(groups[ndrained])
                        ndrained += 1
                    if b == B - 1 and qi == NQI - 1:
                        # kernel tail: no later compute hides this chain, so
                        # chunk the reciprocal/divide per 128-query block and
                        # emit each row-tile as soon as its block is ready
                        # (ACT copies + split DMA drain)
                        for sub in range(NDC):
                            c0 = sub * 128
                            for h in range(HPC):
                                recl = wk.tile([128, 128], F32, tag="reclc")
                                nc.vector.reciprocal_approx_fast(
                                    recl, lpss[h][:, c0:c0 + 128])
                                nc.vector.tensor_tensor(
                                    ot[:, h, q0 + c0:q0 + c0 + 128],
                                    avs[h][:, c0:c0 + 128], recl, op=OP.mult)
                            emit_qs_tail(b0, ot, qi * NDC + sub)
                        continue
                    for h in range(HPC):
                        recl = wk.tile([128, TOK], F32, tag="recl")
                        nc.vector.reciprocal_approx_fast(recl, lpss[h])
                        nc.vector.tensor_tensor(
                            ot[:, h, q0:q0 + TOK], avs[h], recl, op=OP.mult)
                    # enqueue this qi's out-projection groups; they drain
                    # through the next query tile's (or next batch's first)
                    # attention loop.  Cast-engine split: the drain context
                    # for qi 3 and 0 is exp-light (few ki steps / next
                    # batch's qi=0), so ACT takes more copies there.
                    d_qi = (qi + 1) % NQI
                    n_act = 2 if d_qi <= 1 else 1
                    for qs in range(qi * NDC, (qi + 1) * NDC):
                        ysb = ysp.tile([128, D], BF16, tag="ysb",
                                       name=f"ysb_{b0}_{qs}")
                        for do in range(NDO):
                            eng = "act" if do < n_act else "dve"
                            equeue.append((b0, ot, qs, do, ysb, eng))
                    if qi == NQI - 2 and b < B - 1:
                        # prefetch next batch's first x tile during this
                        # B-phase so the batch boundary never waits on DMA
                        xt_pre = xp.tile([128, NKO, TOK], BF16, tag="xt")
                        for i in range(2):
                            hk = NKO // 2
                            nc.sync.dma_start(
                                out=xt_pre[:, i * hk:(i + 1) * hk, :],
                                in_=xTr[:, i * hk:(i + 1) * hk,
                                        (b + 1) * S:(b + 1) * S + TOK])

            while equeue:
                emit_group(equeue.pop(0))

    nc.compile()
    return nc


def _host_prep(x, rope_cos, rope_sin, Wqkv, bqkv, Wout, B, S, D, H, n_cores):
    """Build per-core input maps (bf16 data, fp32 biases)."""
    import ml_dtypes
    BF = ml_dtypes.bfloat16

    T = B * S
    HPC = H // n_cores
    orig = _perm_orig_of_p()
    quad_j = np.arange(DH)
    jmod = quad_j % 32
    i_of_p = (quad_j // 32) * 16 + (jmod % 16)
    sign = np.where(jmod < 16, -1.0, 1.0).astype(np.float32)

    xT = np.ascontiguousarray(x.reshape(T, D).T.astype(BF))  # [D, T]
    cosP = np.ascontiguousarray(rope_cos[:, i_of_p].T.astype(BF))
    sinP = np.ascontiguousarray((rope_sin[:, i_of_p] * sign).T.astype(BF))

    pl = np.arange(128)[:, None]
    ql = np.arange(128)[None, :]
    # additive causal mask for the 128-wide diagonal block: 0 at/below the
    # diagonal (key p <= query q), -1e9 above (exp -> exact 0)
    madd = np.ascontiguousarray(
        np.where(pl <= ql, 0.0, -1e9).astype(BF))  # [128, 128]
    ident = np.ascontiguousarray(np.eye(128, dtype=np.float32).astype(BF))

    ones = np.ones((128, 128), dtype=BF)

    NKO = D // 128
    in_maps = []
    for c in range(n_cores):
        heads = [c * HPC + i for i in range(HPC)]
        wq = [Wqkv[:, h * DH + orig] for h in heads]
        wk = [Wqkv[:, H * DH + h * DH + orig] for h in heads]
        wv = [Wqkv[:, 2 * H * DH + h * DH:2 * H * DH + (h + 1) * DH]
              for h in heads]
        w_c = np.concatenate(wq + wk + wv, axis=1)  # [D, WCOLS]
        CB = w_c.shape[1] // 128
        # [p, cb, ko, 128]: each column-block slice is a contiguous
        # 4KB-per-partition DMA
        w4 = np.ascontiguousarray(
            w_c.reshape(NKO, 128, CB, 128).transpose(1, 2, 0, 3).astype(BF))
        wout_c = np.ascontiguousarray(
            Wout[c * HPC * DH:(c + 1) * HPC * DH, :].astype(BF))
        qb_cols = ([bqkv[h * DH + orig] for h in heads] +
                   [bqkv[H * DH + h * DH + orig] for h in heads])
        qb = np.ascontiguousarray(np.stack(qb_cols, axis=1).astype(np.float32))
        vb_flat = np.concatenate(
            [bqkv[2 * H * DH + h * DH:2 * H * DH + (h + 1) * DH]
             for h in heads])
        vb = np.ascontiguousarray(
            np.broadcast_to(vb_flat[None, :], (128, HPC * DH)).astype(
                np.float32))
        in_maps.append({
            "xT": xT, "w4": w4, "wout": wout_c, "cosP": cosP, "sinP": sinP,
            "madd": madd, "ident": ident, "ones": ones, "qb": qb, "vb": vb,
        })
    return in_maps


def _run(x, rope_cos, rope_sin, Wqkv, bqkv, Wout, bout,
         B, S, D, H, n_cores, trace=False):
    _ensure_imports()
    from concourse.bass_utils import run_bass_kernel_spmd

    HPC = H // n_cores
    import time as _time
    _t0 = _time.time()
    nc = _build_program(B, S, D, HPC)
    print(f"[kernel] build+compile wall: {_time.time() - _t0:.1f}s", flush=True)
    in_maps = _host_prep(np.asarray(x, dtype=np.float32),
                         np.asarray(rope_cos, dtype=np.float32),
                         np.asarray(rope_sin, dtype=np.float32),
                         np.asarray(Wqkv, dtype=np.float32),
                         np.asarray(bqkv, dtype=np.float32),
                         np.asarray(Wout, dtype=np.float32),
                         B, S, D, H, n_cores)
    _t0 = _time.time()
    res = run_bass_kernel_spmd(nc, in_maps, list(range(n_cores)), trace=trace)
    print(f"[kernel] spmd run wall: {_time.time() - _t0:.1f}s", flush=True)
    y = res.results[0]["y"].astype(np.float64)
    for i in range(1, n_cores):
        y += res.results[i]["y"]
    y += np.asarray(bout, dtype=np.float64)[None, :]
    out = y.astype(np.float32).reshape(B, S, D)
    return out, res


def kernel(x, rope_cos, rope_sin, Wqkv, bqkv, Wout, bout):
    out, _ = _run(x, rope_cos, rope_sin, Wqkv, bqkv, Wout, bout,
                  B=4, S=2048, D=2048, H=16, n_cores=8)
    return out



# revision 27
# speedup vs baseline: 1.0344x; 1.0160x over previous
"""Causal self-attention (RoPE, 16 heads) on 8 TRN2 NeuronCores.

Problem: x[4,2048,2048] @ Wqkv -> RoPE(q,k) -> causal softmax(qk^T/sqrt(128)) @ v
         -> out proj Wout.  B=4, S=2048, D=2048, H=16, DH=128.

Sharding: tensor-parallel over heads. Each of the 8 cores computes 2 heads:
QKV projection columns for its heads, RoPE, attention, and its partial of the
output projection (row-sharded Wout). Host sums the 8 partials (+bout).

Design (vs the 903us v1 two-phase fp32r baseline; ~666us fast-mode):
  * bf16 operands everywhere (fp32 PSUM accumulation) — same PE rate as
    float32r, but cheaper weight loads, 2x DVE elementwise, half the DMA.
  * Fully fused per-batch pipeline: QKV+RoPE -> attention -> out-proj with
    Q^T/K^T/V/O^T resident in SBUF (no DRAM scratch round trip, no phase
    barrier, no per-head reload stalls).
  * Causal trimming: for the diagonal 128-k chunk dg, the moving q-range
    starts at dg*128 (N in {512,384,256,128}); only the leading 128-wide
    diagonal block needs the 0/1 mask (applied in place on DVE).
  * Attention latency chain (st -> exp on ACT -> av) hidden by interleaving
    both heads' chains per ki step and emitting the out-projection row-tiles
    of query tile qi right after qi completes (PE filler work); each batch's
    last out-proj group is deferred past the next batch's first QKV tile so
    the boundary always has ready PE work.
  * Softmax denominator via ones-matmul into PSUM; non-diagonal exp chunks
    are pre-summed in groups of four on DVE (bf16) and diagonal chunks in
    overlapping pairs (dg1 into dg0's q-range in place, dg3 into dg2's), so
    ~1/4 as many ones-matmuls stream through the PE.
  * reciprocal_approx_fast for 1/l (~5x faster than DVE reciprocal);
    PSUM->SBUF y copies on DVE as fp32->bf16 casts (keeps the in-order ACT
    queue free for exp); y partials in bf16, summed on host in fp64.
  * Startup: first x tile + first weight quarter interleaved in small DMAs
    ahead of all bulk loads (first matmul at ~14us instead of ~38us).
PSUM budget (8 banks): mm(2, shared QKV-acc/out-proj) + st(2) +
  av_h0/av_h1/lps_h0/lps_h1 (1 each).
"""

import math

import numpy as np


def _ensure_imports():
    try:
        import concourse.bass  # noqa: F401
    except ImportError:
        import sys
        for p in (
            "/root/.axon_site",
            "/root/.axon_site/_ro/trn_rl_repo",
            "/root/.axon_site/_ro/pypackages",
            "/opt/trn_rl_repo",
        ):
            if p not in sys.path:
                sys.path.append(p)


DH = 128
TOK = 512            # token tile (matmul moving free dim)
SHUF_MASK = [(i + 16) % 32 for i in range(32)]


def _perm_orig_of_p():
    """orig head-dim index stored at partition p, for the RoPE layout.

    Partition p = 32*quad + j. Rotation pair index i = 16*quad + (j % 16).
    j < 16 holds the even element (2i), j >= 16 holds the odd (2i+1).
    """
    orig = np.empty(DH, dtype=np.int64)
    for p in range(DH):
        quad, j = divmod(p, 32)
        i = 16 * quad + (j % 16)
        orig[p] = 2 * i if j < 16 else 2 * i + 1
    return orig


def _build_program(B, S, D, HPC):
    """Build the per-core SPMD program. Returns compiled Bacc."""
    import concourse.mybir as mybir
    import concourse.tile as tile
    from concourse import bacc
    from contextlib import ExitStack

    F32 = mybir.dt.float32
    BF16 = mybir.dt.bfloat16
    AF = mybir.ActivationFunctionType
    OP = mybir.AluOpType

    T = B * S
    NKO = D // 128           # contraction chunks for projections
    QCOLS = 2 * HPC          # q + k col-tiles of 128
    VCOLS = HPC * 128
    WCOLS = QCOLS * 128 + VCOLS
    NQI = S // TOK           # q tiles per (b,h)
    NDC = TOK // 128         # 128-chunks per token tile (diag masks)
    NDO = D // TOK           # output Dout tiles
    NKV = S // 128           # v chunks per batch
    scale = 1.0 / math.sqrt(DH)

    CB = WCOLS // 128        # weight column blocks (q,q,k,k,v,v)

    nc = bacc.Bacc()
    xT = nc.dram_tensor("xT", [D, T], BF16, kind="ExternalInput")
    w4 = nc.dram_tensor("w4", [128, CB, NKO, 128], BF16,
                        kind="ExternalInput")
    wout = nc.dram_tensor("wout", [VCOLS, D], BF16, kind="ExternalInput")
    cosP = nc.dram_tensor("cosP", [128, S], BF16, kind="ExternalInput")
    sinP = nc.dram_tensor("sinP", [128, S], BF16, kind="ExternalInput")
    madd = nc.dram_tensor("madd", [128, 128], BF16, kind="ExternalInput")
    ident = nc.dram_tensor("ident", [128, 128], BF16, kind="ExternalInput")
    ones = nc.dram_tensor("ones", [128, 128], BF16, kind="ExternalInput")
    qb = nc.dram_tensor("qb", [128, QCOLS], F32, kind="ExternalInput")
    vb = nc.dram_tensor("vb", [128, VCOLS], F32, kind="ExternalInput")
    y = nc.dram_tensor("y", [T, D], BF16, kind="ExternalOutput")

    xTr = xT.rearrange("(ko p) t -> p ko t", p=128)
    wout_r = wout.rearrange("(h p) d -> p h d", p=128)

    with tile.TileContext(nc) as tc:
        with ExitStack() as ctx:
            s1 = ctx.enter_context(tc.tile_pool(name="singles", bufs=1))
            xp = ctx.enter_context(tc.tile_pool(name="xp", bufs=2))
            qkvp = ctx.enter_context(tc.tile_pool(name="qkvp", bufs=2))
            wk = ctx.enter_context(tc.tile_pool(name="wk", bufs=2))
            ptp = ctx.enter_context(tc.tile_pool(name="ptp", bufs=4))
            ptq = ctx.enter_context(tc.tile_pool(name="ptq", bufs=10))
            ysp = ctx.enter_context(tc.tile_pool(name="ysp", bufs=8))
            psA = ctx.enter_context(
                tc.tile_pool(name="psA", bufs=2, space="PSUM"))
            psB = ctx.enter_context(
                tc.tile_pool(name="psB", bufs=2, space="PSUM"))
            psC = ctx.enter_context(
                tc.tile_pool(name="psC", bufs=1, space="PSUM"))

            # ---- resident tensors -------------------------------------
            # Each dma_start costs ~650ns of issue time on its engine
            # queue, so startup keeps the sync queue to the critical path
            # (first weight column block + the first x tile) and routes
            # every bulk load through the scalar queue, which is idle at
            # startup.  The w4 host layout makes each column-block slice a
            # fully contiguous 4KB-per-partition transfer.
            xt00 = xp.tile([128, NKO, TOK], BF16, tag="xt")
            w_sb = s1.tile([128, CB, NKO, 128], BF16)
            qb_sb = s1.tile([128, QCOLS], F32)
            vb_sb = s1.tile([128, VCOLS], F32)
            cos_sb = s1.tile([128, S], BF16)
            sin_sb = s1.tile([128, S], BF16)
            hk0 = NKO // 2
            nc.sync.dma_start(out=w_sb[:, 0, 0:hk0], in_=w4[:, 0, 0:hk0])
            qk = NKO // 4
            nc.sync.dma_start(out=xt00[:, 0:qk, :], in_=xTr[:, 0:qk, 0:TOK])
            nc.sync.dma_start(out=w_sb[:, 0, hk0:NKO], in_=w4[:, 0, hk0:NKO])
            for i in range(1, 4):
                nc.sync.dma_start(out=xt00[:, i * qk:(i + 1) * qk, :],
                                  in_=xTr[:, i * qk:(i + 1) * qk, 0:TOK])
            nc.sync.dma_start(out=w_sb[:, 1], in_=w4[:, 1])
            nc.scalar.dma_start(out=qb_sb, in_=qb[:, :])
            nc.scalar.dma_start(out=vb_sb, in_=vb[:, :])
            # cos/sin and the remaining weight blocks are issued inside the
            # first QKV tile, gated behind its first bias activation on the
            # scalar queue, so their transfers don't steal HBM bandwidth
            # from the critical startup path above
            # allocated now, DMA'd after the first xt tile (see loop)
            wout_sb = s1.tile([128, HPC, D], BF16)
            madd_sb = s1.tile([128, 128], BF16)
            ident_sb = s1.tile([128, 128], BF16)
            ones_sb = s1.tile([128, 128], BF16)

            def emit_group(g):
                # one out-projection (qs, do) group: 2 accumulating matmuls
                # + a PSUM->SBUF cast on the chosen engine + the row DMA
                # after its last group.  Groups are drained one-or-two per
                # ki step of the NEXT query tile's attention loop so the
                # PE's spare time there absorbs them and the casts never
                # pace the pipeline.
                b0_, ot_, qs, do, ysb, eng = g
                yp = psA.tile([128, TOK], F32, tag="mm",
                              name=f"yp_{b0_}_{qs}_{do}")
                for h in range(HPC):
                    nc.tensor.matmul(
                        yp, ot_[:, h, qs * 128:(qs + 1) * 128],
                        wout_sb[:, h, do * TOK:(do + 1) * TOK],
                        start=(h == 0), stop=(h == HPC - 1))
                if eng == "act":
                    nc.scalar.activation(
                        ysb[:, do * TOK:(do + 1) * TOK], yp, AF.Copy)
                else:
                    nc.vector.tensor_copy(
                        ysb[:, do * TOK:(do + 1) * TOK], yp)
                if do == NDO - 1:
                    nc.sync.dma_start(
                        out=y[b0_ + qs * 128:b0_ + (qs + 1) * 128, :],
                        in_=ysb)

            def emit_qs_tail(b0_, ot_, qs):
                # kernel-tail row-tile: copies split DVE/ACT so neither
                # in-order queue paces the drain; DMA in halves so the last
                # transfer is small and overlaps the remaining copies
                ysb = ysp.tile([128, D], BF16, tag="ysb",
                               name=f"ysb_{b0_}_{qs}")
                for do in range(NDO):
                    yp = psA.tile([128, TOK], F32, tag="mm",
                                  name=f"yp_{b0_}_{qs}_{do}")
                    for h in range(HPC):
                        nc.tensor.matmul(
                            yp, ot_[:, h, qs * 128:(qs + 1) * 128],
                            wout_sb[:, h, do * TOK:(do + 1) * TOK],
                            start=(h == 0), stop=(h == HPC - 1))
                    if do == 0:
                        nc.vector.tensor_copy(
                            ysb[:, do * TOK:(do + 1) * TOK], yp)
                    else:
                        nc.scalar.activation(
                            ysb[:, do * TOK:(do + 1) * TOK], yp, AF.Copy)
                    if do % 2 == 1:
                        nc.sync.dma_start(
                            out=y[b0_ + qs * 128:b0_ + (qs + 1) * 128,
                                  (do - 1) * TOK:(do + 1) * TOK],
                            in_=ysb[:, (do - 1) * TOK:(do + 1) * TOK])

            equeue = []
            xt_pre = None
            for b in range(B):
                b0 = b * S
                qt = qkvp.tile([128, HPC, S], BF16, tag="qt")
                kt = qkvp.tile([128, HPC, S], BF16, tag="kt")
                vt = qkvp.tile([128, NKV, VCOLS], BF16, tag="vt")
                ot = qkvp.tile([128, HPC, S], BF16, tag="ot")
                for t in range(NQI):
                    # ---- A(t): QKV projection + RoPE for token tile t ----
                    tg = b0 + t * TOK
                    if b == 0 and t == 0:
                        xt = xt00  # prefetched before the resident loads
                    elif xt_pre is not None:
                        xt = xt_pre  # prefetched during previous B-phase
                        xt_pre = None
                    else:
                        xt = xp.tile([128, NKO, TOK], BF16, tag="xt")
                        for i in range(2):
                            hk = NKO // 2
                            nc.scalar.dma_start(
                                out=xt[:, i * hk:(i + 1) * hk, :],
                                in_=xTr[:, i * hk:(i + 1) * hk, tg:tg + TOK])
                    for c4 in range(QCOLS):
                        acc = psA.tile([128, TOK], F32, tag="mm")
                        for ko in range(NKO):
                            nc.tensor.matmul(
                                acc, w_sb[:, c4, ko, :],
                                xt[:, ko, :],
                                start=(ko == 0), stop=(ko == NKO - 1))
                        raw = wk.tile([128, TOK], BF16, tag="raw")
                        nc.scalar.activation(raw, acc, AF.Identity,
                                             bias=qb_sb[:, c4:c4 + 1])
                        if b == 0 and t == 0 and c4 == 0:
                            # bulk loads, gated behind the first bias so the
                            # opening x/weight streams get full bandwidth
                            nc.scalar.dma_start(out=cos_sb, in_=cosP[:, :])
                            nc.scalar.dma_start(out=sin_sb, in_=sinP[:, :])
                            nc.scalar.dma_start(out=w_sb[:, 2:4],
                                                in_=w4[:, 2:4])
                            nc.scalar.dma_start(out=w_sb[:, 4:CB],
                                                in_=w4[:, 4:CB])
                            nc.scalar.dma_start(out=madd_sb, in_=madd[:, :])
                            nc.scalar.dma_start(out=ident_sb,
                                                in_=ident[:, :])
                            nc.scalar.dma_start(out=ones_sb, in_=ones[:, :])
                            nc.scalar.dma_start(out=wout_sb, in_=wout_r)
                        sw = wk.tile([128, TOK], BF16, tag="sw")
                        # partition-only permute: bitcast to u32 halves the
                        # streamed element count (pairs along free dim)
                        nc.vector.stream_shuffle(
                            sw.bitcast(mybir.dt.uint32),
                            raw.bitcast(mybir.dt.uint32), SHUF_MASK)
                        m1 = wk.tile([128, TOK], BF16, tag="m1")
                        nc.vector.tensor_tensor(
                            m1, raw, cos_sb[:, t * TOK:(t + 1) * TOK],
                            op=OP.mult)
                        m2 = wk.tile([128, TOK], BF16, tag="m2")
                        nc.vector.tensor_tensor(
                            m2, sw, sin_sb[:, t * TOK:(t + 1) * TOK],
                            op=OP.mult)
                        dst = qt if c4 < HPC else kt
                        nc.vector.tensor_tensor(
                            dst[:, c4 % HPC, t * TOK:(t + 1) * TOK], m1, m2,
                            op=OP.add)
                    for sub in range(NDC):
                        accv = psA.tile([128, VCOLS], F32, tag="mm")
                        for ko in range(NKO):
                            nc.tensor.matmul(
                                accv, xt[:, ko, sub * 128:(sub + 1) * 128],
                                w_sb[:, QCOLS:CB, ko, :],
                                start=(ko == 0), stop=(ko == NKO - 1))
                        nc.vector.tensor_tensor(
                            vt[:, t * NDC + sub, :], accv, vb_sb, op=OP.add)

                # ---- B: attention per query tile (heads interleaved),
                # ---- each followed by its out-projection row-tiles (C)
                for qi in range(NQI):
                    q0 = qi * TOK
                    nki = NDC * qi + NDC
                    avs, lpss = [], []
                    for h in range(HPC):
                        av_h = psC.tile([128, TOK], F32, tag=f"av{h}",
                                        name=f"av{h}_{b}_{qi}")
                        lps_h = psC.tile([128, TOK], F32, tag=f"lps{h}",
                                         name=f"lps{h}_{b}_{qi}")
                        avs.append(av_h)
                        lpss.append(lps_h)
                    pend = [[], []]          # ungrouped non-diag pt, per head
                    pend_d = [None, None]    # unpaired diag pt, per head
                    lps_open = [False] * HPC
                    groups = equeue          # previous qi's out-proj groups
                    equeue = []
                    ndrained = 0
                    for ki in range(nki):
                        dg = ki - NDC * qi
                        qoff = max(dg, 0) * 128
                        N = TOK - qoff
                        pts = []
                        for h in range(HPC):
                            # both heads' score matmuls + exps issued first so
                            # ACT gets the pair ASAP and each head's PV work
                            # overlaps the other head's exp
                            st = psB.tile([128, TOK], F32, tag="st")
                            if dg >= 0:
                                # causal mask folded into the score psum: an
                                # identity-stationary matmul adds -1e9 above
                                # the diagonal of the leading 128-block, so
                                # exp emits exact zeros there and the DVE
                                # mask multiply disappears from the st->av
                                # chain
                                nc.tensor.matmul(
                                    st[:, :N],
                                    kt[:, h, ki * 128:(ki + 1) * 128],
                                    qt[:, h, q0 + qoff:q0 + TOK],
                                    start=True, stop=False)
                                nc.tensor.matmul(
                                    st[:, 0:128], ident_sb, madd_sb,
                                    start=False, stop=True)
                            else:
                                nc.tensor.matmul(
                                    st[:, :N],
                                    kt[:, h, ki * 128:(ki + 1) * 128],
                                    qt[:, h, q0 + qoff:q0 + TOK],
                                    start=True, stop=True)
                            pt = ptq.tile([128, TOK], BF16, tag="pt",
                                          name=f"pt_{b}_{qi}_{ki}_{h}")
                            nc.scalar.activation(pt[:, :N], st[:, :N], AF.Exp,
                                                 scale=scale)
                            pts.append(pt)
                        for h in range(HPC):
                            pt = pts[h]
                            nc.tensor.matmul(
                                avs[h][:, qoff:TOK],
                                vt[:, ki, h * 128:(h + 1) * 128], pt[:, :N],
                                start=(ki == 0), stop=(ki == nki - 1))
                            # softmax denominator: pre-sum groups of four
                            # non-diag exp chunks on DVE (bf16) so only one
                            # ones-matmul streams per quad (non-diag count
                            # per qi is 4*qi — always a multiple of 4);
                            # diag chunks pair (dg0+dg1, dg2+dg3) by adding
                            # the later chunk into the earlier one's
                            # overlapping q-range in place
                            if dg < 0:
                                pend[h].append(pt)
                                if len(pend[h]) < 4:
                                    continue
                                p0, p1, p2, p3 = pend[h]
                                pend[h] = []
                                pa = ptp.tile([128, TOK], BF16, tag="ppa",
                                              name=f"pa_{b}_{qi}_{ki}_{h}")
                                nc.vector.tensor_tensor(pa, p0, p1, op=OP.add)
                                pb = ptp.tile([128, TOK], BF16, tag="ppb",
                                              name=f"pb_{b}_{qi}_{ki}_{h}")
                                nc.vector.tensor_tensor(pb, p2, p3, op=OP.add)
                                pp = ptp.tile([128, TOK], BF16, tag="pp",
                                              name=f"pp_{b}_{qi}_{ki}_{h}")
                                nc.vector.tensor_tensor(pp, pa, pb, op=OP.add)
                                nc.tensor.matmul(
                                    lpss[h][:, qoff:TOK], ones_sb, pp[:, :N],
                                    start=(not lps_open[h]), stop=False)
                                lps_open[h] = True
                            elif dg in (0, 2):
                                pend_d[h] = pt
                            else:
                                base = pend_d[h]
                                pend_d[h] = None
                                # base covers q-local [qoff-128, TOK); this
                                # chunk covers [qoff, TOK) = base cols 128:
                                nc.vector.tensor_tensor(
                                    base[:, 128:128 + N], base[:, 128:128 + N],
                                    pt[:, :N], op=OP.add)
                                nc.tensor.matmul(
                                    lpss[h][:, qoff - 128:TOK], ones_sb,
                                    base[:, :N + 128],
                                    start=(not lps_open[h]),
                                    stop=(ki == nki - 1))
                                lps_open[h] = True
                        # schedule shifted one ki late: the first groups'
                        # ot chunks are still in the previous qi's DVE
                        # reciprocal chain at ki=0
                        tgt = (len(groups) * ki) // nki
                        while ndrained < tgt:
                            emit_group(groups[ndrained])
                            ndrained += 1
                    while ndrained < len(groups):
                        emit_group(groups[ndrained])
                        ndrained += 1
                    if b == B - 1 and qi == NQI - 1:
                        # kernel tail: no later compute hides this chain, so
                        # chunk the reciprocal/divide per 128-query block and
                        # emit each row-tile as soon as its block is ready
                        # (ACT copies + split DMA drain)
                        for sub in range(NDC):
                            c0 = sub * 128
                            for h in range(HPC):
                                recl = wk.tile([128, 128], F32, tag="reclc")
                                nc.vector.reciprocal_approx_fast(
                                    recl, lpss[h][:, c0:c0 + 128])
                                nc.vector.tensor_tensor(
                                    ot[:, h, q0 + c0:q0 + c0 + 128],
                                    avs[h][:, c0:c0 + 128], recl, op=OP.mult)
                            emit_qs_tail(b0, ot, qi * NDC + sub)
                        continue
                    # reciprocal/divide split so the first 128-query block's
                    # ot lands early: the first out-proj groups drained in
                    # the next qi's loop only wait ~1us, not the full chain
                    for c0, cw in ((0, 128), (128, TOK - 128)):
                        for h in range(HPC):
                            recl = wk.tile([128, cw], F32,
                                           tag=f"recl{cw}")
                            nc.vector.reciprocal_approx_fast(
                                recl, lpss[h][:, c0:c0 + cw])
                            nc.vector.tensor_tensor(
                                ot[:, h, q0 + c0:q0 + c0 + cw],
                                avs[h][:, c0:c0 + cw], recl, op=OP.mult)
                    # enqueue this qi's out-projection groups; they drain
                    # through the next query tile's (or next batch's first)
                    # attention loop.  Cast-engine split: the drain context
                    # for qi 3 and 0 is exp-light (few ki steps / next
                    # batch's qi=0), so ACT takes more copies there.
                    d_qi = (qi + 1) % NQI
                    n_act = 2 if d_qi <= 1 else 1
                    for qs in range(qi * NDC, (qi + 1) * NDC):
                        ysb = ysp.tile([128, D], BF16, tag="ysb",
                                       name=f"ysb_{b0}_{qs}")
                        for do in range(NDO):
                            eng = "act" if do < n_act else "dve"
                            equeue.append((b0, ot, qs, do, ysb, eng))
                    if qi == NQI - 2 and b < B - 1:
                        # prefetch next batch's first x tile during this
                        # B-phase so the batch boundary never waits on DMA
                        xt_pre = xp.tile([128, NKO, TOK], BF16, tag="xt")
                        for i in range(2):
                            hk = NKO // 2
                            nc.sync.dma_start(
                                out=xt_pre[:, i * hk:(i + 1) * hk, :],
                                in_=xTr[:, i * hk:(i + 1) * hk,
                                        (b + 1) * S:(b + 1) * S + TOK])

            while equeue:
                emit_group(equeue.pop(0))

    nc.compile()
    return nc


def _host_prep(x, rope_cos, rope_sin, Wqkv, bqkv, Wout, B, S, D, H, n_cores):
    """Build per-core input maps (bf16 data, fp32 biases)."""
    import ml_dtypes
    BF = ml_dtypes.bfloat16

    T = B * S
    HPC = H // n_cores
    orig = _perm_orig_of_p()
    quad_j = np.arange(DH)
    jmod = quad_j % 32
    i_of_p = (quad_j // 32) * 16 + (jmod % 16)
    sign = np.where(jmod < 16, -1.0, 1.0).astype(np.float32)

    xT = np.ascontiguousarray(x.reshape(T, D).T.astype(BF))  # [D, T]
    cosP = np.ascontiguousarray(rope_cos[:, i_of_p].T.astype(BF))
    sinP = np.ascontiguousarray((rope_sin[:, i_of_p] * sign).T.astype(BF))

    pl = np.arange(128)[:, None]
    ql = np.arange(128)[None, :]
    # additive causal mask for the 128-wide diagonal block: 0 at/below the
    # diagonal (key p <= query q), -1e9 above (exp -> exact 0)
    madd = np.ascontiguousarray(
        np.where(pl <= ql, 0.0, -1e9).astype(BF))  # [128, 128]
    ident = np.ascontiguousarray(np.eye(128, dtype=np.float32).astype(BF))

    ones = np.ones((128, 128), dtype=BF)

    NKO = D // 128
    in_maps = []
    for c in range(n_cores):
        heads = [c * HPC + i for i in range(HPC)]
        wq = [Wqkv[:, h * DH + orig] for h in heads]
        wk = [Wqkv[:, H * DH + h * DH + orig] for h in heads]
        wv = [Wqkv[:, 2 * H * DH + h * DH:2 * H * DH + (h + 1) * DH]
              for h in heads]
        w_c = np.concatenate(wq + wk + wv, axis=1)  # [D, WCOLS]
        CB = w_c.shape[1] // 128
        # [p, cb, ko, 128]: each column-block slice is a contiguous
        # 4KB-per-partition DMA
        w4 = np.ascontiguousarray(
            w_c.reshape(NKO, 128, CB, 128).transpose(1, 2, 0, 3).astype(BF))
        wout_c = np.ascontiguousarray(
            Wout[c * HPC * DH:(c + 1) * HPC * DH, :].astype(BF))
        qb_cols = ([bqkv[h * DH + orig] for h in heads] +
                   [bqkv[H * DH + h * DH + orig] for h in heads])
        qb = np.ascontiguousarray(np.stack(qb_cols, axis=1).astype(np.float32))
        vb_flat = np.concatenate(
            [bqkv[2 * H * DH + h * DH:2 * H * DH + (h + 1) * DH]
             for h in heads])
        vb = np.ascontiguousarray(
            np.broadcast_to(vb_flat[None, :], (128, HPC * DH)).astype(
                np.float32))
        in_maps.append({
            "xT": xT, "w4": w4, "wout": wout_c, "cosP": cosP, "sinP": sinP,
            "madd": madd, "ident": ident, "ones": ones, "qb": qb, "vb": vb,
        })
    return in_maps


def _run(x, rope_cos, rope_sin, Wqkv, bqkv, Wout, bout,
         B, S, D, H, n_cores, trace=False):
    _ensure_imports()
    from concourse.bass_utils import run_bass_kernel_spmd

    HPC = H // n_cores
    import time as _time
    _t0 = _time.time()
    nc = _build_program(B, S, D, HPC)
    print(f"[kernel] build+compile wall: {_time.time() - _t0:.1f}s", flush=True)
    in_maps = _host_prep(np.asarray(x, dtype=np.float32),
                         np.asarray(rope_cos, dtype=np.float32),
                         np.asarray(rope_sin, dtype=np.float32),
                         np.asarray(Wqkv, dtype=np.float32),
                         np.asarray(bqkv, dtype=np.float32),
                         np.asarray(Wout, dtype=np.float32),
                         B, S, D, H, n_cores)
    _t0 = _time.time()
    res = run_bass_kernel_spmd(nc, in_maps, list(range(n_cores)), trace=trace)
    print(f"[kernel] spmd run wall: {_time.time() - _t0:.1f}s", flush=True)
    y = res.results[0]["y"].astype(np.float64)
    for i in range(1, n_cores):
        y += res.results[i]["y"]
    y += np.asarray(bout, dtype=np.float64)[None, :]
    out = y.astype(np.float32).reshape(B, S, D)
    return out, res


def kernel(x, rope_cos, rope_sin, Wqkv, bqkv, Wout, bout):
    out, _ = _run(x, rope_cos, rope_sin, Wqkv, bqkv, Wout, bout,
                  B=4, S=2048, D=2048, H=16, n_cores=8)
    return out



# revision 29
# speedup vs baseline: 1.0376x; 1.0031x over previous
"""Causal self-attention (RoPE, 16 heads) on 8 TRN2 NeuronCores.

Problem: x[4,2048,2048] @ Wqkv -> RoPE(q,k) -> causal softmax(qk^T/sqrt(128)) @ v
         -> out proj Wout.  B=4, S=2048, D=2048, H=16, DH=128.

Sharding: tensor-parallel over heads. Each of the 8 cores computes 2 heads:
QKV projection columns for its heads, RoPE, attention, and its partial of the
output projection (row-sharded Wout). Host sums the 8 partials (+bout).

Design (vs the 903us v1 two-phase fp32r baseline; ~666us fast-mode):
  * bf16 operands everywhere (fp32 PSUM accumulation) — same PE rate as
    float32r, but cheaper weight loads, 2x DVE elementwise, half the DMA.
  * Fully fused per-batch pipeline: QKV+RoPE -> attention -> out-proj with
    Q^T/K^T/V/O^T resident in SBUF (no DRAM scratch round trip, no phase
    barrier, no per-head reload stalls).
  * Causal trimming: for the diagonal 128-k chunk dg, the moving q-range
    starts at dg*128 (N in {512,384,256,128}); only the leading 128-wide
    diagonal block needs the 0/1 mask (applied in place on DVE).
  * Attention latency chain (st -> exp on ACT -> av) hidden by interleaving
    both heads' chains per ki step and emitting the out-projection row-tiles
    of query tile qi right after qi completes (PE filler work); each batch's
    last out-proj group is deferred past the next batch's first QKV tile so
    the boundary always has ready PE work.
  * Softmax denominator via ones-matmul into PSUM; non-diagonal exp chunks
    are pre-summed in groups of four on DVE (bf16) and diagonal chunks in
    overlapping pairs (dg1 into dg0's q-range in place, dg3 into dg2's), so
    ~1/4 as many ones-matmuls stream through the PE.
  * reciprocal_approx_fast for 1/l (~5x faster than DVE reciprocal);
    PSUM->SBUF y copies on DVE as fp32->bf16 casts (keeps the in-order ACT
    queue free for exp); y partials in bf16, summed on host in fp64.
  * Startup: first x tile + first weight quarter interleaved in small DMAs
    ahead of all bulk loads (first matmul at ~14us instead of ~38us).
PSUM budget (8 banks): mm(2, shared QKV-acc/out-proj) + st(2) +
  av_h0/av_h1/lps_h0/lps_h1 (1 each).
"""

import math

import numpy as np


def _ensure_imports():
    try:
        import concourse.bass  # noqa: F401
    except ImportError:
        import sys
        for p in (
            "/root/.axon_site",
            "/root/.axon_site/_ro/trn_rl_repo",
            "/root/.axon_site/_ro/pypackages",
            "/opt/trn_rl_repo",
        ):
            if p not in sys.path:
                sys.path.append(p)


DH = 128
TOK = 512            # token tile (matmul moving free dim)
SHUF_MASK = [(i + 16) % 32 for i in range(32)]


def _perm_orig_of_p():
    """orig head-dim index stored at partition p, for the RoPE layout.

    Partition p = 32*quad + j. Rotation pair index i = 16*quad + (j % 16).
    j < 16 holds the even element (2i), j >= 16 holds the odd (2i+1).
    """
    orig = np.empty(DH, dtype=np.int64)
    for p in range(DH):
        quad, j = divmod(p, 32)
        i = 16 * quad + (j % 16)
        orig[p] = 2 * i if j < 16 else 2 * i + 1
    return orig


def _build_program(B, S, D, HPC):
    """Build the per-core SPMD program. Returns compiled Bacc."""
    import concourse.mybir as mybir
    import concourse.tile as tile
    from concourse import bacc
    from contextlib import ExitStack

    F32 = mybir.dt.float32
    BF16 = mybir.dt.bfloat16
    AF = mybir.ActivationFunctionType
    OP = mybir.AluOpType

    T = B * S
    NKO = D // 128           # contraction chunks for projections
    QCOLS = 2 * HPC          # q + k col-tiles of 128
    VCOLS = HPC * 128
    WCOLS = QCOLS * 128 + VCOLS
    NQI = S // TOK           # q tiles per (b,h)
    NDC = TOK // 128         # 128-chunks per token tile (diag masks)
    NDO = D // TOK           # output Dout tiles
    NKV = S // 128           # v chunks per batch
    scale = 1.0 / math.sqrt(DH)

    CB = WCOLS // 128        # weight column blocks (q,q,k,k,v,v)

    nc = bacc.Bacc()
    xT = nc.dram_tensor("xT", [D, T], BF16, kind="ExternalInput")
    w4 = nc.dram_tensor("w4", [128, CB, NKO, 128], BF16,
                        kind="ExternalInput")
    wout = nc.dram_tensor("wout", [VCOLS, D], BF16, kind="ExternalInput")
    cosP = nc.dram_tensor("cosP", [128, S], BF16, kind="ExternalInput")
    sinP = nc.dram_tensor("sinP", [128, S], BF16, kind="ExternalInput")
    madd = nc.dram_tensor("madd", [128, 128], BF16, kind="ExternalInput")
    ident = nc.dram_tensor("ident", [128, 128], BF16, kind="ExternalInput")
    ones = nc.dram_tensor("ones", [128, 128], BF16, kind="ExternalInput")
    qb = nc.dram_tensor("qb", [128, QCOLS], F32, kind="ExternalInput")
    vb = nc.dram_tensor("vb", [128, VCOLS], F32, kind="ExternalInput")
    y = nc.dram_tensor("y", [T, D], BF16, kind="ExternalOutput")

    xTr = xT.rearrange("(ko p) t -> p ko t", p=128)
    wout_r = wout.rearrange("(h p) d -> p h d", p=128)

    with tile.TileContext(nc) as tc:
        with ExitStack() as ctx:
            s1 = ctx.enter_context(tc.tile_pool(name="singles", bufs=1))
            xp = ctx.enter_context(tc.tile_pool(name="xp", bufs=2))
            qkvp = ctx.enter_context(tc.tile_pool(name="qkvp", bufs=2))
            wk = ctx.enter_context(tc.tile_pool(name="wk", bufs=2))
            ptp = ctx.enter_context(tc.tile_pool(name="ptp", bufs=4))
            ptq = ctx.enter_context(tc.tile_pool(name="ptq", bufs=10))
            ysp = ctx.enter_context(tc.tile_pool(name="ysp", bufs=8))
            psA = ctx.enter_context(
                tc.tile_pool(name="psA", bufs=2, space="PSUM"))
            psB = ctx.enter_context(
                tc.tile_pool(name="psB", bufs=2, space="PSUM"))
            psC = ctx.enter_context(
                tc.tile_pool(name="psC", bufs=1, space="PSUM"))

            # ---- resident tensors -------------------------------------
            # Each dma_start costs ~650ns of issue time on its engine
            # queue, so startup keeps the sync queue to the critical path
            # (first weight column block + the first x tile) and routes
            # every bulk load through the scalar queue, which is idle at
            # startup.  The w4 host layout makes each column-block slice a
            # fully contiguous 4KB-per-partition transfer.
            xt00 = xp.tile([128, NKO, TOK], BF16, tag="xt")
            w_sb = s1.tile([128, CB, NKO, 128], BF16)
            qb_sb = s1.tile([128, QCOLS], F32)
            vb_sb = s1.tile([128, VCOLS], F32)
            cos_sb = s1.tile([128, S], BF16)
            sin_sb = s1.tile([128, S], BF16)
            # All large startup transfers share the sync hw queue: per-queue
            # transfers are FIFO, so consumption order is guaranteed and the
            # bulk loads cannot steal HBM bandwidth from the critical
            # opening x/weight stream (the tile scheduler reorders
            # instructions by dependency, so cross-queue ordering tricks
            # don't hold).  Only the tiny bias/constant loads use scalar.
            hk0 = NKO // 2
            nc.sync.dma_start(out=w_sb[:, 0, 0:hk0], in_=w4[:, 0, 0:hk0])
            qk = NKO // 4
            nc.sync.dma_start(out=xt00[:, 0:qk, :], in_=xTr[:, 0:qk, 0:TOK])
            nc.sync.dma_start(out=w_sb[:, 0, hk0:NKO], in_=w4[:, 0, hk0:NKO])
            for i in range(1, 4):
                nc.sync.dma_start(out=xt00[:, i * qk:(i + 1) * qk, :],
                                  in_=xTr[:, i * qk:(i + 1) * qk, 0:TOK])
            nc.sync.dma_start(out=w_sb[:, 1], in_=w4[:, 1])
            nc.sync.dma_start(out=cos_sb, in_=cosP[:, :])
            nc.sync.dma_start(out=sin_sb, in_=sinP[:, :])
            nc.sync.dma_start(out=w_sb[:, 2:4], in_=w4[:, 2:4])
            nc.sync.dma_start(out=w_sb[:, 4:CB], in_=w4[:, 4:CB])
            nc.scalar.dma_start(out=qb_sb, in_=qb[:, :])
            nc.scalar.dma_start(out=vb_sb, in_=vb[:, :])
            # allocated now, DMA'd after the first xt tile (see loop)
            wout_sb = s1.tile([128, HPC, D], BF16)
            madd_sb = s1.tile([128, 128], BF16)
            ident_sb = s1.tile([128, 128], BF16)
            ones_sb = s1.tile([128, 128], BF16)

            def emit_group(g):
                # one out-projection (qs, do) group: 2 accumulating matmuls
                # + a PSUM->SBUF cast on the chosen engine + the row DMA
                # after its last group.  Groups are drained one-or-two per
                # ki step of the NEXT query tile's attention loop so the
                # PE's spare time there absorbs them and the casts never
                # pace the pipeline.
                b0_, ot_, qs, do, ysb, eng = g
                yp = psA.tile([128, TOK], F32, tag="mm",
                              name=f"yp_{b0_}_{qs}_{do}")
                for h in range(HPC):
                    nc.tensor.matmul(
                        yp, ot_[:, h, qs * 128:(qs + 1) * 128],
                        wout_sb[:, h, do * TOK:(do + 1) * TOK],
                        start=(h == 0), stop=(h == HPC - 1))
                if eng == "act":
                    nc.scalar.activation(
                        ysb[:, do * TOK:(do + 1) * TOK], yp, AF.Copy)
                else:
                    nc.vector.tensor_copy(
                        ysb[:, do * TOK:(do + 1) * TOK], yp)
                if do == NDO - 1:
                    nc.sync.dma_start(
                        out=y[b0_ + qs * 128:b0_ + (qs + 1) * 128, :],
                        in_=ysb)

            def emit_qs_tail(b0_, ot_, qs):
                # kernel-tail row-tile: copies split DVE/ACT so neither
                # in-order queue paces the drain; DMA in halves so the last
                # transfer is small and overlaps the remaining copies
                ysb = ysp.tile([128, D], BF16, tag="ysb",
                               name=f"ysb_{b0_}_{qs}")
                for do in range(NDO):
                    yp = psA.tile([128, TOK], F32, tag="mm",
                                  name=f"yp_{b0_}_{qs}_{do}")
                    for h in range(HPC):
                        nc.tensor.matmul(
                            yp, ot_[:, h, qs * 128:(qs + 1) * 128],
                            wout_sb[:, h, do * TOK:(do + 1) * TOK],
                            start=(h == 0), stop=(h == HPC - 1))
                    if do == 0:
                        nc.vector.tensor_copy(
                            ysb[:, do * TOK:(do + 1) * TOK], yp)
                    else:
                        nc.scalar.activation(
                            ysb[:, do * TOK:(do + 1) * TOK], yp, AF.Copy)
                    if do % 2 == 1:
                        nc.sync.dma_start(
                            out=y[b0_ + qs * 128:b0_ + (qs + 1) * 128,
                                  (do - 1) * TOK:(do + 1) * TOK],
                            in_=ysb[:, (do - 1) * TOK:(do + 1) * TOK])

            equeue = []
            xt_pre = None
            for b in range(B):
                b0 = b * S
                qt = qkvp.tile([128, HPC, S], BF16, tag="qt")
                kt = qkvp.tile([128, HPC, S], BF16, tag="kt")
                vt = qkvp.tile([128, NKV, VCOLS], BF16, tag="vt")
                ot = qkvp.tile([128, HPC, S], BF16, tag="ot")
                for t in range(NQI):
                    # ---- A(t): QKV projection + RoPE for token tile t ----
                    tg = b0 + t * TOK
                    if b == 0 and t == 0:
                        xt = xt00  # prefetched before the resident loads
                    elif xt_pre is not None:
                        xt = xt_pre  # prefetched during previous B-phase
                        xt_pre = None
                    else:
                        xt = xp.tile([128, NKO, TOK], BF16, tag="xt")
                        for i in range(2):
                            hk = NKO // 2
                            nc.scalar.dma_start(
                                out=xt[:, i * hk:(i + 1) * hk, :],
                                in_=xTr[:, i * hk:(i + 1) * hk, tg:tg + TOK])
                    for c4 in range(QCOLS):
                        acc = psA.tile([128, TOK], F32, tag="mm")
                        for ko in range(NKO):
                            nc.tensor.matmul(
                                acc, w_sb[:, c4, ko, :],
                                xt[:, ko, :],
                                start=(ko == 0), stop=(ko == NKO - 1))
                        raw = wk.tile([128, TOK], BF16, tag="raw")
                        nc.scalar.activation(raw, acc, AF.Identity,
                                             bias=qb_sb[:, c4:c4 + 1])
                        if b == 0 and t == 0 and c4 == 0:
                            # non-critical constants + wout: last in the
                            # sync queue / tiny ones on scalar
                            nc.scalar.dma_start(out=madd_sb, in_=madd[:, :])
                            nc.scalar.dma_start(out=ident_sb,
                                                in_=ident[:, :])
                            nc.scalar.dma_start(out=ones_sb, in_=ones[:, :])
                            nc.sync.dma_start(out=wout_sb, in_=wout_r)
                        sw = wk.tile([128, TOK], BF16, tag="sw")
                        # partition-only permute: bitcast to u32 halves the
                        # streamed element count (pairs along free dim)
                        nc.vector.stream_shuffle(
                            sw.bitcast(mybir.dt.uint32),
                            raw.bitcast(mybir.dt.uint32), SHUF_MASK)
                        m1 = wk.tile([128, TOK], BF16, tag="m1")
                        nc.vector.tensor_tensor(
                            m1, raw, cos_sb[:, t * TOK:(t + 1) * TOK],
                            op=OP.mult)
                        m2 = wk.tile([128, TOK], BF16, tag="m2")
                        nc.vector.tensor_tensor(
                            m2, sw, sin_sb[:, t * TOK:(t + 1) * TOK],
                            op=OP.mult)
                        dst = qt if c4 < HPC else kt
                        nc.vector.tensor_tensor(
                            dst[:, c4 % HPC, t * TOK:(t + 1) * TOK], m1, m2,
                            op=OP.add)
                    for sub in range(NDC):
                        accv = psA.tile([128, VCOLS], F32, tag="mm")
                        for ko in range(NKO):
                            nc.tensor.matmul(
                                accv, xt[:, ko, sub * 128:(sub + 1) * 128],
                                w_sb[:, QCOLS:CB, ko, :],
                                start=(ko == 0), stop=(ko == NKO - 1))
                        nc.vector.tensor_tensor(
                            vt[:, t * NDC + sub, :], accv, vb_sb, op=OP.add)

                # ---- B: attention per query tile (heads interleaved),
                # ---- each followed by its out-projection row-tiles (C)
                for qi in range(NQI):
                    q0 = qi * TOK
                    nki = NDC * qi + NDC
                    avs, lpss = [], []
                    for h in range(HPC):
                        av_h = psC.tile([128, TOK], F32, tag=f"av{h}",
                                        name=f"av{h}_{b}_{qi}")
                        lps_h = psC.tile([128, TOK], F32, tag=f"lps{h}",
                                         name=f"lps{h}_{b}_{qi}")
                        avs.append(av_h)
                        lpss.append(lps_h)
                    pend = [[], []]          # ungrouped non-diag pt, per head
                    pend_d = [None, None]    # unpaired diag pt, per head
                    lps_open = [False] * HPC
                    groups = equeue          # previous qi's out-proj groups
                    equeue = []
                    ndrained = 0
                    for ki in range(nki):
                        dg = ki - NDC * qi
                        qoff = max(dg, 0) * 128
                        N = TOK - qoff
                        pts = []
                        for h in range(HPC):
                            # both heads' score matmuls + exps issued first so
                            # ACT gets the pair ASAP and each head's PV work
                            # overlaps the other head's exp
                            st = psB.tile([128, TOK], F32, tag="st")
                            if dg >= 0:
                                # causal mask folded into the score psum: an
                                # identity-stationary matmul adds -1e9 above
                                # the diagonal of the leading 128-block, so
                                # exp emits exact zeros there and the DVE
                                # mask multiply disappears from the st->av
                                # chain
                                nc.tensor.matmul(
                                    st[:, :N],
                                    kt[:, h, ki * 128:(ki + 1) * 128],
                                    qt[:, h, q0 + qoff:q0 + TOK],
                                    start=True, stop=False)
                                nc.tensor.matmul(
                                    st[:, 0:128], ident_sb, madd_sb,
                                    start=False, stop=True)
                            else:
                                nc.tensor.matmul(
                                    st[:, :N],
                                    kt[:, h, ki * 128:(ki + 1) * 128],
                                    qt[:, h, q0 + qoff:q0 + TOK],
                                    start=True, stop=True)
                            pt = ptq.tile([128, TOK], BF16, tag="pt",
                                          name=f"pt_{b}_{qi}_{ki}_{h}")
                            nc.scalar.activation(pt[:, :N], st[:, :N], AF.Exp,
                                                 scale=scale)
                            pts.append(pt)
                        for h in range(HPC):
                            pt = pts[h]
                            nc.tensor.matmul(
                                avs[h][:, qoff:TOK],
                                vt[:, ki, h * 128:(h + 1) * 128], pt[:, :N],
                                start=(ki == 0), stop=(ki == nki - 1))
                            # softmax denominator: pre-sum groups of four
                            # non-diag exp chunks on DVE (bf16) so only one
                            # ones-matmul streams per quad (non-diag count
                            # per qi is 4*qi — always a multiple of 4);
                            # diag chunks pair (dg0+dg1, dg2+dg3) by adding
                            # the later chunk into the earlier one's
                            # overlapping q-range in place
                            if dg < 0:
                                pend[h].append(pt)
                                if len(pend[h]) < 4:
                                    continue
                                p0, p1, p2, p3 = pend[h]
                                pend[h] = []
                                pa = ptp.tile([128, TOK], BF16, tag="ppa",
                                              name=f"pa_{b}_{qi}_{ki}_{h}")
                                nc.vector.tensor_tensor(pa, p0, p1, op=OP.add)
                                pb = ptp.tile([128, TOK], BF16, tag="ppb",
                                              name=f"pb_{b}_{qi}_{ki}_{h}")
                                nc.vector.tensor_tensor(pb, p2, p3, op=OP.add)
                                pp = ptp.tile([128, TOK], BF16, tag="pp",
                                              name=f"pp_{b}_{qi}_{ki}_{h}")
                                nc.vector.tensor_tensor(pp, pa, pb, op=OP.add)
                                nc.tensor.matmul(
                                    lpss[h][:, qoff:TOK], ones_sb, pp[:, :N],
                                    start=(not lps_open[h]), stop=False)
                                lps_open[h] = True
                            elif dg in (0, 2):
                                pend_d[h] = pt
                            else:
                                base = pend_d[h]
                                pend_d[h] = None
                                # base covers q-local [qoff-128, TOK); this
                                # chunk covers [qoff, TOK) = base cols 128:
                                nc.vector.tensor_tensor(
                                    base[:, 128:128 + N], base[:, 128:128 + N],
                                    pt[:, :N], op=OP.add)
                                nc.tensor.matmul(
                                    lpss[h][:, qoff - 128:TOK], ones_sb,
                                    base[:, :N + 128],
                                    start=(not lps_open[h]),
                                    stop=(ki == nki - 1))
                                lps_open[h] = True
                        # schedule shifted one ki late: the first groups'
                        # ot chunks are still in the previous qi's DVE
                        # reciprocal chain at ki=0
                        tgt = (len(groups) * ki) // nki
                        while ndrained < tgt:
                            emit_group(groups[ndrained])
                            ndrained += 1
                    while ndrained < len(groups):
                        emit_group(groups[ndrained])
                        ndrained += 1
                    if b == B - 1 and qi == NQI - 1:
                        # kernel tail: no later compute hides this chain, so
                        # chunk the reciprocal/divide per 128-query block and
                        # emit each row-tile as soon as its block is ready
                        # (ACT copies + split DMA drain)
                        for sub in range(NDC):
                            c0 = sub * 128
                            for h in range(HPC):
                                recl = wk.tile([128, 128], F32, tag="reclc")
                                nc.vector.reciprocal_approx_fast(
                                    recl, lpss[h][:, c0:c0 + 128])
                                nc.vector.tensor_tensor(
                                    ot[:, h, q0 + c0:q0 + c0 + 128],
                                    avs[h][:, c0:c0 + 128], recl, op=OP.mult)
                            emit_qs_tail(b0, ot, qi * NDC + sub)
                        continue
                    # reciprocal/divide split so the first 128-query block's
                    # ot lands early: the first out-proj groups drained in
                    # the next qi's loop only wait ~1us, not the full chain
                    for c0, cw in ((0, 128), (128, TOK - 128)):
                        for h in range(HPC):
                            recl = wk.tile([128, cw], F32,
                                           tag=f"recl{cw}")
                            nc.vector.reciprocal_approx_fast(
                                recl, lpss[h][:, c0:c0 + cw])
                            nc.vector.tensor_tensor(
                                ot[:, h, q0 + c0:q0 + c0 + cw],
                                avs[h][:, c0:c0 + cw], recl, op=OP.mult)
                    # enqueue this qi's out-projection groups; they drain
                    # through the next query tile's (or next batch's first)
                    # attention loop.  Cast-engine split: the drain context
                    # for qi 3 and 0 is exp-light (few ki steps / next
                    # batch's qi=0), so ACT takes more copies there.
                    d_qi = (qi + 1) % NQI
                    n_act = 2 if d_qi <= 1 else 1
                    for qs in range(qi * NDC, (qi + 1) * NDC):
                        ysb = ysp.tile([128, D], BF16, tag="ysb",
                                       name=f"ysb_{b0}_{qs}")
                        for do in range(NDO):
                            eng = "act" if do < n_act else "dve"
                            equeue.append((b0, ot, qs, do, ysb, eng))
                    if qi == NQI - 2 and b < B - 1:
                        # prefetch next batch's first x tile during this
                        # B-phase so the batch boundary never waits on DMA
                        xt_pre = xp.tile([128, NKO, TOK], BF16, tag="xt")
                        for i in range(2):
                            hk = NKO // 2
                            nc.sync.dma_start(
                                out=xt_pre[:, i * hk:(i + 1) * hk, :],
                                in_=xTr[:, i * hk:(i + 1) * hk,
                                        (b + 1) * S:(b + 1) * S + TOK])

            while equeue:
                emit_group(equeue.pop(0))

    nc.compile()
    return nc


def _host_prep(x, rope_cos, rope_sin, Wqkv, bqkv, Wout, B, S, D, H, n_cores):
    """Build per-core input maps (bf16 data, fp32 biases)."""
    import ml_dtypes
    BF = ml_dtypes.bfloat16

    T = B * S
    HPC = H // n_cores
    orig = _perm_orig_of_p()
    quad_j = np.arange(DH)
    jmod = quad_j % 32
    i_of_p = (quad_j // 32) * 16 + (jmod % 16)
    sign = np.where(jmod < 16, -1.0, 1.0).astype(np.float32)

    xT = np.ascontiguousarray(x.reshape(T, D).T.astype(BF))  # [D, T]
    cosP = np.ascontiguousarray(rope_cos[:, i_of_p].T.astype(BF))
    sinP = np.ascontiguousarray((rope_sin[:, i_of_p] * sign).T.astype(BF))

    pl = np.arange(128)[:, None]
    ql = np.arange(128)[None, :]
    # additive causal mask for the 128-wide diagonal block: 0 at/below the
    # diagonal (key p <= query q), -1e9 above (exp -> exact 0)
    madd = np.ascontiguousarray(
        np.where(pl <= ql, 0.0, -1e9).astype(BF))  # [128, 128]
    ident = np.ascontiguousarray(np.eye(128, dtype=np.float32).astype(BF))

    ones = np.ones((128, 128), dtype=BF)

    NKO = D // 128
    in_maps = []
    for c in range(n_cores):
        heads = [c * HPC + i for i in range(HPC)]
        wq = [Wqkv[:, h * DH + orig] for h in heads]
        wk = [Wqkv[:, H * DH + h * DH + orig] for h in heads]
        wv = [Wqkv[:, 2 * H * DH + h * DH:2 * H * DH + (h + 1) * DH]
              for h in heads]
        w_c = np.concatenate(wq + wk + wv, axis=1)  # [D, WCOLS]
        CB = w_c.shape[1] // 128
        # [p, cb, ko, 128]: each column-block slice is a contiguous
        # 4KB-per-partition DMA
        w4 = np.ascontiguousarray(
            w_c.reshape(NKO, 128, CB, 128).transpose(1, 2, 0, 3).astype(BF))
        wout_c = np.ascontiguousarray(
            Wout[c * HPC * DH:(c + 1) * HPC * DH, :].astype(BF))
        qb_cols = ([bqkv[h * DH + orig] for h in heads] +
                   [bqkv[H * DH + h * DH + orig] for h in heads])
        qb = np.ascontiguousarray(np.stack(qb_cols, axis=1).astype(np.float32))
        vb_flat = np.concatenate(
            [bqkv[2 * H * DH + h * DH:2 * H * DH + (h + 1) * DH]
             for h in heads])
        vb = np.ascontiguousarray(
            np.broadcast_to(vb_flat[None, :], (128, HPC * DH)).astype(
                np.float32))
        in_maps.append({
            "xT": xT, "w4": w4, "wout": wout_c, "cosP": cosP, "sinP": sinP,
            "madd": madd, "ident": ident, "ones": ones, "qb": qb, "vb": vb,
        })
    return in_maps


def _run(x, rope_cos, rope_sin, Wqkv, bqkv, Wout, bout,
         B, S, D, H, n_cores, trace=False):
    _ensure_imports()
    from concourse.bass_utils import run_bass_kernel_spmd

    HPC = H // n_cores
    import time as _time
    _t0 = _time.time()
    nc = _build_program(B, S, D, HPC)
    print(f"[kernel] build+compile wall: {_time.time() - _t0:.1f}s", flush=True)
    in_maps = _host_prep(np.asarray(x, dtype=np.float32),
                         np.asarray(rope_cos, dtype=np.float32),
                         np.asarray(rope_sin, dtype=np.float32),
                         np.asarray(Wqkv, dtype=np.float32),
                         np.asarray(bqkv, dtype=np.float32),
                         np.asarray(Wout, dtype=np.float32),
                         B, S, D, H, n_cores)
    _t0 = _time.time()
    res = run_bass_kernel_spmd(nc, in_maps, list(range(n_cores)), trace=trace)
    print(f"[kernel] spmd run wall: {_time.time() - _t0:.1f}s", flush=True)
    y = res.results[0]["y"].astype(np.float64)
    for i in range(1, n_cores):
        y += res.results[i]["y"]
    y += np.asarray(bout, dtype=np.float64)[None, :]
    out = y.astype(np.float32).reshape(B, S, D)
    return out, res


def kernel(x, rope_cos, rope_sin, Wqkv, bqkv, Wout, bout):
    out, _ = _run(x, rope_cos, rope_sin, Wqkv, bqkv, Wout, bout,
                  B=4, S=2048, D=2048, H=16, n_cores=8)
    return out



# revision 35
# speedup vs baseline: 1.0409x; 1.0032x over previous
"""Causal self-attention (RoPE, 16 heads) on 8 TRN2 NeuronCores.

Problem: x[4,2048,2048] @ Wqkv -> RoPE(q,k) -> causal softmax(qk^T/sqrt(128)) @ v
         -> out proj Wout.  B=4, S=2048, D=2048, H=16, DH=128.

Sharding: tensor-parallel over heads. Each of the 8 cores computes 2 heads:
QKV projection columns for its heads, RoPE, attention, and its partial of the
output projection (row-sharded Wout). Host sums the 8 partials (+bout).

Design (vs the 903us v1 two-phase fp32r baseline; ~666us fast-mode):
  * bf16 operands everywhere (fp32 PSUM accumulation) — same PE rate as
    float32r, but cheaper weight loads, 2x DVE elementwise, half the DMA.
  * Fully fused per-batch pipeline: QKV+RoPE -> attention -> out-proj with
    Q^T/K^T/V/O^T resident in SBUF (no DRAM scratch round trip, no phase
    barrier, no per-head reload stalls).
  * Causal trimming: for the diagonal 128-k chunk dg, the moving q-range
    starts at dg*128 (N in {512,384,256,128}); only the leading 128-wide
    diagonal block needs the 0/1 mask (applied in place on DVE).
  * Attention latency chain (st -> exp on ACT -> av) hidden by interleaving
    both heads' chains per ki step and emitting the out-projection row-tiles
    of query tile qi right after qi completes (PE filler work); each batch's
    last out-proj group is deferred past the next batch's first QKV tile so
    the boundary always has ready PE work.
  * Softmax denominator via ones-matmul into PSUM; non-diagonal exp chunks
    are pre-summed in groups of four on DVE (bf16) and diagonal chunks in
    overlapping pairs (dg1 into dg0's q-range in place, dg3 into dg2's), so
    ~1/4 as many ones-matmuls stream through the PE.
  * reciprocal_approx_fast for 1/l (~5x faster than DVE reciprocal);
    PSUM->SBUF y copies on DVE as fp32->bf16 casts (keeps the in-order ACT
    queue free for exp); y partials in bf16, summed on host in fp64.
  * Startup: first x tile + first weight quarter interleaved in small DMAs
    ahead of all bulk loads (first matmul at ~14us instead of ~38us).
PSUM budget (8 banks): mm(2, shared QKV-acc/out-proj) + st(2) +
  av_h0/av_h1/lps_h0/lps_h1 (1 each).
"""

import math

import numpy as np


def _ensure_imports():
    try:
        import concourse.bass  # noqa: F401
    except ImportError:
        import sys
        for p in (
            "/root/.axon_site",
            "/root/.axon_site/_ro/trn_rl_repo",
            "/root/.axon_site/_ro/pypackages",
            "/opt/trn_rl_repo",
        ):
            if p not in sys.path:
                sys.path.append(p)


DH = 128
TOK = 512            # token tile (matmul moving free dim)
SHUF_MASK = [(i + 16) % 32 for i in range(32)]


def _perm_orig_of_p():
    """orig head-dim index stored at partition p, for the RoPE layout.

    Partition p = 32*quad + j. Rotation pair index i = 16*quad + (j % 16).
    j < 16 holds the even element (2i), j >= 16 holds the odd (2i+1).
    """
    orig = np.empty(DH, dtype=np.int64)
    for p in range(DH):
        quad, j = divmod(p, 32)
        i = 16 * quad + (j % 16)
        orig[p] = 2 * i if j < 16 else 2 * i + 1
    return orig


def _build_program(B, S, D, HPC):
    """Build the per-core SPMD program. Returns compiled Bacc."""
    import concourse.mybir as mybir
    import concourse.tile as tile
    from concourse import bacc
    from contextlib import ExitStack

    F32 = mybir.dt.float32
    BF16 = mybir.dt.bfloat16
    AF = mybir.ActivationFunctionType
    OP = mybir.AluOpType

    T = B * S
    NKO = D // 128           # contraction chunks for projections
    QCOLS = 2 * HPC          # q + k col-tiles of 128
    VCOLS = HPC * 128
    WCOLS = QCOLS * 128 + VCOLS
    NQI = S // TOK           # q tiles per (b,h)
    NDC = TOK // 128         # 128-chunks per token tile (diag masks)
    NDO = D // TOK           # output Dout tiles
    NKV = S // 128           # v chunks per batch
    scale = 1.0 / math.sqrt(DH)

    CB = WCOLS // 128        # weight column blocks (q,q,k,k,v,v)

    nc = bacc.Bacc()
    xT = nc.dram_tensor("xT", [D, T], BF16, kind="ExternalInput")
    w4 = nc.dram_tensor("w4", [128, CB, NKO, 128], BF16,
                        kind="ExternalInput")
    wout = nc.dram_tensor("wout", [VCOLS, D], BF16, kind="ExternalInput")
    cosP = nc.dram_tensor("cosP", [128, S], BF16, kind="ExternalInput")
    sinP = nc.dram_tensor("sinP", [128, S], BF16, kind="ExternalInput")
    madd = nc.dram_tensor("madd", [128, 128], BF16, kind="ExternalInput")
    ident = nc.dram_tensor("ident", [128, 128], BF16, kind="ExternalInput")
    ones = nc.dram_tensor("ones", [128, 128], BF16, kind="ExternalInput")
    qb = nc.dram_tensor("qb", [128, QCOLS], F32, kind="ExternalInput")
    vb = nc.dram_tensor("vb", [128, VCOLS], F32, kind="ExternalInput")
    y = nc.dram_tensor("y", [T, D], BF16, kind="ExternalOutput")

    xTr = xT.rearrange("(ko p) t -> p ko t", p=128)
    wout_r = wout.rearrange("(h p) d -> p h d", p=128)

    with tile.TileContext(nc) as tc:
        with ExitStack() as ctx:
            s1 = ctx.enter_context(tc.tile_pool(name="singles", bufs=1))
            xp = ctx.enter_context(tc.tile_pool(name="xp", bufs=2))
            qkvp = ctx.enter_context(tc.tile_pool(name="qkvp", bufs=2))
            wk = ctx.enter_context(tc.tile_pool(name="wk", bufs=2))
            ptq = ctx.enter_context(tc.tile_pool(name="ptq", bufs=10))
            ysp = ctx.enter_context(tc.tile_pool(name="ysp", bufs=8))
            psA = ctx.enter_context(
                tc.tile_pool(name="psA", bufs=2, space="PSUM"))
            psB = ctx.enter_context(
                tc.tile_pool(name="psB", bufs=2, space="PSUM"))
            psC = ctx.enter_context(
                tc.tile_pool(name="psC", bufs=1, space="PSUM"))

            # ---- resident tensors -------------------------------------
            # Each dma_start costs ~650ns of issue time on its engine
            # queue, so startup keeps the sync queue to the critical path
            # (first weight column block + the first x tile) and routes
            # every bulk load through the scalar queue, which is idle at
            # startup.  The w4 host layout makes each column-block slice a
            # fully contiguous 4KB-per-partition transfer.
            xt00 = xp.tile([128, NKO, TOK], BF16, tag="xt")
            w_sb = s1.tile([128, CB, NKO, 128], BF16)
            qb_sb = s1.tile([128, QCOLS], F32)
            vb_sb = s1.tile([128, VCOLS], F32)
            cos_sb = s1.tile([128, S], BF16)
            sin_sb = s1.tile([128, S], BF16)
            # All large startup transfers share the sync hw queue: per-queue
            # transfers are FIFO, so consumption order is guaranteed and the
            # bulk loads cannot steal HBM bandwidth from the critical
            # opening x/weight stream (the tile scheduler reorders
            # instructions by dependency, so cross-queue ordering tricks
            # don't hold).  Only the tiny bias/constant loads use scalar.
            hk0 = NKO // 2
            nc.sync.dma_start(out=w_sb[:, 0, 0:hk0], in_=w4[:, 0, 0:hk0])
            qk = NKO // 4
            nc.sync.dma_start(out=xt00[:, 0:qk, :], in_=xTr[:, 0:qk, 0:TOK])
            nc.sync.dma_start(out=w_sb[:, 0, hk0:NKO], in_=w4[:, 0, hk0:NKO])
            for i in range(1, 4):
                nc.sync.dma_start(out=xt00[:, i * qk:(i + 1) * qk, :],
                                  in_=xTr[:, i * qk:(i + 1) * qk, 0:TOK])
            nc.sync.dma_start(out=w_sb[:, 1], in_=w4[:, 1])
            nc.sync.dma_start(out=cos_sb, in_=cosP[:, :])
            nc.sync.dma_start(out=sin_sb, in_=sinP[:, :])
            nc.sync.dma_start(out=w_sb[:, 2:4], in_=w4[:, 2:4])
            nc.sync.dma_start(out=w_sb[:, 4:CB], in_=w4[:, 4:CB])
            nc.scalar.dma_start(out=qb_sb, in_=qb[:, :])
            nc.scalar.dma_start(out=vb_sb, in_=vb[:, :])
            # allocated now, DMA'd after the first xt tile (see loop)
            wout_sb = s1.tile([128, HPC, D], BF16)
            madd_sb = s1.tile([128, 128], BF16)
            ident_sb = s1.tile([128, 128], BF16)
            ones_sb = s1.tile([128, 128], BF16)

            def emit_group(g):
                # one out-projection (qs, do) group: 2 accumulating matmuls
                # + a PSUM->SBUF cast on the chosen engine + the row DMA
                # after its last group.  Groups are drained one-or-two per
                # ki step of the NEXT query tile's attention loop so the
                # PE's spare time there absorbs them and the casts never
                # pace the pipeline.
                b0_, ot_, qs, do, ysb, eng = g
                yp = psA.tile([128, TOK], F32, tag="mm",
                              name=f"yp_{b0_}_{qs}_{do}")
                for h in range(HPC):
                    nc.tensor.matmul(
                        yp, ot_[:, h, qs * 128:(qs + 1) * 128],
                        wout_sb[:, h, do * TOK:(do + 1) * TOK],
                        start=(h == 0), stop=(h == HPC - 1))
                if eng == "act":
                    nc.scalar.activation(
                        ysb[:, do * TOK:(do + 1) * TOK], yp, AF.Copy)
                else:
                    nc.vector.tensor_copy(
                        ysb[:, do * TOK:(do + 1) * TOK], yp)
                if do == NDO - 1:
                    nc.sync.dma_start(
                        out=y[b0_ + qs * 128:b0_ + (qs + 1) * 128, :],
                        in_=ysb)

            def emit_qs_tail(b0_, ot_, qs):
                # kernel-tail row-tile: copies split DVE/ACT so neither
                # in-order queue paces the drain; DMA in halves so the last
                # transfer is small and overlaps the remaining copies
                ysb = ysp.tile([128, D], BF16, tag="ysb",
                               name=f"ysb_{b0_}_{qs}")
                for do in range(NDO):
                    yp = psA.tile([128, TOK], F32, tag="mm",
                                  name=f"yp_{b0_}_{qs}_{do}")
                    for h in range(HPC):
                        nc.tensor.matmul(
                            yp, ot_[:, h, qs * 128:(qs + 1) * 128],
                            wout_sb[:, h, do * TOK:(do + 1) * TOK],
                            start=(h == 0), stop=(h == HPC - 1))
                    if do % 2 == 0:
                        nc.vector.tensor_copy(
                            ysb[:, do * TOK:(do + 1) * TOK], yp)
                    else:
                        nc.scalar.activation(
                            ysb[:, do * TOK:(do + 1) * TOK], yp, AF.Copy)
                    if do % 2 == 1:
                        nc.sync.dma_start(
                            out=y[b0_ + qs * 128:b0_ + (qs + 1) * 128,
                                  (do - 1) * TOK:(do + 1) * TOK],
                            in_=ysb[:, (do - 1) * TOK:(do + 1) * TOK])

            equeue = []
            xt_pre = None
            for b in range(B):
                b0 = b * S
                qt = qkvp.tile([128, HPC, S], BF16, tag="qt")
                kt = qkvp.tile([128, HPC, S], BF16, tag="kt")
                vt = qkvp.tile([128, NKV, VCOLS], BF16, tag="vt")
                ot = qkvp.tile([128, HPC, S], BF16, tag="ot")
                for t in range(NQI):
                    # ---- A(t): QKV projection + RoPE for token tile t ----
                    tg = b0 + t * TOK
                    if b == 0 and t == 0:
                        xt = xt00  # prefetched before the resident loads
                    elif xt_pre is not None:
                        xt = xt_pre  # prefetched during previous B-phase
                        xt_pre = None
                    else:
                        xt = xp.tile([128, NKO, TOK], BF16, tag="xt")
                        for i in range(2):
                            hk = NKO // 2
                            nc.scalar.dma_start(
                                out=xt[:, i * hk:(i + 1) * hk, :],
                                in_=xTr[:, i * hk:(i + 1) * hk, tg:tg + TOK])
                    for c4 in range(QCOLS):
                        acc = psA.tile([128, TOK], F32, tag="mm")
                        for ko in range(NKO):
                            nc.tensor.matmul(
                                acc, w_sb[:, c4, ko, :],
                                xt[:, ko, :],
                                start=(ko == 0), stop=(ko == NKO - 1))
                        raw = wk.tile([128, TOK], BF16, tag="raw")
                        nc.scalar.activation(raw, acc, AF.Identity,
                                             bias=qb_sb[:, c4:c4 + 1])
                        if b == 0 and t == 0 and c4 == 0:
                            # non-critical constants + wout: last in the
                            # sync queue / tiny ones on scalar
                            nc.scalar.dma_start(out=madd_sb, in_=madd[:, :])
                            nc.scalar.dma_start(out=ident_sb,
                                                in_=ident[:, :])
                            nc.scalar.dma_start(out=ones_sb, in_=ones[:, :])
                            nc.sync.dma_start(out=wout_sb, in_=wout_r)
                        sw = wk.tile([128, TOK], BF16, tag="sw")
                        # partition-only permute: bitcast to u32 halves the
                        # streamed element count (pairs along free dim)
                        nc.vector.stream_shuffle(
                            sw.bitcast(mybir.dt.uint32),
                            raw.bitcast(mybir.dt.uint32), SHUF_MASK)
                        m1 = wk.tile([128, TOK], BF16, tag="m1")
                        nc.vector.tensor_tensor(
                            m1, raw, cos_sb[:, t * TOK:(t + 1) * TOK],
                            op=OP.mult)
                        m2 = wk.tile([128, TOK], BF16, tag="m2")
                        nc.vector.tensor_tensor(
                            m2, sw, sin_sb[:, t * TOK:(t + 1) * TOK],
                            op=OP.mult)
                        dst = qt if c4 < HPC else kt
                        nc.vector.tensor_tensor(
                            dst[:, c4 % HPC, t * TOK:(t + 1) * TOK], m1, m2,
                            op=OP.add)
                    for sub in range(NDC):
                        accv = psA.tile([128, VCOLS], F32, tag="mm")
                        for ko in range(NKO):
                            nc.tensor.matmul(
                                accv, xt[:, ko, sub * 128:(sub + 1) * 128],
                                w_sb[:, QCOLS:CB, ko, :],
                                start=(ko == 0), stop=(ko == NKO - 1))
                        nc.vector.tensor_tensor(
                            vt[:, t * NDC + sub, :], accv, vb_sb, op=OP.add)

                # ---- B: attention per query tile (heads interleaved),
                # ---- each followed by its out-projection row-tiles (C)
                for qi in range(NQI):
                    q0 = qi * TOK
                    nki = NDC * qi + NDC
                    avs, lpss = [], []
                    for h in range(HPC):
                        av_h = psC.tile([128, TOK], F32, tag=f"av{h}",
                                        name=f"av{h}_{b}_{qi}")
                        lps_h = psC.tile([128, TOK], F32, tag=f"lps{h}",
                                         name=f"lps{h}_{b}_{qi}")
                        avs.append(av_h)
                        lpss.append(lps_h)
                    pend = [[], []]          # ungrouped non-diag pt, per head
                    pend_d = [None, None]    # diag accumulation base, per head
                    lps_open = [False] * HPC
                    nd_flushed = [0] * HPC   # non-diag chunks already summed

                    def tree_add(ps):
                        # pairwise-reduce pt tiles in place (bf16, DVE); the
                        # AV matmuls that read them are already issued
                        cur = list(ps)
                        while len(cur) > 1:
                            nxt = []
                            for i in range(0, len(cur), 2):
                                nc.vector.tensor_tensor(
                                    cur[i], cur[i], cur[i + 1], op=OP.add)
                                nxt.append(cur[i])
                            cur = nxt
                        return cur[0]
                    groups = equeue          # previous qi's out-proj groups
                    equeue = []
                    ndrained = 0
                    for ki in range(nki):
                        dg = ki - NDC * qi
                        qoff = max(dg, 0) * 128
                        N = TOK - qoff
                        pts = []
                        for h in range(HPC):
                            # both heads' score matmuls + exps issued first so
                            # ACT gets the pair ASAP and each head's PV work
                            # overlaps the other head's exp
                            st = psB.tile([128, TOK], F32, tag="st")
                            if dg >= 0:
                                # causal mask folded into the score psum: an
                                # identity-stationary matmul adds -1e9 above
                                # the diagonal of the leading 128-block, so
                                # exp emits exact zeros there and the DVE
                                # mask multiply disappears from the st->av
                                # chain
                                nc.tensor.matmul(
                                    st[:, :N],
                                    kt[:, h, ki * 128:(ki + 1) * 128],
                                    qt[:, h, q0 + qoff:q0 + TOK],
                                    start=True, stop=False)
                                nc.tensor.matmul(
                                    st[:, 0:128], ident_sb, madd_sb,
                                    start=False, stop=True)
                            else:
                                nc.tensor.matmul(
                                    st[:, :N],
                                    kt[:, h, ki * 128:(ki + 1) * 128],
                                    qt[:, h, q0 + qoff:q0 + TOK],
                                    start=True, stop=True)
                            pt = ptq.tile([128, TOK], BF16, tag="pt",
                                          name=f"pt_{b}_{qi}_{ki}_{h}")
                            nc.scalar.activation(pt[:, :N], st[:, :N], AF.Exp,
                                                 scale=scale)
                            pts.append(pt)
                        for h in range(HPC):
                            pt = pts[h]
                            nc.tensor.matmul(
                                avs[h][:, qoff:TOK],
                                vt[:, ki, h * 128:(h + 1) * 128], pt[:, :N],
                                start=(ki == 0), stop=(ki == nki - 1))
                            # softmax denominator: pre-sum groups of four
                            # non-diag exp chunks on DVE (bf16) so only one
                            # ones-matmul streams per quad (non-diag count
                            # per qi is 4*qi — always a multiple of 4);
                            # diag chunks pair (dg0+dg1, dg2+dg3) by adding
                            # the later chunk into the earlier one's
                            # overlapping q-range in place
                            if dg < 0:
                                # non-diag chunks: pre-sum groups of 8 (or 4
                                # for the remainder) on DVE so only one
                                # ones-matmul streams per group
                                pend[h].append(pt)
                                rem = 4 * qi - nd_flushed[h]
                                gsz = 8 if rem >= 8 else 4
                                if len(pend[h]) < gsz:
                                    continue
                                pp = tree_add(pend[h])
                                pend[h] = []
                                nd_flushed[h] += gsz
                                nc.tensor.matmul(
                                    lpss[h][:, :TOK], ones_sb, pp,
                                    start=(not lps_open[h]), stop=False)
                                lps_open[h] = True
                            elif dg == 0:
                                pend_d[h] = pt
                            else:
                                # diag chunks accumulate into dg0's buffer
                                # in place; one ones-matmul streams the full
                                # TOK range at the last chunk
                                base = pend_d[h]
                                nc.vector.tensor_tensor(
                                    base[:, qoff:TOK], base[:, qoff:TOK],
                                    pt[:, :N], op=OP.add)
                                if dg == NDC - 1:
                                    pend_d[h] = None
                                    nc.tensor.matmul(
                                        lpss[h][:, :TOK], ones_sb,
                                        base[:, :TOK],
                                        start=(not lps_open[h]), stop=True)
                                    lps_open[h] = True
                        # schedule shifted one ki late: the first groups'
                        # ot chunks are still in the previous qi's DVE
                        # reciprocal chain at ki=0
                        tgt = (len(groups) * ki) // nki
                        while ndrained < tgt:
                            emit_group(groups[ndrained])
                            ndrained += 1
                    while ndrained < len(groups):
                        emit_group(groups[ndrained])
                        ndrained += 1
                    if b == B - 1 and qi == NQI - 1:
                        # kernel tail: no later compute hides this chain, so
                        # chunk the reciprocal/divide per 128-query block and
                        # emit each row-tile as soon as its block is ready
                        # (ACT copies + split DMA drain)
                        for sub in range(NDC):
                            c0 = sub * 128
                            for h in range(HPC):
                                recl = wk.tile([128, 128], F32, tag="reclc")
                                nc.vector.reciprocal_approx_fast(
                                    recl, lpss[h][:, c0:c0 + 128])
                                nc.vector.tensor_tensor(
                                    ot[:, h, q0 + c0:q0 + c0 + 128],
                                    avs[h][:, c0:c0 + 128], recl, op=OP.mult)
                            emit_qs_tail(b0, ot, qi * NDC + sub)
                        continue
                    # reciprocal/divide split so the first 128-query block's
                    # ot lands early: the first out-proj groups drained in
                    # the next qi's loop only wait ~1us, not the full chain
                    for c0, cw in ((0, 128), (128, TOK - 128)):
                        for h in range(HPC):
                            recl = wk.tile([128, cw], F32,
                                           tag=f"recl{cw}")
                            nc.vector.reciprocal_approx_fast(
                                recl, lpss[h][:, c0:c0 + cw])
                            nc.vector.tensor_tensor(
                                ot[:, h, q0 + c0:q0 + c0 + cw],
                                avs[h][:, c0:c0 + cw], recl, op=OP.mult)
                    # enqueue this qi's out-projection groups; they drain
                    # through the next query tile's (or next batch's first)
                    # attention loop.  Cast-engine split: the drain context
                    # for qi 3 and 0 is exp-light (few ki steps / next
                    # batch's qi=0), so ACT takes more copies there.
                    d_qi = (qi + 1) % NQI
                    n_act = 2 if d_qi == 0 else 1
                    for qs in range(qi * NDC, (qi + 1) * NDC):
                        ysb = ysp.tile([128, D], BF16, tag="ysb",
                                       name=f"ysb_{b0}_{qs}")
                        for do in range(NDO):
                            eng = "act" if do < n_act else "dve"
                            equeue.append((b0, ot, qs, do, ysb, eng))
                    if qi == NQI - 2 and b < B - 1:
                        # prefetch next batch's first x tile during this
                        # B-phase so the batch boundary never waits on DMA
                        xt_pre = xp.tile([128, NKO, TOK], BF16, tag="xt")
                        for i in range(2):
                            hk = NKO // 2
                            nc.sync.dma_start(
                                out=xt_pre[:, i * hk:(i + 1) * hk, :],
                                in_=xTr[:, i * hk:(i + 1) * hk,
                                        (b + 1) * S:(b + 1) * S + TOK])

            while equeue:
                emit_group(equeue.pop(0))

    nc.compile()
    return nc


def _host_prep(x, rope_cos, rope_sin, Wqkv, bqkv, Wout, B, S, D, H, n_cores):
    """Build per-core input maps (bf16 data, fp32 biases)."""
    import ml_dtypes
    BF = ml_dtypes.bfloat16

    T = B * S
    HPC = H // n_cores
    orig = _perm_orig_of_p()
    quad_j = np.arange(DH)
    jmod = quad_j % 32
    i_of_p = (quad_j // 32) * 16 + (jmod % 16)
    sign = np.where(jmod < 16, -1.0, 1.0).astype(np.float32)

    xT = np.ascontiguousarray(x.reshape(T, D).T.astype(BF))  # [D, T]
    cosP = np.ascontiguousarray(rope_cos[:, i_of_p].T.astype(BF))
    sinP = np.ascontiguousarray((rope_sin[:, i_of_p] * sign).T.astype(BF))

    pl = np.arange(128)[:, None]
    ql = np.arange(128)[None, :]
    # additive causal mask for the 128-wide diagonal block: 0 at/below the
    # diagonal (key p <= query q), -1e9 above (exp -> exact 0)
    madd = np.ascontiguousarray(
        np.where(pl <= ql, 0.0, -1e9).astype(BF))  # [128, 128]
    ident = np.ascontiguousarray(np.eye(128, dtype=np.float32).astype(BF))

    ones = np.ones((128, 128), dtype=BF)

    NKO = D // 128
    in_maps = []
    for c in range(n_cores):
        heads = [c * HPC + i for i in range(HPC)]
        wq = [Wqkv[:, h * DH + orig] for h in heads]
        wk = [Wqkv[:, H * DH + h * DH + orig] for h in heads]
        wv = [Wqkv[:, 2 * H * DH + h * DH:2 * H * DH + (h + 1) * DH]
              for h in heads]
        w_c = np.concatenate(wq + wk + wv, axis=1)  # [D, WCOLS]
        CB = w_c.shape[1] // 128
        # [p, cb, ko, 128]: each column-block slice is a contiguous
        # 4KB-per-partition DMA
        w4 = np.ascontiguousarray(
            w_c.reshape(NKO, 128, CB, 128).transpose(1, 2, 0, 3).astype(BF))
        wout_c = np.ascontiguousarray(
            Wout[c * HPC * DH:(c + 1) * HPC * DH, :].astype(BF))
        qb_cols = ([bqkv[h * DH + orig] for h in heads] +
                   [bqkv[H * DH + h * DH + orig] for h in heads])
        qb = np.ascontiguousarray(np.stack(qb_cols, axis=1).astype(np.float32))
        vb_flat = np.concatenate(
            [bqkv[2 * H * DH + h * DH:2 * H * DH + (h + 1) * DH]
             for h in heads])
        vb = np.ascontiguousarray(
            np.broadcast_to(vb_flat[None, :], (128, HPC * DH)).astype(
                np.float32))
        in_maps.append({
            "xT": xT, "w4": w4, "wout": wout_c, "cosP": cosP, "sinP": sinP,
            "madd": madd, "ident": ident, "ones": ones, "qb": qb, "vb": vb,
        })
    return in_maps


def _run(x, rope_cos, rope_sin, Wqkv, bqkv, Wout, bout,
         B, S, D, H, n_cores, trace=False):
    _ensure_imports()
    from concourse.bass_utils import run_bass_kernel_spmd

    HPC = H // n_cores
    import time as _time
    _t0 = _time.time()
    nc = _build_program(B, S, D, HPC)
    print(f"[kernel] build+compile wall: {_time.time() - _t0:.1f}s", flush=True)
    in_maps = _host_prep(np.asarray(x, dtype=np.float32),
                         np.asarray(rope_cos, dtype=np.float32),
                         np.asarray(rope_sin, dtype=np.float32),
                         np.asarray(Wqkv, dtype=np.float32),
                         np.asarray(bqkv, dtype=np.float32),
                         np.asarray(Wout, dtype=np.float32),
                         B, S, D, H, n_cores)
    _t0 = _time.time()
    res = run_bass_kernel_spmd(nc, in_maps, list(range(n_cores)), trace=trace)
    print(f"[kernel] spmd run wall: {_time.time() - _t0:.1f}s", flush=True)
    y = res.results[0]["y"].astype(np.float64)
    for i in range(1, n_cores):
        y += res.results[i]["y"]
    y += np.asarray(bout, dtype=np.float64)[None, :]
    out = y.astype(np.float32).reshape(B, S, D)
    return out, res


def kernel(x, rope_cos, rope_sin, Wqkv, bqkv, Wout, bout):
    out, _ = _run(x, rope_cos, rope_sin, Wqkv, bqkv, Wout, bout,
                  B=4, S=2048, D=2048, H=16, n_cores=8)
    return out



# revision 42
# speedup vs baseline: 1.0445x; 1.0035x over previous
"""Causal self-attention (RoPE, 16 heads) on 8 TRN2 NeuronCores.

Problem: x[4,2048,2048] @ Wqkv -> RoPE(q,k) -> causal softmax(qk^T/sqrt(128)) @ v
         -> out proj Wout.  B=4, S=2048, D=2048, H=16, DH=128.

Sharding: tensor-parallel over heads. Each of the 8 cores computes 2 heads:
QKV projection columns for its heads, RoPE, attention, and its partial of the
output projection (row-sharded Wout). Host sums the 8 partials (+bout).

Design (vs the 903us v1 two-phase fp32r baseline; ~666us fast-mode):
  * bf16 operands everywhere (fp32 PSUM accumulation) — same PE rate as
    float32r, but cheaper weight loads, 2x DVE elementwise, half the DMA.
  * Fully fused per-batch pipeline: QKV+RoPE -> attention -> out-proj with
    Q^T/K^T/V/O^T resident in SBUF (no DRAM scratch round trip, no phase
    barrier, no per-head reload stalls).
  * Causal trimming: for the diagonal 128-k chunk dg, the moving q-range
    starts at dg*128 (N in {512,384,256,128}); only the leading 128-wide
    diagonal block needs the 0/1 mask (applied in place on DVE).
  * Attention latency chain (st -> exp on ACT -> av) hidden by interleaving
    both heads' chains per ki step and emitting the out-projection row-tiles
    of query tile qi right after qi completes (PE filler work); each batch's
    last out-proj group is deferred past the next batch's first QKV tile so
    the boundary always has ready PE work.
  * Softmax denominator via ones-matmul into PSUM; non-diagonal exp chunks
    are pre-summed in groups of four on DVE (bf16) and diagonal chunks in
    overlapping pairs (dg1 into dg0's q-range in place, dg3 into dg2's), so
    ~1/4 as many ones-matmuls stream through the PE.
  * reciprocal_approx_fast for 1/l (~5x faster than DVE reciprocal);
    PSUM->SBUF y copies on DVE as fp32->bf16 casts (keeps the in-order ACT
    queue free for exp); y partials in bf16, summed on host in fp64.
  * Startup: first x tile + first weight quarter interleaved in small DMAs
    ahead of all bulk loads (first matmul at ~14us instead of ~38us).
PSUM budget (8 banks): mm(2, shared QKV-acc/out-proj) + st(2) +
  av_h0/av_h1/lps_h0/lps_h1 (1 each).
"""

import math

import numpy as np


def _ensure_imports():
    try:
        import concourse.bass  # noqa: F401
    except ImportError:
        import sys
        for p in (
            "/root/.axon_site",
            "/root/.axon_site/_ro/trn_rl_repo",
            "/root/.axon_site/_ro/pypackages",
            "/opt/trn_rl_repo",
        ):
            if p not in sys.path:
                sys.path.append(p)


DH = 128
TOK = 512            # token tile (matmul moving free dim)
SHUF_MASK = [(i + 16) % 32 for i in range(32)]


def _perm_orig_of_p():
    """orig head-dim index stored at partition p, for the RoPE layout.

    Partition p = 32*quad + j. Rotation pair index i = 16*quad + (j % 16).
    j < 16 holds the even element (2i), j >= 16 holds the odd (2i+1).
    """
    orig = np.empty(DH, dtype=np.int64)
    for p in range(DH):
        quad, j = divmod(p, 32)
        i = 16 * quad + (j % 16)
        orig[p] = 2 * i if j < 16 else 2 * i + 1
    return orig


def _build_program(B, S, D, HPC):
    """Build the per-core SPMD program. Returns compiled Bacc."""
    import concourse.mybir as mybir
    import concourse.tile as tile
    from concourse import bacc
    from contextlib import ExitStack

    F32 = mybir.dt.float32
    BF16 = mybir.dt.bfloat16
    AF = mybir.ActivationFunctionType
    OP = mybir.AluOpType

    T = B * S
    NKO = D // 128           # contraction chunks for projections
    QCOLS = 2 * HPC          # q + k col-tiles of 128
    VCOLS = HPC * 128
    WCOLS = QCOLS * 128 + VCOLS
    NQI = S // TOK           # q tiles per (b,h)
    NDC = TOK // 128         # 128-chunks per token tile (diag masks)
    NDO = D // TOK           # output Dout tiles
    NKV = S // 128           # v chunks per batch
    scale = 1.0 / math.sqrt(DH)

    CB = WCOLS // 128        # weight column blocks (q,q,k,k,v,v)

    NTT = T // TOK           # token tiles overall

    nc = bacc.Bacc()
    xT = nc.dram_tensor("xT", [128, NTT, NKO, TOK], BF16,
                        kind="ExternalInput")
    w4 = nc.dram_tensor("w4", [128, CB, NKO, 128], BF16,
                        kind="ExternalInput")
    wout = nc.dram_tensor("wout", [VCOLS, D], BF16, kind="ExternalInput")
    cosP = nc.dram_tensor("cosP", [128, S], BF16, kind="ExternalInput")
    sinP = nc.dram_tensor("sinP", [128, S], BF16, kind="ExternalInput")
    madd = nc.dram_tensor("madd", [128, 128], BF16, kind="ExternalInput")
    ident = nc.dram_tensor("ident", [128, 128], BF16, kind="ExternalInput")
    ones = nc.dram_tensor("ones", [128, 128], BF16, kind="ExternalInput")
    qb = nc.dram_tensor("qb", [128, QCOLS], F32, kind="ExternalInput")
    vb = nc.dram_tensor("vb", [128, VCOLS], F32, kind="ExternalInput")
    y = nc.dram_tensor("y", [T, D], BF16, kind="ExternalOutput")

    wout_r = wout.rearrange("(h p) d -> p h d", p=128)

    with tile.TileContext(nc) as tc:
        with ExitStack() as ctx:
            s1 = ctx.enter_context(tc.tile_pool(name="singles", bufs=1))
            xp = ctx.enter_context(tc.tile_pool(name="xp", bufs=2))
            qkvp = ctx.enter_context(tc.tile_pool(name="qkvp", bufs=2))
            wk = ctx.enter_context(tc.tile_pool(name="wk", bufs=2))
            ptq = ctx.enter_context(tc.tile_pool(name="ptq", bufs=10))
            ysp = ctx.enter_context(tc.tile_pool(name="ysp", bufs=8))
            psA = ctx.enter_context(
                tc.tile_pool(name="psA", bufs=2, space="PSUM"))
            psB = ctx.enter_context(
                tc.tile_pool(name="psB", bufs=2, space="PSUM"))
            psC = ctx.enter_context(
                tc.tile_pool(name="psC", bufs=1, space="PSUM"))

            # ---- resident tensors -------------------------------------
            # Each dma_start costs ~650ns of issue time on its engine
            # queue, so startup keeps the sync queue to the critical path
            # (first weight column block + the first x tile) and routes
            # every bulk load through the scalar queue, which is idle at
            # startup.  The w4 host layout makes each column-block slice a
            # fully contiguous 4KB-per-partition transfer.
            xt00 = xp.tile([128, NKO, TOK], BF16, tag="xt")
            w_sb = s1.tile([128, CB, NKO, 128], BF16)
            qb_sb = s1.tile([128, QCOLS], F32)
            vb_sb = s1.tile([128, VCOLS], F32)
            cos_sb = s1.tile([128, S], BF16)
            sin_sb = s1.tile([128, S], BF16)
            # All large startup transfers share the sync hw queue: per-queue
            # transfers are FIFO, so consumption order is guaranteed and the
            # bulk loads cannot steal HBM bandwidth from the critical
            # opening x/weight stream (the tile scheduler reorders
            # instructions by dependency, so cross-queue ordering tricks
            # don't hold).  Only the tiny bias/constant loads use scalar.
            hk0 = NKO // 2
            nc.sync.dma_start(out=w_sb[:, 0, 0:hk0], in_=w4[:, 0, 0:hk0])
            qk = NKO // 4
            nc.sync.dma_start(out=xt00[:, 0:qk, :], in_=xT[:, 0, 0:qk, :])
            nc.sync.dma_start(out=w_sb[:, 0, hk0:NKO], in_=w4[:, 0, hk0:NKO])
            for i in range(1, 4):
                nc.sync.dma_start(out=xt00[:, i * qk:(i + 1) * qk, :],
                                  in_=xT[:, 0, i * qk:(i + 1) * qk, :])
            nc.sync.dma_start(out=w_sb[:, 1], in_=w4[:, 1])
            nc.sync.dma_start(out=cos_sb, in_=cosP[:, :])
            nc.sync.dma_start(out=sin_sb, in_=sinP[:, :])
            nc.sync.dma_start(out=w_sb[:, 2:4], in_=w4[:, 2:4])
            nc.sync.dma_start(out=w_sb[:, 4:CB], in_=w4[:, 4:CB])
            nc.scalar.dma_start(out=qb_sb, in_=qb[:, :])
            nc.scalar.dma_start(out=vb_sb, in_=vb[:, :])
            # allocated now, DMA'd after the first xt tile (see loop)
            wout_sb = s1.tile([128, HPC, D], BF16)
            madd_sb = s1.tile([128, 128], BF16)
            ident_sb = s1.tile([128, 128], BF16)
            ones_sb = s1.tile([128, 128], BF16)

            def emit_group(g, force_act=False):
                # one out-projection (qs, do) group: 2 accumulating matmuls
                # + a PSUM->SBUF cast on the chosen engine + the row DMA
                # after its last group.  Groups are drained one-or-two per
                # ki step of the NEXT query tile's attention loop so the
                # PE's spare time there absorbs them and the casts never
                # pace the pipeline.
                b0_, ot_, qs, do, ysb, eng = g
                if force_act:
                    eng = "act"
                yp = psA.tile([128, TOK], F32, tag="mm",
                              name=f"yp_{b0_}_{qs}_{do}")
                for h in range(HPC):
                    nc.tensor.matmul(
                        yp, ot_[:, h, qs * 128:(qs + 1) * 128],
                        wout_sb[:, h, do * TOK:(do + 1) * TOK],
                        start=(h == 0), stop=(h == HPC - 1))
                if eng == "act":
                    nc.scalar.activation(
                        ysb[:, do * TOK:(do + 1) * TOK], yp, AF.Copy)
                else:
                    nc.vector.tensor_copy(
                        ysb[:, do * TOK:(do + 1) * TOK], yp)
                if do == NDO - 1:
                    nc.sync.dma_start(
                        out=y[b0_ + qs * 128:b0_ + (qs + 1) * 128, :],
                        in_=ysb)

            def emit_qs_tail(b0_, ot_, qs):
                # kernel-tail row-tile: copies split DVE/ACT so neither
                # in-order queue paces the drain; DMA in halves so the last
                # transfer is small and overlaps the remaining copies
                ysb = ysp.tile([128, D], BF16, tag="ysb",
                               name=f"ysb_{b0_}_{qs}")
                for do in range(NDO):
                    yp = psA.tile([128, TOK], F32, tag="mm",
                                  name=f"yp_{b0_}_{qs}_{do}")
                    for h in range(HPC):
                        nc.tensor.matmul(
                            yp, ot_[:, h, qs * 128:(qs + 1) * 128],
                            wout_sb[:, h, do * TOK:(do + 1) * TOK],
                            start=(h == 0), stop=(h == HPC - 1))
                    if do % 2 == 0:
                        nc.vector.tensor_copy(
                            ysb[:, do * TOK:(do + 1) * TOK], yp)
                    else:
                        nc.scalar.activation(
                            ysb[:, do * TOK:(do + 1) * TOK], yp, AF.Copy)
                    if do % 2 == 1:
                        nc.sync.dma_start(
                            out=y[b0_ + qs * 128:b0_ + (qs + 1) * 128,
                                  (do - 1) * TOK:(do + 1) * TOK],
                            in_=ysb[:, (do - 1) * TOK:(do + 1) * TOK])

            equeue = []
            xt_pre = None
            for b in range(B):
                b0 = b * S
                qt = qkvp.tile([128, HPC, S], BF16, tag="qt")
                kt = qkvp.tile([128, HPC, S], BF16, tag="kt")
                vt = qkvp.tile([128, NKV, VCOLS], BF16, tag="vt")
                ot = qkvp.tile([128, HPC, S], BF16, tag="ot")
                for t in range(NQI):
                    # ---- A(t): QKV projection + RoPE for token tile t ----
                    tg = b0 + t * TOK
                    if b == 0 and t == 0:
                        xt = xt00  # prefetched before the resident loads
                    elif xt_pre is not None:
                        xt = xt_pre  # prefetched during previous B-phase
                        xt_pre = None
                    else:
                        xt = xp.tile([128, NKO, TOK], BF16, tag="xt")
                        nc.scalar.dma_start(out=xt, in_=xT[:, tg // TOK])
                    for c4 in range(QCOLS):
                        acc = psA.tile([128, TOK], F32, tag="mm")
                        for ko in range(NKO):
                            nc.tensor.matmul(
                                acc, w_sb[:, c4, ko, :],
                                xt[:, ko, :],
                                start=(ko == 0), stop=(ko == NKO - 1))
                        raw = wk.tile([128, TOK], BF16, tag="raw")
                        nc.scalar.activation(raw, acc, AF.Identity,
                                             bias=qb_sb[:, c4:c4 + 1])
                        if b == 0 and t == 0 and c4 == 0:
                            # non-critical constants + wout: last in the
                            # sync queue / tiny ones on scalar
                            nc.scalar.dma_start(out=madd_sb, in_=madd[:, :])
                            nc.scalar.dma_start(out=ident_sb,
                                                in_=ident[:, :])
                            nc.scalar.dma_start(out=ones_sb, in_=ones[:, :])
                            nc.sync.dma_start(out=wout_sb, in_=wout_r)
                        sw = wk.tile([128, TOK], BF16, tag="sw")
                        # partition-only permute: bitcast to u32 halves the
                        # streamed element count (pairs along free dim)
                        nc.vector.stream_shuffle(
                            sw.bitcast(mybir.dt.uint32),
                            raw.bitcast(mybir.dt.uint32), SHUF_MASK)
                        m1 = wk.tile([128, TOK], BF16, tag="m1")
                        nc.vector.tensor_tensor(
                            m1, raw, cos_sb[:, t * TOK:(t + 1) * TOK],
                            op=OP.mult)
                        m2 = wk.tile([128, TOK], BF16, tag="m2")
                        nc.vector.tensor_tensor(
                            m2, sw, sin_sb[:, t * TOK:(t + 1) * TOK],
                            op=OP.mult)
                        dst = qt if c4 < HPC else kt
                        nc.vector.tensor_tensor(
                            dst[:, c4 % HPC, t * TOK:(t + 1) * TOK], m1, m2,
                            op=OP.add)
                    for sub in range(NDC):
                        accv = psA.tile([128, VCOLS], F32, tag="mm")
                        for ko in range(NKO):
                            nc.tensor.matmul(
                                accv, xt[:, ko, sub * 128:(sub + 1) * 128],
                                w_sb[:, QCOLS:CB, ko, :],
                                start=(ko == 0), stop=(ko == NKO - 1))
                        nc.vector.tensor_tensor(
                            vt[:, t * NDC + sub, :], accv, vb_sb, op=OP.add)

                # ---- B: attention per query tile (heads interleaved),
                # ---- each followed by its out-projection row-tiles (C)
                for qi in range(NQI):
                    q0 = qi * TOK
                    nki = NDC * qi + NDC
                    avs, lpss = [], []
                    for h in range(HPC):
                        av_h = psC.tile([128, TOK], F32, tag=f"av{h}",
                                        name=f"av{h}_{b}_{qi}")
                        lps_h = psC.tile([128, TOK], F32, tag=f"lps{h}",
                                         name=f"lps{h}_{b}_{qi}")
                        avs.append(av_h)
                        lpss.append(lps_h)
                    pend = [[], []]          # ungrouped non-diag pt, per head
                    pend_d = [None, None]    # diag accumulation base, per head
                    lps_open = [False] * HPC
                    nd_flushed = [0] * HPC   # non-diag chunks already summed
                    groups = equeue          # previous qi's out-proj groups
                    equeue = []
                    ndrained = 0
                    for ki in range(nki):
                        dg = ki - NDC * qi
                        qoff = max(dg, 0) * 128
                        N = TOK - qoff
                        pts = []
                        for h in range(HPC):
                            # both heads' score matmuls + exps issued first so
                            # ACT gets the pair ASAP and each head's PV work
                            # overlaps the other head's exp
                            st = psB.tile([128, TOK], F32, tag="st")
                            if dg >= 0:
                                # causal mask folded into the score psum: an
                                # identity-stationary matmul adds -1e9 above
                                # the diagonal of the leading 128-block, so
                                # exp emits exact zeros there and the DVE
                                # mask multiply disappears from the st->av
                                # chain
                                nc.tensor.matmul(
                                    st[:, :N],
                                    kt[:, h, ki * 128:(ki + 1) * 128],
                                    qt[:, h, q0 + qoff:q0 + TOK],
                                    start=True, stop=False)
                                nc.tensor.matmul(
                                    st[:, 0:128], ident_sb, madd_sb,
                                    start=False, stop=True)
                            else:
                                nc.tensor.matmul(
                                    st[:, :N],
                                    kt[:, h, ki * 128:(ki + 1) * 128],
                                    qt[:, h, q0 + qoff:q0 + TOK],
                                    start=True, stop=True)
                            pt = ptq.tile([128, TOK], BF16, tag="pt",
                                          name=f"pt_{b}_{qi}_{ki}_{h}")
                            nc.scalar.activation(pt[:, :N], st[:, :N], AF.Exp,
                                                 scale=scale)
                            pts.append(pt)
                        for h in range(HPC):
                            nc.tensor.matmul(
                                avs[h][:, qoff:TOK],
                                vt[:, ki, h * 128:(h + 1) * 128],
                                pts[h][:, :N],
                                start=(ki == 0), stop=(ki == nki - 1))
                        # drain out-proj groups of the previous query tile
                        # here, before the lps section, so the PE has work
                        # while DVE finishes the pre-sum adds and the
                        # ones-matmuls never stall on them.  (schedule
                        # shifted one ki late: the first groups' ot chunks
                        # are still in the previous qi's reciprocal chain
                        # at ki=0)
                        tgt = (len(groups) * ki) // nki
                        while ndrained < tgt:
                            emit_group(groups[ndrained])
                            ndrained += 1
                        for h in range(HPC):
                            pt = pts[h]
                            # softmax denominator: pre-sum groups of four
                            # non-diag exp chunks on DVE (bf16) so only one
                            # ones-matmul streams per quad (non-diag count
                            # per qi is 4*qi — always a multiple of 4);
                            # diag chunks pair (dg0+dg1, dg2+dg3) by adding
                            # the later chunk into the earlier one's
                            # overlapping q-range in place
                            if dg < 0:
                                # non-diag chunks: pre-sum groups of 8 (or 4
                                # for the remainder) in place on DVE (bf16)
                                # so only one ones-matmul streams per group
                                pend[h].append(pt)
                                rem = 4 * qi - nd_flushed[h]
                                gsz = 8 if rem >= 8 else 4
                                if len(pend[h]) < gsz:
                                    continue
                                cur = pend[h]
                                while len(cur) > 1:
                                    nxt = []
                                    for i in range(0, len(cur), 2):
                                        nc.vector.tensor_tensor(
                                            cur[i], cur[i], cur[i + 1],
                                            op=OP.add)
                                        nxt.append(cur[i])
                                    cur = nxt
                                nc.tensor.matmul(
                                    lpss[h][:, :TOK], ones_sb, cur[0],
                                    start=(not lps_open[h]), stop=False)
                                pend[h] = []
                                nd_flushed[h] += gsz
                                lps_open[h] = True
                            elif dg == 0:
                                pend_d[h] = pt
                            else:
                                # diag chunks accumulate into dg0's buffer
                                # in place; one ones-matmul streams the full
                                # TOK range at the last chunk
                                base = pend_d[h]
                                nc.vector.tensor_tensor(
                                    base[:, qoff:TOK], base[:, qoff:TOK],
                                    pt[:, :N], op=OP.add)
                                if dg == NDC - 1:
                                    pend_d[h] = None
                                    nc.tensor.matmul(
                                        lpss[h][:, :TOK], ones_sb,
                                        base[:, :TOK],
                                        start=(not lps_open[h]), stop=True)
                                    lps_open[h] = True
                    while ndrained < len(groups):
                        emit_group(groups[ndrained])
                        ndrained += 1
                    if b == B - 1 and qi == NQI - 1:
                        # kernel tail: no later compute hides this chain, so
                        # chunk the reciprocal/divide per 128-query block and
                        # emit each row-tile as soon as its block is ready
                        # (ACT copies + split DMA drain)
                        for sub in range(NDC):
                            c0 = sub * 128
                            for h in range(HPC):
                                recl = wk.tile([128, 128], F32, tag="reclc")
                                nc.vector.reciprocal_approx_fast(
                                    recl, lpss[h][:, c0:c0 + 128])
                                nc.vector.tensor_tensor(
                                    ot[:, h, q0 + c0:q0 + c0 + 128],
                                    avs[h][:, c0:c0 + 128], recl, op=OP.mult)
                            emit_qs_tail(b0, ot, qi * NDC + sub)
                        continue
                    # reciprocal/divide split so the first 128-query block's
                    # ot lands early: the first out-proj groups drained in
                    # the next qi's loop only wait ~1us, not the full chain
                    for c0, cw in ((0, 128), (128, TOK - 128)):
                        for h in range(HPC):
                            recl = wk.tile([128, cw], F32,
                                           tag=f"recl{cw}")
                            nc.vector.reciprocal_approx_fast(
                                recl, lpss[h][:, c0:c0 + cw])
                            nc.vector.tensor_tensor(
                                ot[:, h, q0 + c0:q0 + c0 + cw],
                                avs[h][:, c0:c0 + cw], recl, op=OP.mult)
                    # enqueue this qi's out-projection groups; they drain
                    # through the next query tile's (or next batch's first)
                    # attention loop.  Cast-engine split: the drain context
                    # for qi 3 and 0 is exp-light (few ki steps / next
                    # batch's qi=0), so ACT takes more copies there.
                    d_qi = (qi + 1) % NQI
                    n_act = 2 if d_qi == 0 else 1
                    for qs in range(qi * NDC, (qi + 1) * NDC):
                        ysb = ysp.tile([128, D], BF16, tag="ysb",
                                       name=f"ysb_{b0}_{qs}")
                        for do in range(NDO):
                            eng = "act" if do < n_act else "dve"
                            equeue.append((b0, ot, qs, do, ysb, eng))
                    if qi == NQI - 2 and b < B - 1:
                        # prefetch next batch's first x tile during this
                        # B-phase so the batch boundary never waits on DMA
                        xt_pre = xp.tile([128, NKO, TOK], BF16, tag="xt")
                        nc.sync.dma_start(out=xt_pre,
                                          in_=xT[:, (b + 1) * S // TOK])

            while equeue:
                emit_group(equeue.pop(0))

    nc.compile()
    return nc


def _host_prep(x, rope_cos, rope_sin, Wqkv, bqkv, Wout, B, S, D, H, n_cores):
    """Build per-core input maps (bf16 data, fp32 biases)."""
    import ml_dtypes
    BF = ml_dtypes.bfloat16

    T = B * S
    HPC = H // n_cores
    orig = _perm_orig_of_p()
    quad_j = np.arange(DH)
    jmod = quad_j % 32
    i_of_p = (quad_j // 32) * 16 + (jmod % 16)
    sign = np.where(jmod < 16, -1.0, 1.0).astype(np.float32)

    # [p, token-tile, ko, tok]: every x-tile DMA slice is fully contiguous
    # per partition (16KB lines)
    NTT = T // 512
    xT = np.ascontiguousarray(
        x.reshape(NTT, 512, D // 128, 128).transpose(3, 0, 2, 1).astype(BF))
    cosP = np.ascontiguousarray(rope_cos[:, i_of_p].T.astype(BF))
    sinP = np.ascontiguousarray((rope_sin[:, i_of_p] * sign).T.astype(BF))

    pl = np.arange(128)[:, None]
    ql = np.arange(128)[None, :]
    # additive causal mask for the 128-wide diagonal block: 0 at/below the
    # diagonal (key p <= query q), -1e9 above (exp -> exact 0)
    madd = np.ascontiguousarray(
        np.where(pl <= ql, 0.0, -1e9).astype(BF))  # [128, 128]
    ident = np.ascontiguousarray(np.eye(128, dtype=np.float32).astype(BF))

    ones = np.ones((128, 128), dtype=BF)

    NKO = D // 128
    in_maps = []
    for c in range(n_cores):
        heads = [c * HPC + i for i in range(HPC)]
        wq = [Wqkv[:, h * DH + orig] for h in heads]
        wk = [Wqkv[:, H * DH + h * DH + orig] for h in heads]
        wv = [Wqkv[:, 2 * H * DH + h * DH:2 * H * DH + (h + 1) * DH]
              for h in heads]
        w_c = np.concatenate(wq + wk + wv, axis=1)  # [D, WCOLS]
        CB = w_c.shape[1] // 128
        # [p, cb, ko, 128]: each column-block slice is a contiguous
        # 4KB-per-partition DMA
        w4 = np.ascontiguousarray(
            w_c.reshape(NKO, 128, CB, 128).transpose(1, 2, 0, 3).astype(BF))
        wout_c = np.ascontiguousarray(
            Wout[c * HPC * DH:(c + 1) * HPC * DH, :].astype(BF))
        qb_cols = ([bqkv[h * DH + orig] for h in heads] +
                   [bqkv[H * DH + h * DH + orig] for h in heads])
        qb = np.ascontiguousarray(np.stack(qb_cols, axis=1).astype(np.float32))
        vb_flat = np.concatenate(
            [bqkv[2 * H * DH + h * DH:2 * H * DH + (h + 1) * DH]
             for h in heads])
        vb = np.ascontiguousarray(
            np.broadcast_to(vb_flat[None, :], (128, HPC * DH)).astype(
                np.float32))
        in_maps.append({
            "xT": xT, "w4": w4, "wout": wout_c, "cosP": cosP, "sinP": sinP,
            "madd": madd, "ident": ident, "ones": ones, "qb": qb, "vb": vb,
        })
    return in_maps


def _run(x, rope_cos, rope_sin, Wqkv, bqkv, Wout, bout,
         B, S, D, H, n_cores, trace=False):
    _ensure_imports()
    from concourse.bass_utils import run_bass_kernel_spmd

    HPC = H // n_cores
    import time as _time
    _t0 = _time.time()
    nc = _build_program(B, S, D, HPC)
    print(f"[kernel] build+compile wall: {_time.time() - _t0:.1f}s", flush=True)
    in_maps = _host_prep(np.asarray(x, dtype=np.float32),
                         np.asarray(rope_cos, dtype=np.float32),
                         np.asarray(rope_sin, dtype=np.float32),
                         np.asarray(Wqkv, dtype=np.float32),
                         np.asarray(bqkv, dtype=np.float32),
                         np.asarray(Wout, dtype=np.float32),
                         B, S, D, H, n_cores)
    _t0 = _time.time()
    res = run_bass_kernel_spmd(nc, in_maps, list(range(n_cores)), trace=trace)
    print(f"[kernel] spmd run wall: {_time.time() - _t0:.1f}s", flush=True)
    y = res.results[0]["y"].astype(np.float64)
    for i in range(1, n_cores):
        y += res.results[i]["y"]
    y += np.asarray(bout, dtype=np.float64)[None, :]
    out = y.astype(np.float32).reshape(B, S, D)
    return out, res


def kernel(x, rope_cos, rope_sin, Wqkv, bqkv, Wout, bout):
    out, _ = _run(x, rope_cos, rope_sin, Wqkv, bqkv, Wout, bout,
                  B=4, S=2048, D=2048, H=16, n_cores=8)
    return out



# revision 44
# speedup vs baseline: 1.0515x; 1.0067x over previous
"""Causal self-attention (RoPE, 16 heads) on 8 TRN2 NeuronCores.

Problem: x[4,2048,2048] @ Wqkv -> RoPE(q,k) -> causal softmax(qk^T/sqrt(128)) @ v
         -> out proj Wout.  B=4, S=2048, D=2048, H=16, DH=128.

Sharding: tensor-parallel over heads. Each of the 8 cores computes 2 heads:
QKV projection columns for its heads, RoPE, attention, and its partial of the
output projection (row-sharded Wout). Host sums the 8 partials (+bout).

Design (vs the 903us v1 two-phase fp32r baseline; ~666us fast-mode):
  * bf16 operands everywhere (fp32 PSUM accumulation) — same PE rate as
    float32r, but cheaper weight loads, 2x DVE elementwise, half the DMA.
  * Fully fused per-batch pipeline: QKV+RoPE -> attention -> out-proj with
    Q^T/K^T/V/O^T resident in SBUF (no DRAM scratch round trip, no phase
    barrier, no per-head reload stalls).
  * Causal trimming: for the diagonal 128-k chunk dg, the moving q-range
    starts at dg*128 (N in {512,384,256,128}); only the leading 128-wide
    diagonal block needs the 0/1 mask (applied in place on DVE).
  * Attention latency chain (st -> exp on ACT -> av) hidden by interleaving
    both heads' chains per ki step and emitting the out-projection row-tiles
    of query tile qi right after qi completes (PE filler work); each batch's
    last out-proj group is deferred past the next batch's first QKV tile so
    the boundary always has ready PE work.
  * Softmax denominator via ones-matmul into PSUM; non-diagonal exp chunks
    are pre-summed in groups of four on DVE (bf16) and diagonal chunks in
    overlapping pairs (dg1 into dg0's q-range in place, dg3 into dg2's), so
    ~1/4 as many ones-matmuls stream through the PE.
  * reciprocal_approx_fast for 1/l (~5x faster than DVE reciprocal);
    PSUM->SBUF y copies on DVE as fp32->bf16 casts (keeps the in-order ACT
    queue free for exp); y partials in bf16, summed on host in fp64.
  * Startup: first x tile + first weight quarter interleaved in small DMAs
    ahead of all bulk loads (first matmul at ~14us instead of ~38us).
PSUM budget (8 banks): mm(2, shared QKV-acc/out-proj) + st(2) +
  av_h0/av_h1/lps_h0/lps_h1 (1 each).
"""

import math

import numpy as np


def _ensure_imports():
    try:
        import concourse.bass  # noqa: F401
    except ImportError:
        import sys
        for p in (
            "/root/.axon_site",
            "/root/.axon_site/_ro/trn_rl_repo",
            "/root/.axon_site/_ro/pypackages",
            "/opt/trn_rl_repo",
        ):
            if p not in sys.path:
                sys.path.append(p)


DH = 128
TOK = 512            # token tile (matmul moving free dim)
SHUF_MASK = [(i + 16) % 32 for i in range(32)]


def _perm_orig_of_p():
    """orig head-dim index stored at partition p, for the RoPE layout.

    Partition p = 32*quad + j. Rotation pair index i = 16*quad + (j % 16).
    j < 16 holds the even element (2i), j >= 16 holds the odd (2i+1).
    """
    orig = np.empty(DH, dtype=np.int64)
    for p in range(DH):
        quad, j = divmod(p, 32)
        i = 16 * quad + (j % 16)
        orig[p] = 2 * i if j < 16 else 2 * i + 1
    return orig


def _build_program(B, S, D, HPC):
    """Build the per-core SPMD program. Returns compiled Bacc."""
    import concourse.mybir as mybir
    import concourse.tile as tile
    from concourse import bacc
    from contextlib import ExitStack

    F32 = mybir.dt.float32
    BF16 = mybir.dt.bfloat16
    AF = mybir.ActivationFunctionType
    OP = mybir.AluOpType

    T = B * S
    NKO = D // 128           # contraction chunks for projections
    QCOLS = 2 * HPC          # q + k col-tiles of 128
    VCOLS = HPC * 128
    WCOLS = QCOLS * 128 + VCOLS
    NQI = S // TOK           # q tiles per (b,h)
    NDC = TOK // 128         # 128-chunks per token tile (diag masks)
    NDO = D // TOK           # output Dout tiles
    NKV = S // 128           # v chunks per batch
    scale = 1.0 / math.sqrt(DH)

    CB = WCOLS // 128        # weight column blocks (q,q,k,k,v,v)

    NTT = T // TOK           # token tiles overall

    nc = bacc.Bacc()
    xT = nc.dram_tensor("xT", [128, NTT, NKO, TOK], BF16,
                        kind="ExternalInput")
    w4 = nc.dram_tensor("w4", [128, CB, NKO, 128], BF16,
                        kind="ExternalInput")
    wout = nc.dram_tensor("wout", [VCOLS, D], BF16, kind="ExternalInput")
    cosP = nc.dram_tensor("cosP", [128, S], BF16, kind="ExternalInput")
    sinP = nc.dram_tensor("sinP", [128, S], BF16, kind="ExternalInput")
    madd = nc.dram_tensor("madd", [128, 128], BF16, kind="ExternalInput")
    ident = nc.dram_tensor("ident", [128, 128], BF16, kind="ExternalInput")
    ones = nc.dram_tensor("ones", [128, 128], BF16, kind="ExternalInput")
    qb = nc.dram_tensor("qb", [128, QCOLS], F32, kind="ExternalInput")
    vb = nc.dram_tensor("vb", [128, VCOLS], F32, kind="ExternalInput")
    y = nc.dram_tensor("y", [T, D], BF16, kind="ExternalOutput")

    wout_r = wout.rearrange("(h p) d -> p h d", p=128)

    with tile.TileContext(nc) as tc:
        with ExitStack() as ctx:
            s1 = ctx.enter_context(tc.tile_pool(name="singles", bufs=1))
            xp = ctx.enter_context(tc.tile_pool(name="xp", bufs=2))
            qkvp = ctx.enter_context(tc.tile_pool(name="qkvp", bufs=2))
            wk = ctx.enter_context(tc.tile_pool(name="wk", bufs=2))
            ptq = ctx.enter_context(tc.tile_pool(name="ptq", bufs=10))
            ysp = ctx.enter_context(tc.tile_pool(name="ysp", bufs=8))
            psA = ctx.enter_context(
                tc.tile_pool(name="psA", bufs=2, space="PSUM"))
            psB = ctx.enter_context(
                tc.tile_pool(name="psB", bufs=2, space="PSUM"))
            psC = ctx.enter_context(
                tc.tile_pool(name="psC", bufs=1, space="PSUM"))

            # ---- resident tensors -------------------------------------
            # Each dma_start costs ~650ns of issue time on its engine
            # queue, so startup keeps the sync queue to the critical path
            # (first weight column block + the first x tile) and routes
            # every bulk load through the scalar queue, which is idle at
            # startup.  The w4 host layout makes each column-block slice a
            # fully contiguous 4KB-per-partition transfer.
            xt00 = xp.tile([128, NKO, TOK], BF16, tag="xt")
            w_sb = s1.tile([128, CB, NKO, 128], BF16)
            qb_sb = s1.tile([128, QCOLS], F32)
            vb_sb = s1.tile([128, VCOLS], F32)
            cos_sb = s1.tile([128, S], BF16)
            sin_sb = s1.tile([128, S], BF16)
            # All large startup transfers share the sync hw queue: per-queue
            # transfers are FIFO, so consumption order is guaranteed and the
            # bulk loads cannot steal HBM bandwidth from the critical
            # opening x/weight stream (the tile scheduler reorders
            # instructions by dependency, so cross-queue ordering tricks
            # don't hold).  Only the tiny bias/constant loads use scalar.
            hk0 = NKO // 2
            nc.sync.dma_start(out=w_sb[:, 0, 0:hk0], in_=w4[:, 0, 0:hk0])
            qk = NKO // 4
            nc.sync.dma_start(out=xt00[:, 0:qk, :], in_=xT[:, 0, 0:qk, :])
            nc.sync.dma_start(out=w_sb[:, 0, hk0:NKO], in_=w4[:, 0, hk0:NKO])
            for i in range(1, 4):
                nc.sync.dma_start(out=xt00[:, i * qk:(i + 1) * qk, :],
                                  in_=xT[:, 0, i * qk:(i + 1) * qk, :])
            nc.sync.dma_start(out=w_sb[:, 1], in_=w4[:, 1])
            nc.sync.dma_start(out=cos_sb, in_=cosP[:, :])
            nc.sync.dma_start(out=sin_sb, in_=sinP[:, :])
            nc.sync.dma_start(out=w_sb[:, 2:4], in_=w4[:, 2:4])
            nc.sync.dma_start(out=w_sb[:, 4:CB], in_=w4[:, 4:CB])
            nc.scalar.dma_start(out=qb_sb, in_=qb[:, :])
            nc.scalar.dma_start(out=vb_sb, in_=vb[:, :])
            # allocated now, DMA'd after the first xt tile (see loop)
            wout_sb = s1.tile([128, HPC, D], BF16)
            madd_sb = s1.tile([128, 128], BF16)
            ident_sb = s1.tile([128, 128], BF16)
            ones_sb = s1.tile([128, 128], BF16)

            def emit_group(g, force_act=False):
                # one out-projection (qs, do) group: 2 accumulating matmuls
                # + a PSUM->SBUF cast on the chosen engine + the row DMA
                # after its last group.  Groups are drained one-or-two per
                # ki step of the NEXT query tile's attention loop so the
                # PE's spare time there absorbs them and the casts never
                # pace the pipeline.
                b0_, ot_, qs, do, ysb, eng = g
                if force_act:
                    eng = "act"
                yp = psA.tile([128, TOK], F32, tag="mm",
                              name=f"yp_{b0_}_{qs}_{do}")
                for h in range(HPC):
                    nc.tensor.matmul(
                        yp, ot_[:, h, qs * 128:(qs + 1) * 128],
                        wout_sb[:, h, do * TOK:(do + 1) * TOK],
                        start=(h == 0), stop=(h == HPC - 1))
                if eng == "act":
                    nc.scalar.activation(
                        ysb[:, do * TOK:(do + 1) * TOK], yp, AF.Copy)
                else:
                    nc.vector.tensor_copy(
                        ysb[:, do * TOK:(do + 1) * TOK], yp)
                if do == NDO - 1:
                    nc.sync.dma_start(
                        out=y[b0_ + qs * 128:b0_ + (qs + 1) * 128, :],
                        in_=ysb)

            def emit_qs_tail(b0_, ot_, qs):
                # kernel-tail row-tile: copies split DVE/ACT so neither
                # in-order queue paces the drain; DMA in halves so the last
                # transfer is small and overlaps the remaining copies
                ysb = ysp.tile([128, D], BF16, tag="ysb",
                               name=f"ysb_{b0_}_{qs}")
                for do in range(NDO):
                    yp = psA.tile([128, TOK], F32, tag="mm",
                                  name=f"yp_{b0_}_{qs}_{do}")
                    for h in range(HPC):
                        nc.tensor.matmul(
                            yp, ot_[:, h, qs * 128:(qs + 1) * 128],
                            wout_sb[:, h, do * TOK:(do + 1) * TOK],
                            start=(h == 0), stop=(h == HPC - 1))
                    if do % 2 == 0:
                        nc.vector.tensor_copy(
                            ysb[:, do * TOK:(do + 1) * TOK], yp)
                    else:
                        nc.scalar.activation(
                            ysb[:, do * TOK:(do + 1) * TOK], yp, AF.Copy)
                    if do % 2 == 1:
                        nc.sync.dma_start(
                            out=y[b0_ + qs * 128:b0_ + (qs + 1) * 128,
                                  (do - 1) * TOK:(do + 1) * TOK],
                            in_=ysb[:, (do - 1) * TOK:(do + 1) * TOK])

            equeue = []
            xt_pre = None
            for b in range(B):
                b0 = b * S
                qt = qkvp.tile([128, HPC, S], BF16, tag="qt")
                kt = qkvp.tile([128, HPC, S], BF16, tag="kt")
                vt = qkvp.tile([128, NKV, VCOLS], BF16, tag="vt")
                ot = qkvp.tile([128, HPC, S], BF16, tag="ot")
                for t in range(NQI):
                    # ---- A(t): QKV projection + RoPE for token tile t ----
                    tg = b0 + t * TOK
                    if b == 0 and t == 0:
                        xt = xt00  # prefetched before the resident loads
                    elif xt_pre is not None:
                        xt = xt_pre  # prefetched during previous B-phase
                        xt_pre = None
                    else:
                        xt = xp.tile([128, NKO, TOK], BF16, tag="xt")
                        nc.scalar.dma_start(out=xt, in_=xT[:, tg // TOK])
                    for c4 in range(QCOLS):
                        acc = psA.tile([128, TOK], F32, tag="mm")
                        for ko in range(NKO):
                            nc.tensor.matmul(
                                acc, w_sb[:, c4, ko, :],
                                xt[:, ko, :],
                                start=(ko == 0), stop=(ko == NKO - 1))
                        raw = wk.tile([128, TOK], BF16, tag="raw")
                        nc.scalar.activation(raw, acc, AF.Identity,
                                             bias=qb_sb[:, c4:c4 + 1])
                        if b == 0 and t == 0 and c4 == 0:
                            # non-critical constants + wout: last in the
                            # sync queue / tiny ones on scalar
                            nc.scalar.dma_start(out=madd_sb, in_=madd[:, :])
                            nc.scalar.dma_start(out=ident_sb,
                                                in_=ident[:, :])
                            nc.scalar.dma_start(out=ones_sb, in_=ones[:, :])
                            nc.sync.dma_start(out=wout_sb, in_=wout_r)
                        sw = wk.tile([128, TOK], BF16, tag="sw")
                        # partition-only permute: bitcast to u32 halves the
                        # streamed element count (pairs along free dim)
                        nc.vector.stream_shuffle(
                            sw.bitcast(mybir.dt.uint32),
                            raw.bitcast(mybir.dt.uint32), SHUF_MASK)
                        m1 = wk.tile([128, TOK], BF16, tag="m1")
                        nc.vector.tensor_tensor(
                            m1, raw, cos_sb[:, t * TOK:(t + 1) * TOK],
                            op=OP.mult)
                        m2 = wk.tile([128, TOK], BF16, tag="m2")
                        nc.vector.tensor_tensor(
                            m2, sw, sin_sb[:, t * TOK:(t + 1) * TOK],
                            op=OP.mult)
                        dst = qt if c4 < HPC else kt
                        nc.vector.tensor_tensor(
                            dst[:, c4 % HPC, t * TOK:(t + 1) * TOK], m1, m2,
                            op=OP.add)
                    for sub in range(NDC):
                        accv = psA.tile([128, VCOLS], F32, tag="mm")
                        for ko in range(NKO):
                            nc.tensor.matmul(
                                accv, xt[:, ko, sub * 128:(sub + 1) * 128],
                                w_sb[:, QCOLS:CB, ko, :],
                                start=(ko == 0), stop=(ko == NKO - 1))
                        nc.vector.tensor_tensor(
                            vt[:, t * NDC + sub, :], accv, vb_sb, op=OP.add)

                # ---- B: attention per query tile (heads interleaved),
                # ---- each followed by its out-projection row-tiles (C)
                for qi in range(NQI):
                    q0 = qi * TOK
                    nki = NDC * qi + NDC
                    avs, lpss = [], []
                    for h in range(HPC):
                        av_h = psC.tile([128, TOK], F32, tag=f"av{h}",
                                        name=f"av{h}_{b}_{qi}")
                        lps_h = psC.tile([128, TOK], F32, tag=f"lps{h}",
                                         name=f"lps{h}_{b}_{qi}")
                        avs.append(av_h)
                        lpss.append(lps_h)
                    pend = [[], []]          # ungrouped non-diag pt, per head
                    pend_d = [None, None]    # diag accumulation base, per head
                    lps_open = [False] * HPC
                    nd_flushed = [0] * HPC   # non-diag chunks already summed
                    groups = equeue          # previous qi's out-proj groups
                    equeue = []
                    ndrained = 0
                    def av_lps(ki, dg, qoff, N, pts):
                        # AV + softmax-denominator processing for chunk ki
                        # (runs one ki behind the score/exp pair so the AV
                        # matmuls never expose the exp latency)
                        for h in range(HPC):
                            nc.tensor.matmul(
                                avs[h][:, qoff:TOK],
                                vt[:, ki, h * 128:(h + 1) * 128],
                                pts[h][:, :N],
                                start=(ki == 0), stop=(ki == nki - 1))
                        for h in range(HPC):
                            pt = pts[h]
                            if dg < 0:
                                # non-diag chunks: pre-sum groups of 8 (or 4
                                # for the remainder) in place on DVE (bf16)
                                # so only one ones-matmul streams per group
                                pend[h].append(pt)
                                rem = 4 * qi - nd_flushed[h]
                                gsz = 8 if rem >= 8 else 4
                                if len(pend[h]) < gsz:
                                    continue
                                cur = pend[h]
                                while len(cur) > 1:
                                    nxt = []
                                    for i in range(0, len(cur), 2):
                                        nc.vector.tensor_tensor(
                                            cur[i], cur[i], cur[i + 1],
                                            op=OP.add)
                                        nxt.append(cur[i])
                                    cur = nxt
                                nc.tensor.matmul(
                                    lpss[h][:, :TOK], ones_sb, cur[0],
                                    start=(not lps_open[h]), stop=False)
                                pend[h] = []
                                nd_flushed[h] += gsz
                                lps_open[h] = True
                            elif dg == 0:
                                pend_d[h] = pt
                            else:
                                # diag chunks accumulate into dg0's buffer
                                # in place; one ones-matmul streams the full
                                # TOK range at the last chunk
                                base = pend_d[h]
                                nc.vector.tensor_tensor(
                                    base[:, qoff:TOK], base[:, qoff:TOK],
                                    pt[:, :N], op=OP.add)
                                if dg == NDC - 1:
                                    pend_d[h] = None
                                    nc.tensor.matmul(
                                        lpss[h][:, :TOK], ones_sb,
                                        base[:, :TOK],
                                        start=(not lps_open[h]), stop=True)
                                    lps_open[h] = True

                    prev = None
                    for ki in range(nki):
                        dg = ki - NDC * qi
                        qoff = max(dg, 0) * 128
                        N = TOK - qoff
                        pts = []
                        for h in range(HPC):
                            # both heads' score matmuls + exps issued first so
                            # ACT gets the pair ASAP and each head's PV work
                            # overlaps the other head's exp
                            st = psB.tile([128, TOK], F32, tag="st")
                            if dg >= 0:
                                # causal mask folded into the score psum: an
                                # identity-stationary matmul adds -1e9 above
                                # the diagonal of the leading 128-block, so
                                # exp emits exact zeros there and the DVE
                                # mask multiply disappears from the st->av
                                # chain
                                nc.tensor.matmul(
                                    st[:, :N],
                                    kt[:, h, ki * 128:(ki + 1) * 128],
                                    qt[:, h, q0 + qoff:q0 + TOK],
                                    start=True, stop=False)
                                nc.tensor.matmul(
                                    st[:, 0:128], ident_sb, madd_sb,
                                    start=False, stop=True)
                            else:
                                nc.tensor.matmul(
                                    st[:, :N],
                                    kt[:, h, ki * 128:(ki + 1) * 128],
                                    qt[:, h, q0 + qoff:q0 + TOK],
                                    start=True, stop=True)
                            pt = ptq.tile([128, TOK], BF16, tag="pt",
                                          name=f"pt_{b}_{qi}_{ki}_{h}")
                            nc.scalar.activation(pt[:, :N], st[:, :N], AF.Exp,
                                                 scale=scale)
                            pts.append(pt)
                        if prev is not None:
                            av_lps(*prev)
                        # drain out-proj groups of the previous query tile,
                        # shifted one ki late: the first groups' ot chunks
                        # are still in the previous qi's reciprocal chain
                        # at ki=0
                        tgt = (len(groups) * ki) // nki
                        while ndrained < tgt:
                            emit_group(groups[ndrained])
                            ndrained += 1
                        prev = (ki, dg, qoff, N, pts)
                    if prev is not None:
                        av_lps(*prev)
                    while ndrained < len(groups):
                        emit_group(groups[ndrained])
                        ndrained += 1
                    if b == B - 1 and qi == NQI - 1:
                        # kernel tail: no later compute hides this chain, so
                        # chunk the reciprocal/divide per 128-query block and
                        # emit each row-tile as soon as its block is ready
                        # (ACT copies + split DMA drain)
                        for sub in range(NDC):
                            c0 = sub * 128
                            for h in range(HPC):
                                recl = wk.tile([128, 128], F32, tag="reclc")
                                nc.vector.reciprocal_approx_fast(
                                    recl, lpss[h][:, c0:c0 + 128])
                                nc.vector.tensor_tensor(
                                    ot[:, h, q0 + c0:q0 + c0 + 128],
                                    avs[h][:, c0:c0 + 128], recl, op=OP.mult)
                            emit_qs_tail(b0, ot, qi * NDC + sub)
                        continue
                    # reciprocal/divide split so the first 128-query block's
                    # ot lands early: the first out-proj groups drained in
                    # the next qi's loop only wait ~1us, not the full chain
                    for c0, cw in ((0, 128), (128, TOK - 128)):
                        for h in range(HPC):
                            recl = wk.tile([128, cw], F32,
                                           tag=f"recl{cw}")
                            nc.vector.reciprocal_approx_fast(
                                recl, lpss[h][:, c0:c0 + cw])
                            nc.vector.tensor_tensor(
                                ot[:, h, q0 + c0:q0 + c0 + cw],
                                avs[h][:, c0:c0 + cw], recl, op=OP.mult)
                    # enqueue this qi's out-projection groups; they drain
                    # through the next query tile's (or next batch's first)
                    # attention loop.  Cast-engine split: the drain context
                    # for qi 3 and 0 is exp-light (few ki steps / next
                    # batch's qi=0), so ACT takes more copies there.
                    d_qi = (qi + 1) % NQI
                    n_act = 2 if d_qi == 0 else 1
                    for qs in range(qi * NDC, (qi + 1) * NDC):
                        ysb = ysp.tile([128, D], BF16, tag="ysb",
                                       name=f"ysb_{b0}_{qs}")
                        for do in range(NDO):
                            eng = "act" if do < n_act else "dve"
                            equeue.append((b0, ot, qs, do, ysb, eng))
                    if qi == NQI - 2 and b < B - 1:
                        # prefetch next batch's first x tile during this
                        # B-phase so the batch boundary never waits on DMA
                        xt_pre = xp.tile([128, NKO, TOK], BF16, tag="xt")
                        nc.sync.dma_start(out=xt_pre,
                                          in_=xT[:, (b + 1) * S // TOK])

            while equeue:
                emit_group(equeue.pop(0))

    nc.compile()
    return nc


def _host_prep(x, rope_cos, rope_sin, Wqkv, bqkv, Wout, B, S, D, H, n_cores):
    """Build per-core input maps (bf16 data, fp32 biases)."""
    import ml_dtypes
    BF = ml_dtypes.bfloat16

    T = B * S
    HPC = H // n_cores
    orig = _perm_orig_of_p()
    quad_j = np.arange(DH)
    jmod = quad_j % 32
    i_of_p = (quad_j // 32) * 16 + (jmod % 16)
    sign = np.where(jmod < 16, -1.0, 1.0).astype(np.float32)

    # [p, token-tile, ko, tok]: every x-tile DMA slice is fully contiguous
    # per partition (16KB lines)
    NTT = T // 512
    xT = np.ascontiguousarray(
        x.reshape(NTT, 512, D // 128, 128).transpose(3, 0, 2, 1).astype(BF))
    cosP = np.ascontiguousarray(rope_cos[:, i_of_p].T.astype(BF))
    sinP = np.ascontiguousarray((rope_sin[:, i_of_p] * sign).T.astype(BF))

    pl = np.arange(128)[:, None]
    ql = np.arange(128)[None, :]
    # additive causal mask for the 128-wide diagonal block: 0 at/below the
    # diagonal (key p <= query q), -1e9 above (exp -> exact 0)
    madd = np.ascontiguousarray(
        np.where(pl <= ql, 0.0, -1e9).astype(BF))  # [128, 128]
    ident = np.ascontiguousarray(np.eye(128, dtype=np.float32).astype(BF))

    ones = np.ones((128, 128), dtype=BF)

    NKO = D // 128
    in_maps = []
    for c in range(n_cores):
        heads = [c * HPC + i for i in range(HPC)]
        wq = [Wqkv[:, h * DH + orig] for h in heads]
        wk = [Wqkv[:, H * DH + h * DH + orig] for h in heads]
        wv = [Wqkv[:, 2 * H * DH + h * DH:2 * H * DH + (h + 1) * DH]
              for h in heads]
        w_c = np.concatenate(wq + wk + wv, axis=1)  # [D, WCOLS]
        CB = w_c.shape[1] // 128
        # [p, cb, ko, 128]: each column-block slice is a contiguous
        # 4KB-per-partition DMA
        w4 = np.ascontiguousarray(
            w_c.reshape(NKO, 128, CB, 128).transpose(1, 2, 0, 3).astype(BF))
        wout_c = np.ascontiguousarray(
            Wout[c * HPC * DH:(c + 1) * HPC * DH, :].astype(BF))
        qb_cols = ([bqkv[h * DH + orig] for h in heads] +
                   [bqkv[H * DH + h * DH + orig] for h in heads])
        qb = np.ascontiguousarray(np.stack(qb_cols, axis=1).astype(np.float32))
        vb_flat = np.concatenate(
            [bqkv[2 * H * DH + h * DH:2 * H * DH + (h + 1) * DH]
             for h in heads])
        vb = np.ascontiguousarray(
            np.broadcast_to(vb_flat[None, :], (128, HPC * DH)).astype(
                np.float32))
        in_maps.append({
            "xT": xT, "w4": w4, "wout": wout_c, "cosP": cosP, "sinP": sinP,
            "madd": madd, "ident": ident, "ones": ones, "qb": qb, "vb": vb,
        })
    return in_maps


def _run(x, rope_cos, rope_sin, Wqkv, bqkv, Wout, bout,
         B, S, D, H, n_cores, trace=False):
    _ensure_imports()
    from concourse.bass_utils import run_bass_kernel_spmd

    HPC = H // n_cores
    import time as _time
    _t0 = _time.time()
    nc = _build_program(B, S, D, HPC)
    print(f"[kernel] build+compile wall: {_time.time() - _t0:.1f}s", flush=True)
    in_maps = _host_prep(np.asarray(x, dtype=np.float32),
                         np.asarray(rope_cos, dtype=np.float32),
                         np.asarray(rope_sin, dtype=np.float32),
                         np.asarray(Wqkv, dtype=np.float32),
                         np.asarray(bqkv, dtype=np.float32),
                         np.asarray(Wout, dtype=np.float32),
                         B, S, D, H, n_cores)
    _t0 = _time.time()
    res = run_bass_kernel_spmd(nc, in_maps, list(range(n_cores)), trace=trace)
    print(f"[kernel] spmd run wall: {_time.time() - _t0:.1f}s", flush=True)
    y = res.results[0]["y"].astype(np.float64)
    for i in range(1, n_cores):
        y += res.results[i]["y"]
    y += np.asarray(bout, dtype=np.float64)[None, :]
    out = y.astype(np.float32).reshape(B, S, D)
    return out, res


def kernel(x, rope_cos, rope_sin, Wqkv, bqkv, Wout, bout):
    out, _ = _run(x, rope_cos, rope_sin, Wqkv, bqkv, Wout, bout,
                  B=4, S=2048, D=2048, H=16, n_cores=8)
    return out



# revision 45
# speedup vs baseline: 1.0566x; 1.0049x over previous
"""Causal self-attention (RoPE, 16 heads) on 8 TRN2 NeuronCores.

Problem: x[4,2048,2048] @ Wqkv -> RoPE(q,k) -> causal softmax(qk^T/sqrt(128)) @ v
         -> out proj Wout.  B=4, S=2048, D=2048, H=16, DH=128.

Sharding: tensor-parallel over heads. Each of the 8 cores computes 2 heads:
QKV projection columns for its heads, RoPE, attention, and its partial of the
output projection (row-sharded Wout). Host sums the 8 partials (+bout).

Design (vs the 903us v1 two-phase fp32r baseline; ~666us fast-mode):
  * bf16 operands everywhere (fp32 PSUM accumulation) — same PE rate as
    float32r, but cheaper weight loads, 2x DVE elementwise, half the DMA.
  * Fully fused per-batch pipeline: QKV+RoPE -> attention -> out-proj with
    Q^T/K^T/V/O^T resident in SBUF (no DRAM scratch round trip, no phase
    barrier, no per-head reload stalls).
  * Causal trimming: for the diagonal 128-k chunk dg, the moving q-range
    starts at dg*128 (N in {512,384,256,128}); only the leading 128-wide
    diagonal block needs the 0/1 mask (applied in place on DVE).
  * Attention latency chain (st -> exp on ACT -> av) hidden by interleaving
    both heads' chains per ki step and emitting the out-projection row-tiles
    of query tile qi right after qi completes (PE filler work); each batch's
    last out-proj group is deferred past the next batch's first QKV tile so
    the boundary always has ready PE work.
  * Softmax denominator via ones-matmul into PSUM; non-diagonal exp chunks
    are pre-summed in groups of four on DVE (bf16) and diagonal chunks in
    overlapping pairs (dg1 into dg0's q-range in place, dg3 into dg2's), so
    ~1/4 as many ones-matmuls stream through the PE.
  * reciprocal_approx_fast for 1/l (~5x faster than DVE reciprocal);
    PSUM->SBUF y copies on DVE as fp32->bf16 casts (keeps the in-order ACT
    queue free for exp); y partials in bf16, summed on host in fp64.
  * Startup: first x tile + first weight quarter interleaved in small DMAs
    ahead of all bulk loads (first matmul at ~14us instead of ~38us).
PSUM budget (8 banks): mm(2, shared QKV-acc/out-proj) + st(2) +
  av_h0/av_h1/lps_h0/lps_h1 (1 each).
"""

import math

import numpy as np


def _ensure_imports():
    try:
        import concourse.bass  # noqa: F401
    except ImportError:
        import sys
        for p in (
            "/root/.axon_site",
            "/root/.axon_site/_ro/trn_rl_repo",
            "/root/.axon_site/_ro/pypackages",
            "/opt/trn_rl_repo",
        ):
            if p not in sys.path:
                sys.path.append(p)


DH = 128
TOK = 512            # token tile (matmul moving free dim)
SHUF_MASK = [(i + 16) % 32 for i in range(32)]


def _perm_orig_of_p():
    """orig head-dim index stored at partition p, for the RoPE layout.

    Partition p = 32*quad + j. Rotation pair index i = 16*quad + (j % 16).
    j < 16 holds the even element (2i), j >= 16 holds the odd (2i+1).
    """
    orig = np.empty(DH, dtype=np.int64)
    for p in range(DH):
        quad, j = divmod(p, 32)
        i = 16 * quad + (j % 16)
        orig[p] = 2 * i if j < 16 else 2 * i + 1
    return orig


def _build_program(B, S, D, HPC):
    """Build the per-core SPMD program. Returns compiled Bacc."""
    import concourse.mybir as mybir
    import concourse.tile as tile
    from concourse import bacc
    from contextlib import ExitStack

    F32 = mybir.dt.float32
    BF16 = mybir.dt.bfloat16
    AF = mybir.ActivationFunctionType
    OP = mybir.AluOpType

    T = B * S
    NKO = D // 128           # contraction chunks for projections
    QCOLS = 2 * HPC          # q + k col-tiles of 128
    VCOLS = HPC * 128
    WCOLS = QCOLS * 128 + VCOLS
    NQI = S // TOK           # q tiles per (b,h)
    NDC = TOK // 128         # 128-chunks per token tile (diag masks)
    NDO = D // TOK           # output Dout tiles
    NKV = S // 128           # v chunks per batch
    scale = 1.0 / math.sqrt(DH)

    CB = WCOLS // 128        # weight column blocks (q,q,k,k,v,v)

    NTT = T // TOK           # token tiles overall

    nc = bacc.Bacc()
    xT = nc.dram_tensor("xT", [128, NTT, NKO, TOK], BF16,
                        kind="ExternalInput")
    w4 = nc.dram_tensor("w4", [128, CB, NKO, 128], BF16,
                        kind="ExternalInput")
    wout = nc.dram_tensor("wout", [VCOLS, D], BF16, kind="ExternalInput")
    cosP = nc.dram_tensor("cosP", [128, S], BF16, kind="ExternalInput")
    sinP = nc.dram_tensor("sinP", [128, S], BF16, kind="ExternalInput")
    madd = nc.dram_tensor("madd", [128, 128], BF16, kind="ExternalInput")
    ident = nc.dram_tensor("ident", [128, 128], BF16, kind="ExternalInput")
    ones = nc.dram_tensor("ones", [128, 128], BF16, kind="ExternalInput")
    qb = nc.dram_tensor("qb", [128, QCOLS], F32, kind="ExternalInput")
    vb = nc.dram_tensor("vb", [128, VCOLS], F32, kind="ExternalInput")
    y = nc.dram_tensor("y", [T, D], BF16, kind="ExternalOutput")

    wout_r = wout.rearrange("(h p) d -> p h d", p=128)

    with tile.TileContext(nc) as tc:
        with ExitStack() as ctx:
            s1 = ctx.enter_context(tc.tile_pool(name="singles", bufs=1))
            xp = ctx.enter_context(tc.tile_pool(name="xp", bufs=2))
            qkvp = ctx.enter_context(tc.tile_pool(name="qkvp", bufs=2))
            wk = ctx.enter_context(tc.tile_pool(name="wk", bufs=2))
            ptq = ctx.enter_context(tc.tile_pool(name="ptq", bufs=10))
            ysp = ctx.enter_context(tc.tile_pool(name="ysp", bufs=8))
            psA = ctx.enter_context(
                tc.tile_pool(name="psA", bufs=2, space="PSUM"))
            psB = ctx.enter_context(
                tc.tile_pool(name="psB", bufs=2, space="PSUM"))
            psC = ctx.enter_context(
                tc.tile_pool(name="psC", bufs=1, space="PSUM"))

            # ---- resident tensors -------------------------------------
            # Each dma_start costs ~650ns of issue time on its engine
            # queue, so startup keeps the sync queue to the critical path
            # (first weight column block + the first x tile) and routes
            # every bulk load through the scalar queue, which is idle at
            # startup.  The w4 host layout makes each column-block slice a
            # fully contiguous 4KB-per-partition transfer.
            xt00 = xp.tile([128, NKO, TOK], BF16, tag="xt")
            w_sb = s1.tile([128, CB, NKO, 128], BF16)
            qb_sb = s1.tile([128, QCOLS], F32)
            vb_sb = s1.tile([128, VCOLS], F32)
            cos_sb = s1.tile([128, S], BF16)
            sin_sb = s1.tile([128, S], BF16)
            # All large startup transfers share the sync hw queue: per-queue
            # transfers are FIFO, so consumption order is guaranteed and the
            # bulk loads cannot steal HBM bandwidth from the critical
            # opening x/weight stream (the tile scheduler reorders
            # instructions by dependency, so cross-queue ordering tricks
            # don't hold).  Only the tiny bias/constant loads use scalar.
            hk0 = NKO // 2
            nc.sync.dma_start(out=w_sb[:, 0, 0:hk0], in_=w4[:, 0, 0:hk0])
            qk = NKO // 4
            nc.sync.dma_start(out=xt00[:, 0:qk, :], in_=xT[:, 0, 0:qk, :])
            nc.sync.dma_start(out=w_sb[:, 0, hk0:NKO], in_=w4[:, 0, hk0:NKO])
            for i in range(1, 4):
                nc.sync.dma_start(out=xt00[:, i * qk:(i + 1) * qk, :],
                                  in_=xT[:, 0, i * qk:(i + 1) * qk, :])
            nc.sync.dma_start(out=w_sb[:, 1], in_=w4[:, 1])
            nc.sync.dma_start(out=cos_sb, in_=cosP[:, :])
            nc.sync.dma_start(out=sin_sb, in_=sinP[:, :])
            nc.sync.dma_start(out=w_sb[:, 2:4], in_=w4[:, 2:4])
            nc.sync.dma_start(out=w_sb[:, 4:CB], in_=w4[:, 4:CB])
            nc.scalar.dma_start(out=qb_sb, in_=qb[:, :])
            nc.scalar.dma_start(out=vb_sb, in_=vb[:, :])
            # allocated now, DMA'd after the first xt tile (see loop)
            wout_sb = s1.tile([128, HPC, D], BF16)
            madd_sb = s1.tile([128, 128], BF16)
            ident_sb = s1.tile([128, 128], BF16)
            ones_sb = s1.tile([128, 128], BF16)

            def emit_group(g, force_act=False):
                # one out-projection (qs, do) group: 2 accumulating matmuls
                # + a PSUM->SBUF cast on the chosen engine + the row DMA
                # after its last group.  Groups are drained one-or-two per
                # ki step of the NEXT query tile's attention loop so the
                # PE's spare time there absorbs them and the casts never
                # pace the pipeline.
                b0_, ot_, qs, do, ysb, eng = g
                if force_act:
                    eng = "act"
                yp = psA.tile([128, TOK], F32, tag="mm",
                              name=f"yp_{b0_}_{qs}_{do}")
                for h in range(HPC):
                    nc.tensor.matmul(
                        yp, ot_[:, h, qs * 128:(qs + 1) * 128],
                        wout_sb[:, h, do * TOK:(do + 1) * TOK],
                        start=(h == 0), stop=(h == HPC - 1))
                if eng == "act":
                    nc.scalar.activation(
                        ysb[:, do * TOK:(do + 1) * TOK], yp, AF.Copy)
                else:
                    nc.vector.tensor_copy(
                        ysb[:, do * TOK:(do + 1) * TOK], yp)
                if do == NDO - 1:
                    nc.sync.dma_start(
                        out=y[b0_ + qs * 128:b0_ + (qs + 1) * 128, :],
                        in_=ysb)

            def emit_qs_tail(b0_, ot_, qs):
                # kernel-tail row-tile: copies split DVE/ACT so neither
                # in-order queue paces the drain; DMA in halves so the last
                # transfer is small and overlaps the remaining copies
                ysb = ysp.tile([128, D], BF16, tag="ysb",
                               name=f"ysb_{b0_}_{qs}")
                for do in range(NDO):
                    yp = psA.tile([128, TOK], F32, tag="mm",
                                  name=f"yp_{b0_}_{qs}_{do}")
                    for h in range(HPC):
                        nc.tensor.matmul(
                            yp, ot_[:, h, qs * 128:(qs + 1) * 128],
                            wout_sb[:, h, do * TOK:(do + 1) * TOK],
                            start=(h == 0), stop=(h == HPC - 1))
                    if do % 2 == 0:
                        nc.vector.tensor_copy(
                            ysb[:, do * TOK:(do + 1) * TOK], yp)
                    else:
                        nc.scalar.activation(
                            ysb[:, do * TOK:(do + 1) * TOK], yp, AF.Copy)
                    if do % 2 == 1:
                        nc.sync.dma_start(
                            out=y[b0_ + qs * 128:b0_ + (qs + 1) * 128,
                                  (do - 1) * TOK:(do + 1) * TOK],
                            in_=ysb[:, (do - 1) * TOK:(do + 1) * TOK])

            equeue = []
            xt_pre = None
            for b in range(B):
                b0 = b * S
                qt = qkvp.tile([128, HPC, S], BF16, tag="qt")
                kt = qkvp.tile([128, HPC, S], BF16, tag="kt")
                vt = qkvp.tile([128, NKV, VCOLS], BF16, tag="vt")
                ot = qkvp.tile([128, HPC, S], BF16, tag="ot")
                for t in range(NQI):
                    # ---- A(t): QKV projection + RoPE for token tile t ----
                    tg = b0 + t * TOK
                    if b == 0 and t == 0:
                        xt = xt00  # prefetched before the resident loads
                    elif xt_pre is not None:
                        xt = xt_pre  # prefetched during previous B-phase
                        xt_pre = None
                    else:
                        xt = xp.tile([128, NKO, TOK], BF16, tag="xt")
                        nc.scalar.dma_start(out=xt, in_=xT[:, tg // TOK])
                    for c4 in range(QCOLS):
                        acc = psA.tile([128, TOK], F32, tag="mm")
                        for ko in range(NKO):
                            nc.tensor.matmul(
                                acc, w_sb[:, c4, ko, :],
                                xt[:, ko, :],
                                start=(ko == 0), stop=(ko == NKO - 1))
                        raw = wk.tile([128, TOK], BF16, tag="raw")
                        nc.scalar.activation(raw, acc, AF.Identity,
                                             bias=qb_sb[:, c4:c4 + 1])
                        if b == 0 and t == 0 and c4 == 0:
                            # non-critical constants + wout: last in the
                            # sync queue / tiny ones on scalar
                            nc.scalar.dma_start(out=madd_sb, in_=madd[:, :])
                            nc.scalar.dma_start(out=ident_sb,
                                                in_=ident[:, :])
                            nc.scalar.dma_start(out=ones_sb, in_=ones[:, :])
                            nc.sync.dma_start(out=wout_sb, in_=wout_r)
                        sw = wk.tile([128, TOK], BF16, tag="sw")
                        # partition-only permute: bitcast to u32 halves the
                        # streamed element count (pairs along free dim)
                        nc.vector.stream_shuffle(
                            sw.bitcast(mybir.dt.uint32),
                            raw.bitcast(mybir.dt.uint32), SHUF_MASK)
                        m1 = wk.tile([128, TOK], BF16, tag="m1")
                        nc.vector.tensor_tensor(
                            m1, raw, cos_sb[:, t * TOK:(t + 1) * TOK],
                            op=OP.mult)
                        m2 = wk.tile([128, TOK], BF16, tag="m2")
                        nc.vector.tensor_tensor(
                            m2, sw, sin_sb[:, t * TOK:(t + 1) * TOK],
                            op=OP.mult)
                        dst = qt if c4 < HPC else kt
                        nc.vector.tensor_tensor(
                            dst[:, c4 % HPC, t * TOK:(t + 1) * TOK], m1, m2,
                            op=OP.add)
                    for sub in range(NDC):
                        accv = psA.tile([128, VCOLS], F32, tag="mm")
                        for ko in range(NKO):
                            nc.tensor.matmul(
                                accv, xt[:, ko, sub * 128:(sub + 1) * 128],
                                w_sb[:, QCOLS:CB, ko, :],
                                start=(ko == 0), stop=(ko == NKO - 1))
                        nc.vector.tensor_tensor(
                            vt[:, t * NDC + sub, :], accv, vb_sb, op=OP.add)

                # ---- B: attention per query tile (heads interleaved),
                # ---- each followed by its out-projection row-tiles (C)
                for qi in range(NQI):
                    q0 = qi * TOK
                    nki = NDC * qi + NDC
                    avs, lpss = [], []
                    for h in range(HPC):
                        av_h = psC.tile([128, TOK], F32, tag=f"av{h}",
                                        name=f"av{h}_{b}_{qi}")
                        lps_h = psC.tile([128, TOK], F32, tag=f"lps{h}",
                                         name=f"lps{h}_{b}_{qi}")
                        avs.append(av_h)
                        lpss.append(lps_h)
                    pend = [[], []]          # ungrouped non-diag pt, per head
                    pend_d = [None, None]    # diag accumulation base, per head
                    lps_open = [False] * HPC
                    nd_flushed = [0] * HPC   # non-diag chunks already summed
                    groups = equeue          # previous qi's out-proj groups
                    equeue = []
                    ndrained = 0
                    def av_lps(ki, dg, qoff, N, pts):
                        # AV + softmax-denominator processing for chunk ki
                        # (runs one ki behind the score/exp pair so the AV
                        # matmuls never expose the exp latency)
                        for h in range(HPC):
                            nc.tensor.matmul(
                                avs[h][:, qoff:TOK],
                                vt[:, ki, h * 128:(h + 1) * 128],
                                pts[h][:, :N],
                                start=(ki == 0), stop=(ki == nki - 1))
                        for h in range(HPC):
                            pt = pts[h]
                            if dg < 0:
                                # non-diag chunks: pre-sum groups of 8 (or 4
                                # for the remainder) in place on DVE (bf16)
                                # so only one ones-matmul streams per group
                                pend[h].append(pt)
                                rem = 4 * qi - nd_flushed[h]
                                gsz = 8 if rem >= 8 else 4
                                if len(pend[h]) < gsz:
                                    continue
                                cur = pend[h]
                                while len(cur) > 1:
                                    nxt = []
                                    for i in range(0, len(cur), 2):
                                        nc.vector.tensor_tensor(
                                            cur[i], cur[i], cur[i + 1],
                                            op=OP.add)
                                        nxt.append(cur[i])
                                    cur = nxt
                                nc.tensor.matmul(
                                    lpss[h][:, :TOK], ones_sb, cur[0],
                                    start=(not lps_open[h]), stop=False)
                                pend[h] = []
                                nd_flushed[h] += gsz
                                lps_open[h] = True
                            elif dg == 0:
                                pend_d[h] = pt
                            else:
                                # diag chunks accumulate into dg0's buffer
                                # in place; one ones-matmul streams the full
                                # TOK range at the last chunk
                                base = pend_d[h]
                                nc.vector.tensor_tensor(
                                    base[:, qoff:TOK], base[:, qoff:TOK],
                                    pt[:, :N], op=OP.add)
                                if dg == NDC - 1:
                                    pend_d[h] = None
                                    nc.tensor.matmul(
                                        lpss[h][:, :TOK], ones_sb,
                                        base[:, :TOK],
                                        start=(not lps_open[h]), stop=True)
                                    lps_open[h] = True

                    prev = None
                    for ki in range(nki):
                        dg = ki - NDC * qi
                        qoff = max(dg, 0) * 128
                        N = TOK - qoff
                        pts = []
                        for h in range(HPC):
                            # both heads' score matmuls + exps issued first so
                            # ACT gets the pair ASAP and each head's PV work
                            # overlaps the other head's exp
                            st = psB.tile([128, TOK], F32, tag="st")
                            if dg >= 0:
                                # causal mask folded into the score psum: an
                                # identity-stationary matmul adds -1e9 above
                                # the diagonal of the leading 128-block, so
                                # exp emits exact zeros there and the DVE
                                # mask multiply disappears from the st->av
                                # chain
                                nc.tensor.matmul(
                                    st[:, :N],
                                    kt[:, h, ki * 128:(ki + 1) * 128],
                                    qt[:, h, q0 + qoff:q0 + TOK],
                                    start=True, stop=False)
                                nc.tensor.matmul(
                                    st[:, 0:128], ident_sb, madd_sb,
                                    start=False, stop=True)
                            else:
                                nc.tensor.matmul(
                                    st[:, :N],
                                    kt[:, h, ki * 128:(ki + 1) * 128],
                                    qt[:, h, q0 + qoff:q0 + TOK],
                                    start=True, stop=True)
                            pt = ptq.tile([128, TOK], BF16, tag="pt",
                                          name=f"pt_{b}_{qi}_{ki}_{h}")
                            nc.scalar.activation(pt[:, :N], st[:, :N], AF.Exp,
                                                 scale=scale)
                            pts.append(pt)
                        if prev is not None:
                            av_lps(*prev)
                        # drain out-proj groups of the previous query tile,
                        # shifted one ki late: the first groups' ot chunks
                        # are still in the previous qi's reciprocal chain
                        # at ki=0
                        last_qi = b == B - 1 and qi == NQI - 1
                        cap = len(groups) - 4 if last_qi else len(groups)
                        tgt = min((len(groups) * ki) // nki, cap)
                        while ndrained < tgt:
                            emit_group(groups[ndrained])
                            ndrained += 1
                        prev = (ki, dg, qoff, N, pts)
                    if prev is not None:
                        av_lps(*prev)
                    # remainder groups: in the final qi four are held back
                    # and emitted here with ACT casts, so the PE has work
                    # while DVE runs the tail lps/reciprocal chain
                    while ndrained < len(groups):
                        emit_group(groups[ndrained],
                                   force_act=(b == B - 1 and qi == NQI - 1))
                        ndrained += 1
                    if b == B - 1 and qi == NQI - 1:
                        # kernel tail: no later compute hides this chain, so
                        # chunk the reciprocal/divide per 128-query block and
                        # emit each row-tile as soon as its block is ready
                        # (ACT copies + split DMA drain)
                        for sub in range(NDC):
                            c0 = sub * 128
                            for h in range(HPC):
                                recl = wk.tile([128, 128], F32, tag="reclc")
                                nc.vector.reciprocal_approx_fast(
                                    recl, lpss[h][:, c0:c0 + 128])
                                nc.vector.tensor_tensor(
                                    ot[:, h, q0 + c0:q0 + c0 + 128],
                                    avs[h][:, c0:c0 + 128], recl, op=OP.mult)
                            emit_qs_tail(b0, ot, qi * NDC + sub)
                        continue
                    # reciprocal/divide split so the first 128-query block's
                    # ot lands early: the first out-proj groups drained in
                    # the next qi's loop only wait ~1us, not the full chain
                    for c0, cw in ((0, 128), (128, TOK - 128)):
                        for h in range(HPC):
                            recl = wk.tile([128, cw], F32,
                                           tag=f"recl{cw}")
                            nc.vector.reciprocal_approx_fast(
                                recl, lpss[h][:, c0:c0 + cw])
                            nc.vector.tensor_tensor(
                                ot[:, h, q0 + c0:q0 + c0 + cw],
                                avs[h][:, c0:c0 + cw], recl, op=OP.mult)
                    # enqueue this qi's out-projection groups; they drain
                    # through the next query tile's (or next batch's first)
                    # attention loop.  Cast-engine split: the drain context
                    # for qi 3 and 0 is exp-light (few ki steps / next
                    # batch's qi=0), so ACT takes more copies there.
                    d_qi = (qi + 1) % NQI
                    n_act = 2 if d_qi == 0 else 1
                    for qs in range(qi * NDC, (qi + 1) * NDC):
                        ysb = ysp.tile([128, D], BF16, tag="ysb",
                                       name=f"ysb_{b0}_{qs}")
                        for do in range(NDO):
                            eng = "act" if do < n_act else "dve"
                            equeue.append((b0, ot, qs, do, ysb, eng))
                    if qi == NQI - 2 and b < B - 1:
                        # prefetch next batch's first x tile during this
                        # B-phase so the batch boundary never waits on DMA
                        xt_pre = xp.tile([128, NKO, TOK], BF16, tag="xt")
                        nc.sync.dma_start(out=xt_pre,
                                          in_=xT[:, (b + 1) * S // TOK])

            while equeue:
                emit_group(equeue.pop(0))

    nc.compile()
    return nc


def _host_prep(x, rope_cos, rope_sin, Wqkv, bqkv, Wout, B, S, D, H, n_cores):
    """Build per-core input maps (bf16 data, fp32 biases)."""
    import ml_dtypes
    BF = ml_dtypes.bfloat16

    T = B * S
    HPC = H // n_cores
    orig = _perm_orig_of_p()
    quad_j = np.arange(DH)
    jmod = quad_j % 32
    i_of_p = (quad_j // 32) * 16 + (jmod % 16)
    sign = np.where(jmod < 16, -1.0, 1.0).astype(np.float32)

    # [p, token-tile, ko, tok]: every x-tile DMA slice is fully contiguous
    # per partition (16KB lines)
    NTT = T // 512
    xT = np.ascontiguousarray(
        x.reshape(NTT, 512, D // 128, 128).transpose(3, 0, 2, 1).astype(BF))
    cosP = np.ascontiguousarray(rope_cos[:, i_of_p].T.astype(BF))
    sinP = np.ascontiguousarray((rope_sin[:, i_of_p] * sign).T.astype(BF))

    pl = np.arange(128)[:, None]
    ql = np.arange(128)[None, :]
    # additive causal mask for the 128-wide diagonal block: 0 at/below the
    # diagonal (key p <= query q), -1e9 above (exp -> exact 0)
    madd = np.ascontiguousarray(
        np.where(pl <= ql, 0.0, -1e9).astype(BF))  # [128, 128]
    ident = np.ascontiguousarray(np.eye(128, dtype=np.float32).astype(BF))

    ones = np.ones((128, 128), dtype=BF)

    NKO = D // 128
    in_maps = []
    for c in range(n_cores):
        heads = [c * HPC + i for i in range(HPC)]
        wq = [Wqkv[:, h * DH + orig] for h in heads]
        wk = [Wqkv[:, H * DH + h * DH + orig] for h in heads]
        wv = [Wqkv[:, 2 * H * DH + h * DH:2 * H * DH + (h + 1) * DH]
              for h in heads]
        w_c = np.concatenate(wq + wk + wv, axis=1)  # [D, WCOLS]
        CB = w_c.shape[1] // 128
        # [p, cb, ko, 128]: each column-block slice is a contiguous
        # 4KB-per-partition DMA
        w4 = np.ascontiguousarray(
            w_c.reshape(NKO, 128, CB, 128).transpose(1, 2, 0, 3).astype(BF))
        wout_c = np.ascontiguousarray(
            Wout[c * HPC * DH:(c + 1) * HPC * DH, :].astype(BF))
        qb_cols = ([bqkv[h * DH + orig] for h in heads] +
                   [bqkv[H * DH + h * DH + orig] for h in heads])
        qb = np.ascontiguousarray(np.stack(qb_cols, axis=1).astype(np.float32))
        vb_flat = np.concatenate(
            [bqkv[2 * H * DH + h * DH:2 * H * DH + (h + 1) * DH]
             for h in heads])
        vb = np.ascontiguousarray(
            np.broadcast_to(vb_flat[None, :], (128, HPC * DH)).astype(
                np.float32))
        in_maps.append({
            "xT": xT, "w4": w4, "wout": wout_c, "cosP": cosP, "sinP": sinP,
            "madd": madd, "ident": ident, "ones": ones, "qb": qb, "vb": vb,
        })
    return in_maps


def _run(x, rope_cos, rope_sin, Wqkv, bqkv, Wout, bout,
         B, S, D, H, n_cores, trace=False):
    _ensure_imports()
    from concourse.bass_utils import run_bass_kernel_spmd

    HPC = H // n_cores
    import time as _time
    _t0 = _time.time()
    nc = _build_program(B, S, D, HPC)
    print(f"[kernel] build+compile wall: {_time.time() - _t0:.1f}s", flush=True)
    in_maps = _host_prep(np.asarray(x, dtype=np.float32),
                         np.asarray(rope_cos, dtype=np.float32),
                         np.asarray(rope_sin, dtype=np.float32),
                         np.asarray(Wqkv, dtype=np.float32),
                         np.asarray(bqkv, dtype=np.float32),
                         np.asarray(Wout, dtype=np.float32),
                         B, S, D, H, n_cores)
    _t0 = _time.time()
    res = run_bass_kernel_spmd(nc, in_maps, list(range(n_cores)), trace=trace)
    print(f"[kernel] spmd run wall: {_time.time() - _t0:.1f}s", flush=True)
    y = res.results[0]["y"].astype(np.float64)
    for i in range(1, n_cores):
        y += res.results[i]["y"]
    y += np.asarray(bout, dtype=np.float64)[None, :]
    out = y.astype(np.float32).reshape(B, S, D)
    return out, res


def kernel(x, rope_cos, rope_sin, Wqkv, bqkv, Wout, bout):
    out, _ = _run(x, rope_cos, rope_sin, Wqkv, bqkv, Wout, bout,
                  B=4, S=2048, D=2048, H=16, n_cores=8)
    return out

